# revision 56
# baseline (speedup 1.0000x reference)
"""GQA attention block (QKV proj + RoPE + causal attention + o_proj),
tensor-parallel over heads across 8 TRN2 NeuronCores.

Sharding: core c owns q heads [4c, 4c+4) (512 q dims), kv head c
(128 kv dims), and wo columns [512c, 512c+512). Each core computes a
full-shape partial of the output projection; the host sums the 8
partials (the "all-reduce") and transposes back.

Layout convention on device: activations are kept feature-major
([dim, seq]) so every matmul contracts over the partition axis with
no transposes:
  QT/KT [d, s]  ->  scores^T [ks, qs] = KT_tile^T . QT   (lhsT=KT, rhs=QT)
  softmax over ks = partition axis: exp on ACT, denominator via
  ones-matmul on PE, division folded into the PV output scaling
  PV: OT [dv, qs] = V_nat^T . P                           (lhsT=V, rhs=P)
  o_proj: outT [e, s] = woT^T . OT                        (lhsT=woT, rhs=OT)

Precision plan: the dense GEMMs (QKV proj, o_proj) run as fp8e4
DoubleRow matmuls (2 K-tiles contracted per instruction at 0.5
cycles/row) with a hi/lo residual split of both operands and the
three significant cross terms (hi.hi + lo.hi + hi.lo) accumulated in
fp32 PSUM - ~1.5e-3 relative error at 0.75x the bf16/fp32r cycle
cost. Weights are pre-scaled by 64 (power of two, folded back into
the PSUM->SBUF copy scale) so their hi/lo parts stay in fp8e4 normal
range; the attention output is pre-scaled by 16 (via the den "ones"
stationary = 1/16) for the same reason. q/k/v/P/scores run in bf16
(same PE rate as fp32r, half the SBUF/DMA). Output partials are
stored bf16 and summed on host.

Scheduling: weights arrive host-pretiled partition-major in a few
large staged DMAs (the HWDGE dispatch ring costs 625ns per DMA
instruction); x arrives as host-split fp8 hi/lo streams. Attention
runs qt descending with a 2-unit score lookahead and cross-head
score pre-issue; the latency-bound qt<=2 iterations interleave
o_proj column-block emissions between units as PE filler (gated so
an o_proj chunk is only emitted after the attention chunk feeding it
is complete), with the remaining o_proj drained at deeper PSUM
rotation afterwards.
"""

import sys
from contextlib import ExitStack

import numpy as np
import ml_dtypes

for _p in ("/opt/trn_rl_repo", "/opt/trn_rl_repo/concourse"):
    if _p not in sys.path:
        sys.path.insert(0, _p)

import concourse.bacc as bacc
import concourse.bass as bass
import concourse.tile as tile
from concourse import mybir
from concourse.bass_utils import run_bass_kernel_spmd

F32 = mybir.dt.float32
F32R = mybir.dt.float32r
BF16 = mybir.dt.bfloat16
F8 = mybir.dt.float8e4
E4NP = ml_dtypes.float8_e4m3
BF16NP = ml_dtypes.bfloat16
AF = mybir.ActivationFunctionType
DR = mybir.MatmulPerfMode.DoubleRow

DIM = 4096
SEQ = 2048
HD = 128          # head dim
NCORES = 8
HQ = 4            # q heads per core
DQ = HQ * HD      # 512 q dims per core
NKT = DIM // HD   # 32 contraction tiles
NPAIR = NKT // 2  # 16 DoubleRow k-tile pairs
SQT = SEQ // 512  # 4 seq chunks of 512
INV_SQRT_HD = 1.0 / np.sqrt(np.float32(HD))
EXP_BIAS = -12.0  # constant shift inside exp; cancels in softmax
WSCALE = 64.0     # weight pre-scale so fp8 hi/lo stays in normal range
OTSCALE = 16.0    # attention-output pre-scale for its fp8 hi/lo split

TRACE = False
LAST_RESULT = None

_cache = {}


def _build(mask_mode):
    """mask_mode: 'zeros' | 'causal' | 'general'."""
    nc = bacc.Bacc("TRN2", target_bir_lowering=False)
    xh = nc.dram_tensor("xh", [DIM, SEQ], F8, kind="ExternalInput")
    xl = nc.dram_tensor("xl", [DIM, SEQ], F8, kind="ExternalInput")
    # weights arrive pre-tiled partition-major: [p, (ktile m)]
    wqh = nc.dram_tensor("wqh", [HD, NKT * DQ], F8, kind="ExternalInput")
    wql = nc.dram_tensor("wql", [HD, NKT * DQ], F8, kind="ExternalInput")
    wkh = nc.dram_tensor("wkh", [HD, NKT * HD], F8, kind="ExternalInput")
    wkl = nc.dram_tensor("wkl", [HD, NKT * HD], F8, kind="ExternalInput")
    wvh = nc.dram_tensor("wvh", [HD, NKT * HD], F8, kind="ExternalInput")
    wvl = nc.dram_tensor("wvl", [HD, NKT * HD], F8, kind="ExternalInput")
    woh = nc.dram_tensor("woh", [HD, HQ * DIM], F8, kind="ExternalInput")
    wol = nc.dram_tensor("wol", [HD, HQ * DIM], F8, kind="ExternalInput")
    cs = nc.dram_tensor("cs", [HD, SEQ], BF16, kind="ExternalInput")
    sn = nc.dram_tensor("sn", [HD, SEQ], BF16, kind="ExternalInput")
    psw = nc.dram_tensor("psw", [HD, HD], F32R, kind="ExternalInput")
    idn = nc.dram_tensor("idn", [HD, HD], BF16, kind="ExternalInput")
    mkt = None
    if mask_mode == "causal":
        # 4 relative diagonal-tile masks (pattern repeats for every qt)
        mkt = nc.dram_tensor("mkt", [HD, 4 * 512], BF16, kind="ExternalInput")
    elif mask_mode == "general":
        mkt = nc.dram_tensor("mkt", [SEQ, SEQ], F32, kind="ExternalInput")
    outt = nc.dram_tensor("outt", [DIM, SEQ], BF16, kind="ExternalOutput")

    QSCALE = float(INV_SQRT_HD / WSCALE)
    KSCALE = float(1.0 / WSCALE)
    OSCALE = float(1.0 / (WSCALE * OTSCALE))

    with ExitStack() as ctx:
        tc = ctx.enter_context(tile.TileContext(nc))

        # ---- persistent pools ----
        const = ctx.enter_context(tc.tile_pool(name="const", bufs=1))
        ones_f32 = const.tile([HD, HD], F32, tag="ones32")
        # den is accumulated pre-divided by OTSCALE so inv = OTSCALE/den and
        # the attention output is scaled into fp8-friendly range for the
        # o_proj hi/lo split; the final output copy divides it back out.
        nc.vector.memset(ones_f32[:], 1.0 / OTSCALE)
        ones_sb = const.tile([HD, HD], BF16, tag="ones")
        nc.scalar.activation(ones_sb[:], ones_f32[:], AF.Copy)
        ebias = const.tile([HD, 1], F32, tag="ebias")
        nc.vector.memset(ebias[:], EXP_BIAS)

        qkvpool = ctx.enter_context(tc.tile_pool(name="qkv", bufs=1))
        # per-chunk tiles so attention reads only depend on the chunks they
        # actually touch (no false whole-tile hazards on the last chunk)
        qrope = [[qkvpool.tile([HD, 512], BF16, tag=f"qr{h}_{c}",
                               name=f"qr{h}_{c}") for c in range(SQT)]
                 for h in range(HQ)]
        krope = [qkvpool.tile([HD, 512], BF16, tag=f"kr{c}", name=f"kr{c}")
                 for c in range(SQT)]
        vnat = [qkvpool.tile([HD, 512], BF16, tag=f"vn{c}", name=f"vn{c}")
                for c in range(SQT)]

        def kr_at(kst):
            return krope[kst // 4][:, (kst % 4) * HD:(kst % 4 + 1) * HD]

        def vn_at(kst):
            return vnat[kst // 4][:, (kst % 4) * HD:(kst % 4 + 1) * HD]

        # ---- phase 1: QKV projection (fp8 DoubleRow 3-term) + RoPE ----
        with ExitStack() as p1:
            wpool = p1.enter_context(tc.tile_pool(name="w1", bufs=1))
            wq_sb = [wpool.tile([HD, NKT, DQ], F8, tag=f"wq{t}", name=f"wq{t}")
                     for t in range(2)]
            wk_sb = [wpool.tile([HD, NKT, HD], F8, tag=f"wk{t}", name=f"wk{t}")
                     for t in range(2)]
            wv_sb = [wpool.tile([HD, NKT, HD], F8, tag=f"wv{t}", name=f"wv{t}")
                     for t in range(2)]
            cs_sb = wpool.tile([HD, SEQ], BF16, tag="cs")
            sn_sb = wpool.tile([HD, SEQ], BF16, tag="sn")
            psw_sb = wpool.tile([HD, HD], F32R, tag="psw")
            idn_sb = wpool.tile([HD, HD], BF16, tag="idn")

            def _wslice(dst3d, dram, m, lo, hi):
                # ktiles [lo, hi) of a [p, (k m)] pretiled weight tensor
                nc.sync.dma_start(
                    dst3d[:, lo:hi, :],
                    dram[:, lo * m:hi * m].rearrange("p (k m) -> p k m",
                                                     k=hi - lo))

            def emit_w_dma(kg):
                # batched staging: kg==0 -> ktiles 0-4 of everything (small,
                # fast first batch); kg==1 -> ktiles 4-16; kg==3 -> 16-32.
                # One DMA instruction per tensor per batch keeps the HWDGE
                # dispatch ring (625ns/instruction) off the critical path.
                def _wbatch(lo, hi):
                    for t in range(2):
                        if not (t == 0 and lo == 0):
                            _wslice(wq_sb[t], (wqh, wql)[t], DQ, lo, hi)
                        _wslice(wk_sb[t], (wkh, wkl)[t], HD, lo, hi)
                        _wslice(wv_sb[t], (wvh, wvl)[t], HD, lo, hi)

                if kg == 0:
                    _wbatch(0, 4)
                elif kg == 1:
                    _wbatch(4, 16)
                elif kg == 3:
                    _wbatch(16, NKT)
                elif kg == 5:
                    nc.sync.dma_start(psw_sb[:], psw[:])
                    nc.sync.dma_start(idn_sb[:], idn[:])
                    nc.sync.dma_start(cs_sb[:], cs[:])
                    nc.sync.dma_start(sn_sb[:], sn[:])

            xpool = p1.enter_context(tc.tile_pool(name="xstream", bufs=3))
            rtmp = p1.enter_context(tc.tile_pool(name="rtmp", bufs=2))
            ps1 = p1.enter_context(tc.tile_pool(name="ps1", bufs=1, space="PSUM"))
            ps1q = p1.enter_context(tc.tile_pool(name="ps1q", bufs=4, space="PSUM"))
            ps1m = p1.enter_context(tc.tile_pool(name="ps1m", bufs=1, space="PSUM"))

            for st in range(SQT):
                ss = slice(st * 512, (st + 1) * 512)
                pq = [ps1q.tile([HD, 512], F32, tag="pq", name=f"pq{i}")
                      for i in range(HQ)]
                pk = ps1.tile([HD, 512], F32, tag="pk")
                pv = ps1.tile([HD, 512], F32, tag="pv")
                for kg in range(NKT // 4):
                    if st == 0 and kg == 0:
                        _wslice(wq_sb[0], wqh, DQ, 0, 4)
                    xq8 = [xpool.tile([HD, 4, 512], F8, tag=f"xt{t}",
                                      name=f"xt{t}") for t in range(2)]
                    nc.sync.dma_start(
                        xq8[0][:],
                        xh[kg * 4 * HD:(kg + 1) * 4 * HD, ss]
                        .rearrange("(k p) m -> p k m", p=HD))
                    nc.sync.dma_start(
                        xq8[1][:],
                        xl[kg * 4 * HD:(kg + 1) * 4 * HD, ss]
                        .rearrange("(k p) m -> p k m", p=HD))
                    if st == 0:
                        emit_w_dma(kg)
                    for j in range(2):
                        pp = kg * 2 + j           # global pair index
                        kpair = slice(2 * pp, 2 * pp + 2)
                        xsl = [x8[:, 2 * j:2 * j + 2, :] for x8 in xq8]
                        first = (kg == 0 and j == 0)
                        last = (kg == NKT // 4 - 1 and j == 1)
                        # 3 terms: hi.hi, lo.hi, hi.lo
                        terms = ((0, 0), (1, 0), (0, 1))
                        for ti, (wi, xi) in enumerate(terms):
                            fl = dict(start=(first and ti == 0),
                                      stop=(last and ti == len(terms) - 1))
                            for mt in range(HQ):
                                msl = slice(mt * HD, (mt + 1) * HD)
                                nc.tensor.matmul(
                                    pq[mt][:], wq_sb[wi][:, kpair, msl],
                                    xsl[xi], perf_mode=DR, **fl)
                            nc.tensor.matmul(
                                pk[:], wk_sb[wi][:, kpair, :], xsl[xi],
                                perf_mode=DR, **fl)
                            nc.tensor.matmul(
                                pv[:], wv_sb[wi][:, kpair, :], xsl[xi],
                                perf_mode=DR, **fl)

                # RoPE: q head 0, then k (attention (h=0,qt) needs both
                # first), then remaining q heads. 1/sqrt(hd) and the fp8
                # descale are folded into the PSUM copy.
                def rope_one(src_ps, dst, dst_sl, scale, on_act):
                    raw = rtmp.tile([HD, 512], F32R, tag="qraw")
                    if on_act:
                        nc.scalar.activation(raw[:], src_ps[:], AF.Copy,
                                             scale=scale)
                    else:
                        nc.vector.tensor_scalar_mul(raw[:], src_ps[:], scale)
                    swp = ps1m.tile([HD, 512], F32, tag="psw")
                    nc.tensor.matmul(swp[:], psw_sb[:], raw[:],
                                     start=True, stop=True)
                    t1 = rtmp.tile([HD, 512], F32, tag="t1", bufs=1)
                    nc.vector.tensor_mul(t1[:], raw[:], cs_sb[:, ss])
                    t2 = rtmp.tile([HD, 512], F32, tag="t2", bufs=1)
                    nc.vector.tensor_mul(t2[:], swp[:], sn_sb[:, ss])
                    nc.vector.tensor_add(dst[:, dst_sl], t1[:], t2[:])

                for mt in range(HQ):
                    rope_one(pq[mt], qrope[mt][st], slice(0, 512), QSCALE,
                             mt % 2 == 0)
                rope_one(pk, krope[st], slice(0, 512), KSCALE, True)
                # v: descale + bf16, then transpose to [seq, dv] blocks
                vraw = rtmp.tile([HD, 512], BF16, tag="vraw", bufs=1)
                nc.scalar.activation(vraw[:], pv[:], AF.Copy, scale=KSCALE)
                for j in range(4):
                    vt = ps1m.tile([HD, HD], BF16, tag="pvt")
                    nc.tensor.transpose(vt[:], vraw[:, j * HD:(j + 1) * HD],
                                        idn_sb[:])
                    if j % 2 == 0:
                        nc.scalar.activation(
                            vnat[st][:, j * HD:(j + 1) * HD], vt[:], AF.Copy)
                    else:
                        nc.vector.tensor_copy(
                            vnat[st][:, j * HD:(j + 1) * HD], vt[:])

        # ---- phase 2: attention;  phase 3: output projection ----
        with ExitStack() as p2:
            wopool = p2.enter_context(tc.tile_pool(name="wo", bufs=1))
            wo_sb = [wopool.tile([HD, HQ, DIM], F8, tag=f"wo{t}", name=f"wo{t}")
                     for t in range(2)]
            wo_dma_emitted = [False]

            def emit_wo_dmas():
                if not wo_dma_emitted[0]:
                    wo_dma_emitted[0] = True
                    nc.sync.dma_start(
                        wo_sb[0][:], woh[:].rearrange("p (k m) -> p k m", k=HQ))
                    nc.sync.dma_start(
                        wo_sb[1][:], wol[:].rearrange("p (k m) -> p k m", k=HQ))

            otpool = p2.enter_context(tc.tile_pool(name="ot", bufs=1))
            # attention output per head, fp8 hi/lo split for the o_proj
            ot8 = [otpool.tile([HD, HQ, SEQ], F8, tag=f"ot8{t}", name=f"ot8{t}")
                   for t in range(2)]

            mpool = p2.enter_context(tc.tile_pool(name="mk", bufs=1))
            spool = p2.enter_context(tc.tile_pool(name="sp", bufs=2))

            mk_sb = None
            if mask_mode == "causal":
                mk_sb = mpool.tile([HD, 4, 512], BF16, tag="mkd")
                nc.sync.dma_start(
                    mk_sb[:], mkt[:].rearrange("p (k m) -> p k m", k=4))

            gen_masks = {}

            def emit_gen_masks(qt):
                qs = slice(qt * 512, (qt + 1) * 512)
                out = {}
                for kst in range(16):
                    m = mpool.tile([HD, 512], F32, tag=f"mk{kst}",
                                   name=f"mk{kst}")
                    nc.sync.dma_start(
                        m[:], mkt[kst * HD:(kst + 1) * HD, qs])
                    out[kst] = m
                return out

            def npair_of(qt):
                return 2 * qt if mask_mode == "causal" else 8

            def nunit_of(qt):
                return npair_of(qt) + (4 if mask_mode == "causal" else 0)

            def issue_scores_for(qt, h, i, ps2):
                npair = npair_of(qt)
                qs = slice(qt * 512, (qt + 1) * 512)
                sp = ps2.tile([HD, 1024], F32, tag="pst")
                if i < npair:
                    for u in range(2):
                        kst = 2 * i + u
                        nc.tensor.matmul(
                            sp[:, u * 512:(u + 1) * 512],
                            kr_at(kst),
                            qrope[h][qt][:],
                            start=True, stop=True)
                else:
                    # diagonal tile, columns < c0 fully masked
                    r = i - npair
                    kst = 4 * qt + r
                    c0 = r * HD
                    nc.tensor.matmul(
                        sp[:, c0:512],
                        kr_at(kst),
                        qrope[h][qt][:, c0:512],
                        start=True, stop=True)
                return sp

            def issue_exp_for(qt, i, sp, ppool):
                npair = npair_of(qt)
                pb = ppool.tile([HD, 1024], BF16, tag="pexp")
                if i < npair:
                    if mask_mode == "general":
                        tmp = ppool.tile([HD, 1024], F32, tag="padd", bufs=2)
                        for u in range(2):
                            usl = slice(u * 512, (u + 1) * 512)
                            nc.vector.tensor_add(
                                tmp[:, usl], sp[:, usl],
                                gen_masks[qt][2 * i + u][:])
                        nc.scalar.activation(pb[:], tmp[:], AF.Exp,
                                             bias=ebias[:])
                    else:
                        nc.scalar.activation(pb[:], sp[:], AF.Exp,
                                             bias=ebias[:])
                else:
                    r = i - npair
                    c0 = r * HD
                    tmp = ppool.tile([HD, 1024], F32, tag="padd", bufs=2)
                    nc.vector.tensor_add(
                        tmp[:, c0:512], sp[:, c0:512], mk_sb[:, r, c0:])
                    nc.scalar.activation(pb[:, c0:512], tmp[:, c0:512],
                                         AF.Exp, bias=ebias[:])
                return pb

            pre_store = {}

            def attn_iter(qt, h, ps2, ps2a, ppool, filler,
                          prescore_next=None, lookahead=2):
                qs = slice(qt * 512, (qt + 1) * 512)
                npair = npair_of(qt)
                nunit = nunit_of(qt)
                sps = [None] * nunit
                pbs = [None] * nunit

                pre = pre_store.pop((qt, h), None)
                if pre is not None:
                    sps[0], sps[1] = pre
                    if lookahead > 2 and nunit > 2:
                        sps[2] = issue_scores_for(qt, h, 2, ps2)
                else:
                    for j in range(min(lookahead, nunit)):
                        sps[j] = issue_scores_for(qt, h, j, ps2)

                den = ps2a.tile([HD, 512], F32, tag="pden")
                otp = ps2a.tile([HD, 512], F32, tag="pot")
                for i in range(nunit):
                    if lookahead + i < nunit and sps[lookahead + i] is None:
                        sps[lookahead + i] = issue_scores_for(
                            qt, h, lookahead + i, ps2)
                    pbs[i] = issue_exp_for(qt, i, sps[i], ppool)
                    fl_last = (i == nunit - 1)
                    if i < npair:
                        for u in range(2):
                            kst = 2 * i + u
                            fl = dict(
                                start=(i == 0 and u == 0),
                                stop=(fl_last and u == 1))
                            pr = pbs[i][:, u * 512:(u + 1) * 512]
                            nc.tensor.matmul(
                                den[:], ones_sb[:], pr, **fl)
                            nc.tensor.matmul(
                                otp[:], vn_at(kst), pr, **fl)
                    else:
                        r = i - npair
                        kst = 4 * qt + r
                        c0 = r * HD
                        fl = dict(start=(i == 0), stop=fl_last)
                        pr = pbs[i][:, c0:512]
                        nc.tensor.matmul(
                            den[:, c0:], ones_sb[:], pr, **fl)
                        nc.tensor.matmul(
                            otp[:, c0:], vn_at(kst), pr, **fl)
                    if fl_last and prescore_next is not None:
                        # pre-issue the next iteration's first two score
                        # units so its exp pipeline starts before this
                        # iteration's DVE drain
                        qn, hn = prescore_next
                        pre_store[(qn, hn)] = (
                            issue_scores_for(qn, hn, 0, ps2),
                            issue_scores_for(qn, hn, 1, ps2))
                    if filler is not None:
                        filler()
                inv = spool.tile([HD, 512], F32, tag="inv")
                nc.vector.reciprocal(inv[:], den[:])
                ots = spool.tile([HD, 512], F32, tag="ots")
                nc.vector.tensor_mul(ots[:], otp[:], inv[:])
                # fp8 hi/lo split of the attention output
                nc.scalar.activation(ot8[0][:, h, qs], ots[:], AF.Copy)
                nc.vector.tensor_sub(ot8[1][:, h, qs], ots[:],
                                     ot8[0][:, h, qs])
                if filler is not None:
                    filler()

            # ---- phase 3 emitter: o_proj (fp8 DoubleRow 3-term), one
            # [128,512] column block per generator step so it can be
            # interleaved into the attention tail as PE filler work ----
            OTERMS = ((0, 0), (1, 0), (0, 1))

            def oproj_units(sts, ps3, opool):
                for st in sts:
                    ss = slice(st * 512, (st + 1) * 512)
                    for eg in range(DIM // HD // 4):
                        last_grp = (st == 0 and eg == DIM // HD // 4 - 1)
                        ocp = opool.tile([HD, 4, 512], BF16, tag="ocp")
                        for ej in range(4):
                            et = eg * 4 + ej
                            esl = slice(et * HD, (et + 1) * HD)
                            po = ps3.tile([HD, 512], F32, tag="po")
                            for pi in range(2):
                                hpair = slice(2 * pi, 2 * pi + 2)
                                for ti, (wi, oi) in enumerate(OTERMS):
                                    nc.tensor.matmul(
                                        po[:],
                                        wo_sb[wi][:, hpair, esl],
                                        ot8[oi][:, hpair, ss],
                                        perf_mode=DR,
                                        start=(pi == 0 and ti == 0),
                                        stop=(pi == 1 and ti == 2),
                                    )
                            osl = ocp[:, ej, :]
                            if ej % 2 == 0:
                                nc.scalar.activation(osl, po[:], AF.Copy,
                                                     scale=OSCALE)
                            else:
                                nc.vector.tensor_scalar_mul(osl, po[:], OSCALE)
                            if last_grp:
                                # final tiles: store per-slice so the last
                                # DMA isn't gated on all four copies
                                nc.sync.dma_start(
                                    outt[et * HD:(et + 1) * HD, ss], osl)
                            yield
                        if not last_grp:
                            nc.sync.dma_start(
                                outt[eg * 4 * HD:(eg + 1) * 4 * HD, ss]
                                .rearrange("(e p) m -> p e m", p=HD),
                                ocp[:])

            if mask_mode == "causal":
                with ExitStack() as patt:
                    ppool = patt.enter_context(tc.tile_pool(name="pp", bufs=4))
                    ps2 = patt.enter_context(
                        tc.tile_pool(name="ps2", bufs=3, space="PSUM"))
                    ps2a = patt.enter_context(
                        tc.tile_pool(name="ps2a", bufs=1, space="PSUM"))
                    emit_wo_dmas()
                    for h in range(HQ):
                        nxt = (3, h + 1) if h + 1 < HQ else None
                        attn_iter(3, h, ps2, ps2a, ppool, None,
                                  prescore_next=nxt)
                # tail: interleave o_proj units into the latency-bound
                # qt=1/qt=0 iterations
                with ExitStack() as ptail:
                    ppool2 = ptail.enter_context(
                        tc.tile_pool(name="pp2", bufs=6))
                    ps2t = ptail.enter_context(
                        tc.tile_pool(name="ps2t", bufs=2, space="PSUM"))
                    ps2a2 = ptail.enter_context(
                        tc.tile_pool(name="ps2a2", bufs=1, space="PSUM"))
                    ps3 = ptail.enter_context(
                        tc.tile_pool(name="ps3", bufs=2, space="PSUM"))
                    opool = ptail.enter_context(
                        tc.tile_pool(name="ostage", bufs=3))
                    gen = oproj_units((3, 2), ps3, opool)
                    # st=3 units (32) are ready once qt=3 is done; st=2
                    # units must wait until all of qt=2 has been emitted
                    pulled = [0]
                    limit = [32]
                    _done = object()

                    def filler_gen():
                        if pulled[0] < limit[0]:
                            if next(gen, _done) is not _done:
                                pulled[0] += 1

                    seq = [(qt, h) for qt in (2, 1, 0) for h in range(HQ)]
                    for n, (qt, h) in enumerate(seq[:8]):
                        attn_iter(qt, h, ps2t, ps2a2, ppool2, filler_gen,
                                  prescore_next=seq[n + 1])
                        if (qt, h) == (2, HQ - 1):
                            limit[0] = 64
                    gen2 = oproj_units((1,), ps3, opool)

                    def filler_tail():
                        if next(gen, _done) is _done:
                            next(gen2, None)

                    for h in range(HQ):
                        nxt = (0, h + 1) if h + 1 < HQ else None
                        attn_iter(0, h, ps2t, ps2a2, ppool2, filler_tail,
                                  prescore_next=nxt)
                    for _ in gen:
                        pass
                    for _ in gen2:
                        pass
                # bulk o_proj drain with deep PSUM rotation
                with ExitStack() as p3d:
                    ps3d = p3d.enter_context(
                        tc.tile_pool(name="ps3d", bufs=4, space="PSUM"))
                    opool2 = p3d.enter_context(
                        tc.tile_pool(name="ostage2", bufs=3))
                    for _ in oproj_units((0,), ps3d, opool2):
                        pass
            else:
                with ExitStack() as patt:
                    ppool = patt.enter_context(tc.tile_pool(name="pp", bufs=4))
                    ps2 = patt.enter_context(
                        tc.tile_pool(name="ps2", bufs=3, space="PSUM"))
                    ps2a = patt.enter_context(
                        tc.tile_pool(name="ps2a", bufs=1, space="PSUM"))
                    emit_wo_dmas()
                    for qt in range(SQT - 1, -1, -1):
                        if mask_mode == "general" and qt not in gen_masks:
                            gen_masks[qt] = emit_gen_masks(qt)
                        for h in range(HQ):
                            attn_iter(qt, h, ps2, ps2a, ppool, None)
                    pre_store.clear()
                with ExitStack() as p3:
                    ps3 = p3.enter_context(
                        tc.tile_pool(name="ps3", bufs=4, space="PSUM"))
                    opool = p3.enter_context(
                        tc.tile_pool(name="ostage", bufs=3))
                    for _ in oproj_units((3, 2, 1, 0), ps3, opool):
                        pass

    nc.compile()
    return nc


def _split8(a, scale=1.0):
    s = np.clip(a * np.float32(scale), -224.0, 224.0)
    hi = s.astype(E4NP)
    lo = np.clip(s - hi.astype(np.float32), -224.0, 224.0).astype(E4NP)
    return np.ascontiguousarray(hi), np.ascontiguousarray(lo)


def _prep_consts(freqs_cos, freqs_sin):
    cos = np.asarray(freqs_cos, dtype=np.float32)
    sin = np.asarray(freqs_sin, dtype=np.float32)
    C = np.empty((HD, SEQ), np.float32)
    S = np.empty((HD, SEQ), np.float32)
    C[0::2] = cos.T
    C[1::2] = cos.T
    S[0::2] = -sin.T
    S[1::2] = sin.T
    psw = np.zeros((HD, HD), np.float32)
    j = np.arange(0, HD, 2)
    psw[j + 1, j] = 1.0
    psw[j, j + 1] = 1.0
    idn = np.eye(HD, dtype=np.float32).astype(BF16NP)
    return C, S, psw, idn


def _mask_mode(mask):
    if not mask.any():
        return "zeros"
    neg = mask.min()
    tril = np.tril(np.ones((SEQ, SEQ), dtype=bool))
    if neg <= -1e8 and not mask[tril].any() and np.all(mask[~tril] == neg):
        return "causal"
    return "general"


def kernel(x, wq, wk, wv, wo, freqs_cos, freqs_sin, mask, start_pos):
    global LAST_RESULT
    assert int(start_pos) == 0, "kernel hardcodes start_pos=0 (full prefill)"
    x = np.asarray(x, dtype=np.float32)
    wq = np.asarray(wq, dtype=np.float32)
    wk = np.asarray(wk, dtype=np.float32)
    wv = np.asarray(wv, dtype=np.float32)
    wo = np.asarray(wo, dtype=np.float32)
    mask = np.asarray(mask, dtype=np.float32)

    mode = _mask_mode(mask)
    if mode not in _cache:
        _cache[mode] = _build(mode)
    nc = _cache[mode]

    xt = np.ascontiguousarray(x.reshape(SEQ, DIM).T)
    xh8, xl8 = _split8(xt)
    C, S, psw, idn = _prep_consts(freqs_cos, freqs_sin)
    mkt = None
    if mode == "causal":
        # 4 relative diagonal tile masks: tile r is mask.T[r*128:(r+1)*128,
        # 0:512] (the pattern depends only on kst - 4*qt)
        mt = np.ascontiguousarray(mask.T[:512, :512])
        mkt = np.concatenate([mt[r * HD:(r + 1) * HD, :] for r in range(4)],
                             axis=1)
        mkt = np.ascontiguousarray(mkt).astype(BF16NP)
    elif mode == "general":
        mkt = np.ascontiguousarray(mask.T)

    def _ptile(a, m):
        # [DIM_contract, m] -> partition-major [128, (ktile m)]
        k = a.shape[0] // HD
        return np.ascontiguousarray(
            a.reshape(k, HD, m).transpose(1, 0, 2).reshape(HD, k * m))

    in_maps = []
    for c in range(NCORES):
        wqh8, wql8 = _split8(wq[c * DQ:(c + 1) * DQ, :].T, WSCALE)
        wkh8, wkl8 = _split8(wk[c * HD:(c + 1) * HD, :].T, WSCALE)
        wvh8, wvl8 = _split8(wv[c * HD:(c + 1) * HD, :].T, WSCALE)
        woh8, wol8 = _split8(wo[:, c * DQ:(c + 1) * DQ].T, WSCALE)
        wqh8, wql8 = _ptile(wqh8, DQ), _ptile(wql8, DQ)
        wkh8, wkl8 = _ptile(wkh8, HD), _ptile(wkl8, HD)
        wvh8, wvl8 = _ptile(wvh8, HD), _ptile(wvl8, HD)
        woh8, wol8 = _ptile(woh8, DIM), _ptile(wol8, DIM)
        m = {
            "xh": xh8, "xl": xl8,
            "wqh": wqh8, "wql": wql8,
            "wkh": wkh8, "wkl": wkl8,
            "wvh": wvh8, "wvl": wvl8,
            "woh": woh8, "wol": wol8,
            "cs": C.astype(BF16NP), "sn": S.astype(BF16NP),
            "psw": psw, "idn": idn,
        }
        if mkt is not None:
            m["mkt"] = mkt
        in_maps.append(m)

    res = run_bass_kernel_spmd(nc, in_maps, core_ids=list(range(NCORES)),
                               trace=TRACE)
    LAST_RESULT = res
    acc = np.zeros((DIM, SEQ), dtype=np.float64)
    for c in range(NCORES):
        acc += res.results[c]["outt"].astype(np.float64)
    return np.ascontiguousarray(acc.T).astype(np.float32).reshape(1, SEQ, DIM)


# revision 61
# speedup vs baseline: 1.0238x; 1.0238x over previous
"""GQA attention block (QKV proj + RoPE + causal attention + o_proj),
tensor-parallel over heads across 8 TRN2 NeuronCores.

Sharding: core c owns q heads [4c, 4c+4) (512 q dims), kv head c
(128 kv dims), and wo columns [512c, 512c+512). Each core computes a
full-shape partial of the output projection; the host sums the 8
partials (the "all-reduce") and transposes back.

Layout convention on device: activations are kept feature-major
([dim, seq]) so every matmul contracts over the partition axis with
no transposes:
  QT/KT [d, s]  ->  scores^T [ks, qs] = KT_tile^T . QT   (lhsT=KT, rhs=QT)
  softmax over ks = partition axis: exp on ACT, denominator via
  ones-matmul on PE, division folded into the PV output scaling
  PV: OT [dv, qs] = V_nat^T . P                           (lhsT=V, rhs=P)
  o_proj: outT [e, s] = woT^T . OT                        (lhsT=woT, rhs=OT)

Precision plan: the dense GEMMs (QKV proj, o_proj) run as fp8e4
DoubleRow matmuls (2 K-tiles contracted per instruction at 0.5
cycles/row) with a hi/lo residual split of both operands and the
three significant cross terms (hi.hi + lo.hi + hi.lo) accumulated in
fp32 PSUM - ~1.5e-3 relative error at 0.75x the bf16/fp32r cycle
cost. Weights are pre-scaled by 64 (power of two, folded back into
the PSUM->SBUF copy scale) so their hi/lo parts stay in fp8e4 normal
range; the attention output is pre-scaled by 16 (via the den "ones"
stationary = 1/16) for the same reason. q/k/v/P/scores run in bf16
(same PE rate as fp32r, half the SBUF/DMA). Output partials are
stored bf16 and summed on host.

Scheduling: weights arrive host-pretiled partition-major in a few
large staged DMAs (the HWDGE dispatch ring costs 625ns per DMA
instruction); x arrives as host-split fp8 hi/lo streams. The first
two seq chunks stream x quads interleaved with the matmuls (the DMA
pipe is saturated by weight loading there); the last two hold the
full chunk of x resident (prefetched while DMA is otherwise idle)
and run their six output tiles sequentially, each immediately
followed by its RoPE, so the RoPE chains overlap the next tile's
matmuls and attention starts without waiting on a rope tail.
Attention runs qt descending with a 2-unit score lookahead and
cross-head score pre-issue; the latency-bound qt<=2 iterations
interleave o_proj column-block emissions between units as PE filler
(gated so an o_proj chunk is only emitted after the attention chunk
feeding it is complete), with the remaining o_proj drained at deeper
PSUM rotation afterwards.
"""

import sys
from contextlib import ExitStack

import numpy as np
import ml_dtypes

for _p in ("/opt/trn_rl_repo", "/opt/trn_rl_repo/concourse"):
    if _p not in sys.path:
        sys.path.insert(0, _p)

import concourse.bacc as bacc
import concourse.bass as bass
import concourse.tile as tile
from concourse import mybir
from concourse.bass_utils import run_bass_kernel_spmd

F32 = mybir.dt.float32
F32R = mybir.dt.float32r
BF16 = mybir.dt.bfloat16
F8 = mybir.dt.float8e4
E4NP = ml_dtypes.float8_e4m3
BF16NP = ml_dtypes.bfloat16
AF = mybir.ActivationFunctionType
DR = mybir.MatmulPerfMode.DoubleRow

DIM = 4096
SEQ = 2048
HD = 128          # head dim
NCORES = 8
HQ = 4            # q heads per core
DQ = HQ * HD      # 512 q dims per core
NKT = DIM // HD   # 32 contraction tiles
NPAIR = NKT // 2  # 16 DoubleRow k-tile pairs
SQT = SEQ // 512  # 4 seq chunks of 512
INV_SQRT_HD = 1.0 / np.sqrt(np.float32(HD))
EXP_BIAS = -12.0  # constant shift inside exp; cancels in softmax
WSCALE = 64.0     # weight pre-scale so fp8 hi/lo stays in normal range
OTSCALE = 16.0    # attention-output pre-scale for its fp8 hi/lo split

TRACE = False
LAST_RESULT = None

_cache = {}


def _build(mask_mode):
    """mask_mode: 'zeros' | 'causal' | 'general'."""
    nc = bacc.Bacc("TRN2", target_bir_lowering=False)
    xh = nc.dram_tensor("xh", [DIM, SEQ], F8, kind="ExternalInput")
    xl = nc.dram_tensor("xl", [DIM, SEQ], F8, kind="ExternalInput")
    # weights arrive pre-tiled partition-major: [p, (ktile m)]
    wqh = nc.dram_tensor("wqh", [HD, NKT * DQ], F8, kind="ExternalInput")
    wql = nc.dram_tensor("wql", [HD, NKT * DQ], F8, kind="ExternalInput")
    wkh = nc.dram_tensor("wkh", [HD, NKT * HD], F8, kind="ExternalInput")
    wkl = nc.dram_tensor("wkl", [HD, NKT * HD], F8, kind="ExternalInput")
    wvh = nc.dram_tensor("wvh", [HD, NKT * HD], F8, kind="ExternalInput")
    wvl = nc.dram_tensor("wvl", [HD, NKT * HD], F8, kind="ExternalInput")
    woh = nc.dram_tensor("woh", [HD, HQ * DIM], F8, kind="ExternalInput")
    wol = nc.dram_tensor("wol", [HD, HQ * DIM], F8, kind="ExternalInput")
    cs = nc.dram_tensor("cs", [HD, SEQ], BF16, kind="ExternalInput")
    sn = nc.dram_tensor("sn", [HD, SEQ], BF16, kind="ExternalInput")
    psw = nc.dram_tensor("psw", [HD, HD], F32R, kind="ExternalInput")
    idn = nc.dram_tensor("idn", [HD, HD], BF16, kind="ExternalInput")
    mkt = None
    if mask_mode == "causal":
        # 4 relative diagonal-tile masks (pattern repeats for every qt)
        mkt = nc.dram_tensor("mkt", [HD, 4 * 512], BF16, kind="ExternalInput")
    elif mask_mode == "general":
        mkt = nc.dram_tensor("mkt", [SEQ, SEQ], F32, kind="ExternalInput")
    outt = nc.dram_tensor("outt", [DIM, SEQ], BF16, kind="ExternalOutput")

    QSCALE = float(INV_SQRT_HD / WSCALE)
    KSCALE = float(1.0 / WSCALE)
    OSCALE = float(1.0 / (WSCALE * OTSCALE))

    with ExitStack() as ctx:
        tc = ctx.enter_context(tile.TileContext(nc))

        # ---- persistent pools ----
        const = ctx.enter_context(tc.tile_pool(name="const", bufs=1))
        ones_f32 = const.tile([HD, HD], F32, tag="ones32")
        # den is accumulated pre-divided by OTSCALE so inv = OTSCALE/den and
        # the attention output is scaled into fp8-friendly range for the
        # o_proj hi/lo split; the final output copy divides it back out.
        nc.vector.memset(ones_f32[:], 1.0 / OTSCALE)
        ones_sb = const.tile([HD, HD], BF16, tag="ones")
        nc.scalar.activation(ones_sb[:], ones_f32[:], AF.Copy)
        ebias = const.tile([HD, 1], F32, tag="ebias")
        nc.vector.memset(ebias[:], EXP_BIAS)

        qkvpool = ctx.enter_context(tc.tile_pool(name="qkv", bufs=1))
        # per-chunk tiles so attention reads only depend on the chunks they
        # actually touch (no false whole-tile hazards on the last chunk)
        qrope = [[qkvpool.tile([HD, 512], BF16, tag=f"qr{h}_{c}",
                               name=f"qr{h}_{c}") for c in range(SQT)]
                 for h in range(HQ)]
        krope = [qkvpool.tile([HD, 512], BF16, tag=f"kr{c}", name=f"kr{c}")
                 for c in range(SQT)]
        vnat = [qkvpool.tile([HD, 512], BF16, tag=f"vn{c}", name=f"vn{c}")
                for c in range(SQT)]

        def kr_at(kst):
            return krope[kst // 4][:, (kst % 4) * HD:(kst % 4 + 1) * HD]

        def vn_at(kst):
            return vnat[kst // 4][:, (kst % 4) * HD:(kst % 4 + 1) * HD]

        # ---- phase 1: QKV projection (fp8 DoubleRow 3-term) + RoPE ----
        with ExitStack() as p1:
            wpool = p1.enter_context(tc.tile_pool(name="w1", bufs=1))
            wq_sb = [wpool.tile([HD, NKT, DQ], F8, tag=f"wq{t}", name=f"wq{t}")
                     for t in range(2)]
            wk_sb = [wpool.tile([HD, NKT, HD], F8, tag=f"wk{t}", name=f"wk{t}")
                     for t in range(2)]
            wv_sb = [wpool.tile([HD, NKT, HD], F8, tag=f"wv{t}", name=f"wv{t}")
                     for t in range(2)]
            cs_sb = wpool.tile([HD, SEQ], BF16, tag="cs")
            sn_sb = wpool.tile([HD, SEQ], BF16, tag="sn")
            psw_sb = wpool.tile([HD, HD], F32R, tag="psw")
            idn_sb = wpool.tile([HD, HD], BF16, tag="idn")

            def _wslice(dst3d, dram, m, lo, hi):
                # ktiles [lo, hi) of a [p, (k m)] pretiled weight tensor
                nc.sync.dma_start(
                    dst3d[:, lo:hi, :],
                    dram[:, lo * m:hi * m].rearrange("p (k m) -> p k m",
                                                     k=hi - lo))

            def emit_w_dma(kg):
                # batched staging: kg==0 -> ktiles 0-4 of everything (small,
                # fast first batch); kg==1 -> ktiles 4-16; kg==3 -> 16-32.
                # One DMA instruction per tensor per batch keeps the HWDGE
                # dispatch ring (625ns/instruction) off the critical path.
                def _wbatch(lo, hi):
                    for t in range(2):
                        if not (t == 0 and lo == 0):
                            _wslice(wq_sb[t], (wqh, wql)[t], DQ, lo, hi)
                        _wslice(wk_sb[t], (wkh, wkl)[t], HD, lo, hi)
                        _wslice(wv_sb[t], (wvh, wvl)[t], HD, lo, hi)

                if kg == 0:
                    _wbatch(0, 4)
                elif kg == 1:
                    _wbatch(4, 16)
                elif kg == 3:
                    _wbatch(16, NKT)
                elif kg == 5:
                    nc.sync.dma_start(psw_sb[:], psw[:])
                    nc.sync.dma_start(idn_sb[:], idn[:])
                    nc.sync.dma_start(cs_sb[:], cs[:])
                    nc.sync.dma_start(sn_sb[:], sn[:])

            xpool = p1.enter_context(tc.tile_pool(name="xstream", bufs=3))
            xchpool = p1.enter_context(tc.tile_pool(name="xch", bufs=2))
            rtmp = p1.enter_context(tc.tile_pool(name="rtmp", bufs=2))
            ps1 = p1.enter_context(tc.tile_pool(name="ps1", bufs=1, space="PSUM"))
            ps1q = p1.enter_context(tc.tile_pool(name="ps1q", bufs=4, space="PSUM"))
            ps1m = p1.enter_context(tc.tile_pool(name="ps1m", bufs=1, space="PSUM"))

            TERMS = ((0, 0), (1, 0), (0, 1))
            xch = {}

            def emit_xch_dmas(stc):
                # full-chunk x for the sequential chunks, in 8-ktile slices
                sc_ = slice(stc * 512, (stc + 1) * 512)
                tiles = [xchpool.tile([HD, NKT, 512], F8, tag=f"xch{t}",
                                      name=f"xch{t}_{stc}") for t in range(2)]
                for t, xd in ((0, xh), (1, xl)):
                    for g in range(4):
                        nc.sync.dma_start(
                            tiles[t][:, g * 8:(g + 1) * 8, :],
                            xd[g * 8 * HD:(g + 1) * 8 * HD, sc_]
                            .rearrange("(k p) m -> p k m", p=HD))
                xch[stc] = tiles

            for st in range(SQT):
                ss = slice(st * 512, (st + 1) * 512)
                pq = [ps1q.tile([HD, 512], F32, tag="pq", name=f"pq{i}")
                      for i in range(HQ)]
                pk = ps1.tile([HD, 512], F32, tag="pk")
                pv = ps1.tile([HD, 512], F32, tag="pv")

                def rope_one(src_ps, dst, dst_sl, scale, on_act):
                    raw = rtmp.tile([HD, 512], F32R, tag="qraw")
                    if on_act:
                        nc.scalar.activation(raw[:], src_ps[:], AF.Copy,
                                             scale=scale)
                    else:
                        nc.vector.tensor_scalar_mul(raw[:], src_ps[:], scale)
                    swp = ps1m.tile([HD, 512], F32, tag="psw")
                    nc.tensor.matmul(swp[:], psw_sb[:], raw[:],
                                     start=True, stop=True)
                    t1 = rtmp.tile([HD, 512], F32, tag="t1", bufs=1)
                    nc.vector.tensor_mul(t1[:], raw[:], cs_sb[:, ss])
                    t2 = rtmp.tile([HD, 512], F32, tag="t2", bufs=1)
                    nc.vector.tensor_mul(t2[:], swp[:], sn_sb[:, ss])
                    nc.vector.tensor_add(dst[:, dst_sl], t1[:], t2[:])

                def v_block():
                    # v: descale + bf16, then transpose to [seq, dv] blocks
                    vraw = rtmp.tile([HD, 512], BF16, tag="vraw", bufs=1)
                    nc.scalar.activation(vraw[:], pv[:], AF.Copy, scale=KSCALE)
                    for j in range(4):
                        vt = ps1m.tile([HD, HD], BF16, tag="pvt")
                        nc.tensor.transpose(vt[:],
                                            vraw[:, j * HD:(j + 1) * HD],
                                            idn_sb[:])
                        if j % 2 == 0:
                            nc.scalar.activation(
                                vnat[st][:, j * HD:(j + 1) * HD], vt[:],
                                AF.Copy)
                        else:
                            nc.vector.tensor_copy(
                                vnat[st][:, j * HD:(j + 1) * HD], vt[:])

                if st < 2:
                    # streaming chunks: x quads interleaved with the matmuls
                    for kg in range(NKT // 4):
                        if st == 0 and kg == 0:
                            _wslice(wq_sb[0], wqh, DQ, 0, 4)
                        xq8 = [xpool.tile([HD, 4, 512], F8, tag=f"xt{t}",
                                          name=f"xt{t}") for t in range(2)]
                        nc.sync.dma_start(
                            xq8[0][:],
                            xh[kg * 4 * HD:(kg + 1) * 4 * HD, ss]
                            .rearrange("(k p) m -> p k m", p=HD))
                        nc.sync.dma_start(
                            xq8[1][:],
                            xl[kg * 4 * HD:(kg + 1) * 4 * HD, ss]
                            .rearrange("(k p) m -> p k m", p=HD))
                        if st == 0:
                            emit_w_dma(kg)
                        if st == 1 and kg == 4:
                            emit_xch_dmas(2)
                        for j in range(2):
                            pp = kg * 2 + j       # global pair index
                            kpair = slice(2 * pp, 2 * pp + 2)
                            xsl = [x8[:, 2 * j:2 * j + 2, :] for x8 in xq8]
                            first = (kg == 0 and j == 0)
                            last = (kg == NKT // 4 - 1 and j == 1)
                            for ti, (wi, xi) in enumerate(TERMS):
                                fl = dict(start=(first and ti == 0),
                                          stop=(last and ti == 2))
                                for mt in range(HQ):
                                    msl = slice(mt * HD, (mt + 1) * HD)
                                    nc.tensor.matmul(
                                        pq[mt][:], wq_sb[wi][:, kpair, msl],
                                        xsl[xi], perf_mode=DR, **fl)
                                nc.tensor.matmul(
                                    pk[:], wk_sb[wi][:, kpair, :], xsl[xi],
                                    perf_mode=DR, **fl)
                                nc.tensor.matmul(
                                    pv[:], wv_sb[wi][:, kpair, :], xsl[xi],
                                    perf_mode=DR, **fl)
                    for mt in range(HQ):
                        rope_one(pq[mt], qrope[mt][st], slice(0, 512), QSCALE,
                                 mt % 2 == 0)
                    rope_one(pk, krope[st], slice(0, 512), KSCALE, True)
                    v_block()
                else:
                    # sequential chunks: full-chunk x already resident;
                    # each output tile immediately runs its RoPE so the
                    # chains overlap the next tile's matmuls
                    if st == 2:
                        emit_xch_dmas(3)
                    xt8 = xch.pop(st)

                    def seq_accum(ps, wsb, msl):
                        for ppi in range(NPAIR):
                            kpair = slice(2 * ppi, 2 * ppi + 2)
                            for ti, (wi, xi) in enumerate(TERMS):
                                lhs = (wsb[wi][:, kpair, msl] if msl
                                       else wsb[wi][:, kpair, :])
                                nc.tensor.matmul(
                                    ps[:], lhs, xt8[xi][:, kpair, :],
                                    perf_mode=DR,
                                    start=(ppi == 0 and ti == 0),
                                    stop=(ppi == NPAIR - 1 and ti == 2))

                    for mt in range(HQ):
                        seq_accum(pq[mt], wq_sb, slice(mt * HD, (mt + 1) * HD))
                        rope_one(pq[mt], qrope[mt][st], slice(0, 512), QSCALE,
                                 mt % 2 == 0)
                    seq_accum(pk, wk_sb, None)
                    rope_one(pk, krope[st], slice(0, 512), KSCALE, True)
                    seq_accum(pv, wv_sb, None)
                    v_block()

        # ---- phase 2: attention;  phase 3: output projection ----
        with ExitStack() as p2:
            wopool = p2.enter_context(tc.tile_pool(name="wo", bufs=1))
            wo_sb = [wopool.tile([HD, HQ, DIM], F8, tag=f"wo{t}", name=f"wo{t}")
                     for t in range(2)]
            wo_dma_emitted = [False]

            def emit_wo_dmas():
                if not wo_dma_emitted[0]:
                    wo_dma_emitted[0] = True
                    nc.sync.dma_start(
                        wo_sb[0][:], woh[:].rearrange("p (k m) -> p k m", k=HQ))
                    nc.sync.dma_start(
                        wo_sb[1][:], wol[:].rearrange("p (k m) -> p k m", k=HQ))

            otpool = p2.enter_context(tc.tile_pool(name="ot", bufs=1))
            # attention output per head, fp8 hi/lo split for the o_proj
            ot8 = [otpool.tile([HD, HQ, SEQ], F8, tag=f"ot8{t}", name=f"ot8{t}")
                   for t in range(2)]

            mpool = p2.enter_context(tc.tile_pool(name="mk", bufs=1))
            spool = p2.enter_context(tc.tile_pool(name="sp", bufs=2))

            mk_sb = None
            if mask_mode == "causal":
                mk_sb = mpool.tile([HD, 4, 512], BF16, tag="mkd")
                nc.sync.dma_start(
                    mk_sb[:], mkt[:].rearrange("p (k m) -> p k m", k=4))

            gen_masks = {}

            def emit_gen_masks(qt):
                qs = slice(qt * 512, (qt + 1) * 512)
                out = {}
                for kst in range(16):
                    m = mpool.tile([HD, 512], F32, tag=f"mk{kst}",
                                   name=f"mk{kst}")
                    nc.sync.dma_start(
                        m[:], mkt[kst * HD:(kst + 1) * HD, qs])
                    out[kst] = m
                return out

            def npair_of(qt):
                return 2 * qt if mask_mode == "causal" else 8

            def nunit_of(qt):
                return npair_of(qt) + (4 if mask_mode == "causal" else 0)

            def issue_scores_for(qt, h, i, ps2):
                npair = npair_of(qt)
                qs = slice(qt * 512, (qt + 1) * 512)
                sp = ps2.tile([HD, 1024], F32, tag="pst")
                if i < npair:
                    for u in range(2):
                        kst = 2 * i + u
                        nc.tensor.matmul(
                            sp[:, u * 512:(u + 1) * 512],
                            kr_at(kst),
                            qrope[h][qt][:],
                            start=True, stop=True)
                else:
                    # diagonal tile, columns < c0 fully masked
                    r = i - npair
                    kst = 4 * qt + r
                    c0 = r * HD
                    nc.tensor.matmul(
                        sp[:, c0:512],
                        kr_at(kst),
                        qrope[h][qt][:, c0:512],
                        start=True, stop=True)
                return sp

            def issue_exp_for(qt, i, sp, ppool):
                npair = npair_of(qt)
                pb = ppool.tile([HD, 1024], BF16, tag="pexp")
                if i < npair:
                    if mask_mode == "general":
                        tmp = ppool.tile([HD, 1024], F32, tag="padd", bufs=2)
                        for u in range(2):
                            usl = slice(u * 512, (u + 1) * 512)
                            nc.vector.tensor_add(
                                tmp[:, usl], sp[:, usl],
                                gen_masks[qt][2 * i + u][:])
                        nc.scalar.activation(pb[:], tmp[:], AF.Exp,
                                             bias=ebias[:])
                    else:
                        nc.scalar.activation(pb[:], sp[:], AF.Exp,
                                             bias=ebias[:])
                else:
                    r = i - npair
                    c0 = r * HD
                    tmp = ppool.tile([HD, 1024], F32, tag="padd", bufs=2)
                    nc.vector.tensor_add(
                        tmp[:, c0:512], sp[:, c0:512], mk_sb[:, r, c0:])
                    nc.scalar.activation(pb[:, c0:512], tmp[:, c0:512],
                                         AF.Exp, bias=ebias[:])
                return pb

            pre_store = {}

            def attn_iter(qt, h, ps2, ps2a, ppool, filler,
                          prescore_next=None, lookahead=2):
                qs = slice(qt * 512, (qt + 1) * 512)
                npair = npair_of(qt)
                nunit = nunit_of(qt)
                sps = [None] * nunit
                pbs = [None] * nunit

                pre = pre_store.pop((qt, h), None)
                if pre is not None:
                    sps[0], sps[1] = pre
                    if lookahead > 2 and nunit > 2:
                        sps[2] = issue_scores_for(qt, h, 2, ps2)
                else:
                    for j in range(min(lookahead, nunit)):
                        sps[j] = issue_scores_for(qt, h, j, ps2)

                den = ps2a.tile([HD, 512], F32, tag="pden")
                otp = ps2a.tile([HD, 512], F32, tag="pot")
                for i in range(nunit):
                    if lookahead + i < nunit and sps[lookahead + i] is None:
                        sps[lookahead + i] = issue_scores_for(
                            qt, h, lookahead + i, ps2)
                    pbs[i] = issue_exp_for(qt, i, sps[i], ppool)
                    fl_last = (i == nunit - 1)
                    if i < npair:
                        for u in range(2):
                            kst = 2 * i + u
                            fl = dict(
                                start=(i == 0 and u == 0),
                                stop=(fl_last and u == 1))
                            pr = pbs[i][:, u * 512:(u + 1) * 512]
                            nc.tensor.matmul(
                                den[:], ones_sb[:], pr, **fl)
                            nc.tensor.matmul(
                                otp[:], vn_at(kst), pr, **fl)
                    else:
                        r = i - npair
                        kst = 4 * qt + r
                        c0 = r * HD
                        fl = dict(start=(i == 0), stop=fl_last)
                        pr = pbs[i][:, c0:512]
                        nc.tensor.matmul(
                            den[:, c0:], ones_sb[:], pr, **fl)
                        nc.tensor.matmul(
                            otp[:, c0:], vn_at(kst), pr, **fl)
                    if fl_last and prescore_next is not None:
                        # pre-issue the next iteration's first two score
                        # units so its exp pipeline starts before this
                        # iteration's DVE drain
                        qn, hn = prescore_next
                        pre_store[(qn, hn)] = (
                            issue_scores_for(qn, hn, 0, ps2),
                            issue_scores_for(qn, hn, 1, ps2))
                    if filler is not None:
                        filler()
                inv = spool.tile([HD, 512], F32, tag="inv")
                nc.vector.reciprocal(inv[:], den[:])
                ots = spool.tile([HD, 512], F32, tag="ots")
                nc.vector.tensor_mul(ots[:], otp[:], inv[:])
                # fp8 hi/lo split of the attention output
                nc.scalar.activation(ot8[0][:, h, qs], ots[:], AF.Copy)
                nc.vector.tensor_sub(ot8[1][:, h, qs], ots[:],
                                     ot8[0][:, h, qs])
                if filler is not None:
                    filler()

            # ---- phase 3 emitter: o_proj (fp8 DoubleRow 3-term), one
            # [128,512] column block per generator step so it can be
            # interleaved into the attention tail as PE filler work ----
            OTERMS = ((0, 0), (1, 0), (0, 1))

            def oproj_units(sts, ps3, opool):
                for st in sts:
                    ss = slice(st * 512, (st + 1) * 512)
                    for eg in range(DIM // HD // 4):
                        last_grp = (st == 0 and eg == DIM // HD // 4 - 1)
                        ocp = opool.tile([HD, 4, 512], BF16, tag="ocp")
                        for ej in range(4):
                            et = eg * 4 + ej
                            esl = slice(et * HD, (et + 1) * HD)
                            po = ps3.tile([HD, 512], F32, tag="po")
                            for pi in range(2):
                                hpair = slice(2 * pi, 2 * pi + 2)
                                for ti, (wi, oi) in enumerate(OTERMS):
                                    nc.tensor.matmul(
                                        po[:],
                                        wo_sb[wi][:, hpair, esl],
                                        ot8[oi][:, hpair, ss],
                                        perf_mode=DR,
                                        start=(pi == 0 and ti == 0),
                                        stop=(pi == 1 and ti == 2),
                                    )
                            osl = ocp[:, ej, :]
                            if ej % 2 == 0:
                                nc.scalar.activation(osl, po[:], AF.Copy,
                                                     scale=OSCALE)
                            else:
                                nc.vector.tensor_scalar_mul(osl, po[:], OSCALE)
                            if last_grp:
                                # final tiles: store per-slice so the last
                                # DMA isn't gated on all four copies
                                nc.sync.dma_start(
                                    outt[et * HD:(et + 1) * HD, ss], osl)
                            yield
                        if not last_grp:
                            nc.sync.dma_start(
                                outt[eg * 4 * HD:(eg + 1) * 4 * HD, ss]
                                .rearrange("(e p) m -> p e m", p=HD),
                                ocp[:])

            if mask_mode == "causal":
                with ExitStack() as patt:
                    ppool = patt.enter_context(tc.tile_pool(name="pp", bufs=4))
                    ps2 = patt.enter_context(
                        tc.tile_pool(name="ps2", bufs=3, space="PSUM"))
                    ps2a = patt.enter_context(
                        tc.tile_pool(name="ps2a", bufs=1, space="PSUM"))
                    emit_wo_dmas()
                    for h in range(HQ):
                        nxt = (3, h + 1) if h + 1 < HQ else None
                        attn_iter(3, h, ps2, ps2a, ppool, None,
                                  prescore_next=nxt)
                # tail: interleave o_proj units into the latency-bound
                # qt=1/qt=0 iterations
                with ExitStack() as ptail:
                    ppool2 = ptail.enter_context(
                        tc.tile_pool(name="pp2", bufs=6))
                    ps2t = ptail.enter_context(
                        tc.tile_pool(name="ps2t", bufs=2, space="PSUM"))
                    ps2a2 = ptail.enter_context(
                        tc.tile_pool(name="ps2a2", bufs=1, space="PSUM"))
                    ps3 = ptail.enter_context(
                        tc.tile_pool(name="ps3", bufs=2, space="PSUM"))
                    opool = ptail.enter_context(
                        tc.tile_pool(name="ostage", bufs=3))
                    gen = oproj_units((3, 2), ps3, opool)
                    # st=3 units (32) are ready once qt=3 is done; st=2
                    # units must wait until all of qt=2 has been emitted
                    pulled = [0]
                    limit = [32]
                    _done = object()

                    def filler_gen():
                        if pulled[0] < limit[0]:
                            if next(gen, _done) is not _done:
                                pulled[0] += 1

                    seq = [(qt, h) for qt in (2, 1, 0) for h in range(HQ)]
                    for n, (qt, h) in enumerate(seq[:8]):
                        attn_iter(qt, h, ps2t, ps2a2, ppool2, filler_gen,
                                  prescore_next=seq[n + 1])
                        if (qt, h) == (2, HQ - 1):
                            limit[0] = 64
                    gen2 = oproj_units((1,), ps3, opool)

                    def filler_tail():
                        if next(gen, _done) is _done:
                            next(gen2, None)

                    for h in range(HQ):
                        nxt = (0, h + 1) if h + 1 < HQ else None
                        attn_iter(0, h, ps2t, ps2a2, ppool2, filler_tail,
                                  prescore_next=nxt)
                    for _ in gen:
                        pass
                    for _ in gen2:
                        pass
                # bulk o_proj drain with deep PSUM rotation
                with ExitStack() as p3d:
                    ps3d = p3d.enter_context(
                        tc.tile_pool(name="ps3d", bufs=4, space="PSUM"))
                    opool2 = p3d.enter_context(
                        tc.tile_pool(name="ostage2", bufs=3))
                    for _ in oproj_units((0,), ps3d, opool2):
                        pass
            else:
                with ExitStack() as patt:
                    ppool = patt.enter_context(tc.tile_pool(name="pp", bufs=4))
                    ps2 = patt.enter_context(
                        tc.tile_pool(name="ps2", bufs=3, space="PSUM"))
                    ps2a = patt.enter_context(
                        tc.tile_pool(name="ps2a", bufs=1, space="PSUM"))
                    emit_wo_dmas()
                    for qt in range(SQT - 1, -1, -1):
                        if mask_mode == "general" and qt not in gen_masks:
                            gen_masks[qt] = emit_gen_masks(qt)
                        for h in range(HQ):
                            attn_iter(qt, h, ps2, ps2a, ppool, None)
                    pre_store.clear()
                with ExitStack() as p3:
                    ps3 = p3.enter_context(
                        tc.tile_pool(name="ps3", bufs=4, space="PSUM"))
                    opool = p3.enter_context(
                        tc.tile_pool(name="ostage", bufs=3))
                    for _ in oproj_units((3, 2, 1, 0), ps3, opool):
                        pass

    nc.compile()
    return nc


def _split8(a, scale=1.0):
    s = np.clip(a * np.float32(scale), -224.0, 224.0)
    hi = s.astype(E4NP)
    lo = np.clip(s - hi.astype(np.float32), -224.0, 224.0).astype(E4NP)
    return np.ascontiguousarray(hi), np.ascontiguousarray(lo)


def _prep_consts(freqs_cos, freqs_sin):
    cos = np.asarray(freqs_cos, dtype=np.float32)
    sin = np.asarray(freqs_sin, dtype=np.float32)
    C = np.empty((HD, SEQ), np.float32)
    S = np.empty((HD, SEQ), np.float32)
    C[0::2] = cos.T
    C[1::2] = cos.T
    S[0::2] = -sin.T
    S[1::2] = sin.T
    psw = np.zeros((HD, HD), np.float32)
    j = np.arange(0, HD, 2)
    psw[j + 1, j] = 1.0
    psw[j, j + 1] = 1.0
    idn = np.eye(HD, dtype=np.float32).astype(BF16NP)
    return C, S, psw, idn


def _mask_mode(mask):
    if not mask.any():
        return "zeros"
    neg = mask.min()
    tril = np.tril(np.ones((SEQ, SEQ), dtype=bool))
    if neg <= -1e8 and not mask[tril].any() and np.all(mask[~tril] == neg):
        return "causal"
    return "general"


def kernel(x, wq, wk, wv, wo, freqs_cos, freqs_sin, mask, start_pos):
    global LAST_RESULT
    assert int(start_pos) == 0, "kernel hardcodes start_pos=0 (full prefill)"
    x = np.asarray(x, dtype=np.float32)
    wq = np.asarray(wq, dtype=np.float32)
    wk = np.asarray(wk, dtype=np.float32)
    wv = np.asarray(wv, dtype=np.float32)
    wo = np.asarray(wo, dtype=np.float32)
    mask = np.asarray(mask, dtype=np.float32)

    mode = _mask_mode(mask)
    if mode not in _cache:
        _cache[mode] = _build(mode)
    nc = _cache[mode]

    xt = np.ascontiguousarray(x.reshape(SEQ, DIM).T)
    xh8, xl8 = _split8(xt)
    C, S, psw, idn = _prep_consts(freqs_cos, freqs_sin)
    mkt = None
    if mode == "causal":
        # 4 relative diagonal tile masks: tile r is mask.T[r*128:(r+1)*128,
        # 0:512] (the pattern depends only on kst - 4*qt)
        mt = np.ascontiguousarray(mask.T[:512, :512])
        mkt = np.concatenate([mt[r * HD:(r + 1) * HD, :] for r in range(4)],
                             axis=1)
        mkt = np.ascontiguousarray(mkt).astype(BF16NP)
    elif mode == "general":
        mkt = np.ascontiguousarray(mask.T)

    def _ptile(a, m):
        # [DIM_contract, m] -> partition-major [128, (ktile m)]
        k = a.shape[0] // HD
        return np.ascontiguousarray(
            a.reshape(k, HD, m).transpose(1, 0, 2).reshape(HD, k * m))

    in_maps = []
    for c in range(NCORES):
        wqh8, wql8 = _split8(wq[c * DQ:(c + 1) * DQ, :].T, WSCALE)
        wkh8, wkl8 = _split8(wk[c * HD:(c + 1) * HD, :].T, WSCALE)
        wvh8, wvl8 = _split8(wv[c * HD:(c + 1) * HD, :].T, WSCALE)
        woh8, wol8 = _split8(wo[:, c * DQ:(c + 1) * DQ].T, WSCALE)
        wqh8, wql8 = _ptile(wqh8, DQ), _ptile(wql8, DQ)
        wkh8, wkl8 = _ptile(wkh8, HD), _ptile(wkl8, HD)
        wvh8, wvl8 = _ptile(wvh8, HD), _ptile(wvl8, HD)
        woh8, wol8 = _ptile(woh8, DIM), _ptile(wol8, DIM)
        m = {
            "xh": xh8, "xl": xl8,
            "wqh": wqh8, "wql": wql8,
            "wkh": wkh8, "wkl": wkl8,
            "wvh": wvh8, "wvl": wvl8,
            "woh": woh8, "wol": wol8,
            "cs": C.astype(BF16NP), "sn": S.astype(BF16NP),
            "psw": psw, "idn": idn,
        }
        if mkt is not None:
            m["mkt"] = mkt
        in_maps.append(m)

    res = run_bass_kernel_spmd(nc, in_maps, core_ids=list(range(NCORES)),
                               trace=TRACE)
    LAST_RESULT = res
    acc = np.zeros((DIM, SEQ), dtype=np.float64)
    for c in range(NCORES):
        acc += res.results[c]["outt"].astype(np.float64)
    return np.ascontiguousarray(acc.T).astype(np.float32).reshape(1, SEQ, DIM)


# revision 63
# speedup vs baseline: 1.0361x; 1.0121x over previous
"""GQA attention block (QKV proj + RoPE + causal attention + o_proj),
tensor-parallel over heads across 8 TRN2 NeuronCores.

Sharding: core c owns q heads [4c, 4c+4) (512 q dims), kv head c
(128 kv dims), and wo columns [512c, 512c+512). Each core computes a
full-shape partial of the output projection; the host sums the 8
partials (the "all-reduce") and transposes back.

Layout convention on device: activations are kept feature-major
([dim, seq]) so every matmul contracts over the partition axis with
no transposes:
  QT/KT [d, s]  ->  scores^T [ks, qs] = KT_tile^T . QT   (lhsT=KT, rhs=QT)
  softmax over ks = partition axis: exp on ACT, denominator via
  ones-matmul on PE, division folded into the PV output scaling
  PV: OT [dv, qs] = V_nat^T . P                           (lhsT=V, rhs=P)
  o_proj: outT [e, s] = woT^T . OT                        (lhsT=woT, rhs=OT)

Precision plan: the dense GEMMs (QKV proj, o_proj) run as fp8e4
DoubleRow matmuls (2 K-tiles contracted per instruction at 0.5
cycles/row) with a hi/lo residual split of both operands and the
three significant cross terms (hi.hi + lo.hi + hi.lo) accumulated in
fp32 PSUM - ~1.5e-3 relative error at 0.75x the bf16/fp32r cycle
cost. Weights are pre-scaled by 64 (power of two, folded back into
the PSUM->SBUF copy scale) so their hi/lo parts stay in fp8e4 normal
range; the attention output is pre-scaled by 16 (via the den "ones"
stationary = 1/16) for the same reason. q/k/v/P/scores run in bf16
(same PE rate as fp32r, half the SBUF/DMA). Output partials are
stored bf16 and summed on host.

Scheduling: weights arrive host-pretiled partition-major in a few
large staged DMAs (the HWDGE dispatch ring costs 625ns per DMA
instruction); x arrives as host-split fp8 hi/lo streams. The first
two seq chunks stream x quads interleaved with the matmuls (the DMA
pipe is saturated by weight loading there); the last two hold the
full chunk of x resident (prefetched while DMA is otherwise idle)
and run their six output tiles sequentially, each immediately
followed by its RoPE, so the RoPE chains overlap the next tile's
matmuls and attention starts without waiting on a rope tail.
Attention runs qt descending with a 2-unit score lookahead and
cross-head score pre-issue; the latency-bound qt<=2 iterations
interleave o_proj column-block emissions between units as PE filler
(gated so an o_proj chunk is only emitted after the attention chunk
feeding it is complete), with the remaining o_proj drained at deeper
PSUM rotation afterwards.
"""

import sys
from contextlib import ExitStack

import numpy as np
import ml_dtypes

for _p in ("/opt/trn_rl_repo", "/opt/trn_rl_repo/concourse"):
    if _p not in sys.path:
        sys.path.insert(0, _p)

import concourse.bacc as bacc
import concourse.bass as bass
import concourse.tile as tile
from concourse import mybir
from concourse.bass_utils import run_bass_kernel_spmd

F32 = mybir.dt.float32
F32R = mybir.dt.float32r
BF16 = mybir.dt.bfloat16
F8 = mybir.dt.float8e4
E4NP = ml_dtypes.float8_e4m3
BF16NP = ml_dtypes.bfloat16
AF = mybir.ActivationFunctionType
DR = mybir.MatmulPerfMode.DoubleRow

DIM = 4096
SEQ = 2048
HD = 128          # head dim
NCORES = 8
HQ = 4            # q heads per core
DQ = HQ * HD      # 512 q dims per core
NKT = DIM // HD   # 32 contraction tiles
NPAIR = NKT // 2  # 16 DoubleRow k-tile pairs
SQT = SEQ // 512  # 4 seq chunks of 512
INV_SQRT_HD = 1.0 / np.sqrt(np.float32(HD))
EXP_BIAS = -4.0   # constant shift inside exp; cancels in softmax.
                  # -4 keeps exp outputs within fp8e4 normal range
                  # for the DoubleRow denominator path (max logit
                  # ~5.5 -> p <= e^1.5; typical p ~0.02 >> 2^-9)
WSCALE = 64.0     # weight pre-scale so fp8 hi/lo stays in normal range
OTSCALE = 16.0    # attention-output pre-scale for its fp8 hi/lo split

TRACE = False
LAST_RESULT = None

_cache = {}


def _build(mask_mode):
    """mask_mode: 'zeros' | 'causal' | 'general'."""
    nc = bacc.Bacc("TRN2", target_bir_lowering=False)
    xh = nc.dram_tensor("xh", [DIM, SEQ], F8, kind="ExternalInput")
    xl = nc.dram_tensor("xl", [DIM, SEQ], F8, kind="ExternalInput")
    # weights arrive pre-tiled partition-major: [p, (ktile m)]
    wqh = nc.dram_tensor("wqh", [HD, NKT * DQ], F8, kind="ExternalInput")
    wql = nc.dram_tensor("wql", [HD, NKT * DQ], F8, kind="ExternalInput")
    wkh = nc.dram_tensor("wkh", [HD, NKT * HD], F8, kind="ExternalInput")
    wkl = nc.dram_tensor("wkl", [HD, NKT * HD], F8, kind="ExternalInput")
    wvh = nc.dram_tensor("wvh", [HD, NKT * HD], F8, kind="ExternalInput")
    wvl = nc.dram_tensor("wvl", [HD, NKT * HD], F8, kind="ExternalInput")
    woh = nc.dram_tensor("woh", [HD, HQ * DIM], F8, kind="ExternalInput")
    wol = nc.dram_tensor("wol", [HD, HQ * DIM], F8, kind="ExternalInput")
    cs = nc.dram_tensor("cs", [HD, SEQ], BF16, kind="ExternalInput")
    sn = nc.dram_tensor("sn", [HD, SEQ], BF16, kind="ExternalInput")
    psw = nc.dram_tensor("psw", [HD, HD], F32R, kind="ExternalInput")
    idn = nc.dram_tensor("idn", [HD, HD], BF16, kind="ExternalInput")
    mkt = None
    if mask_mode == "causal":
        # 4 relative diagonal-tile masks (pattern repeats for every qt)
        mkt = nc.dram_tensor("mkt", [HD, 4 * 512], BF16, kind="ExternalInput")
    elif mask_mode == "general":
        mkt = nc.dram_tensor("mkt", [SEQ, SEQ], F32, kind="ExternalInput")
    outt = nc.dram_tensor("outt", [DIM, SEQ], BF16, kind="ExternalOutput")

    QSCALE = float(INV_SQRT_HD / WSCALE)
    KSCALE = float(1.0 / WSCALE)
    OSCALE = float(1.0 / (WSCALE * OTSCALE))

    with ExitStack() as ctx:
        tc = ctx.enter_context(tile.TileContext(nc))

        # ---- persistent pools ----
        const = ctx.enter_context(tc.tile_pool(name="const", bufs=1))
        ones_f32 = const.tile([HD, HD], F32, tag="ones32")
        # den is accumulated pre-divided by OTSCALE so inv = OTSCALE/den and
        # the attention output is scaled into fp8-friendly range for the
        # o_proj hi/lo split; the final output copy divides it back out.
        nc.vector.memset(ones_f32[:], 1.0 / OTSCALE)
        ones_sb = const.tile([HD, HD], BF16, tag="ones")
        nc.scalar.activation(ones_sb[:], ones_f32[:], AF.Copy)
        ebias = const.tile([HD, 1], F32, tag="ebias")
        nc.vector.memset(ebias[:], EXP_BIAS)
        ones8 = const.tile([HD, 2, HD], F8, tag="ones8")
        for _u in range(2):
            nc.scalar.activation(ones8[:, _u, :], ones_f32[:], AF.Copy)

        qkvpool = ctx.enter_context(tc.tile_pool(name="qkv", bufs=1))
        # per-chunk tiles so attention reads only depend on the chunks they
        # actually touch (no false whole-tile hazards on the last chunk)
        qrope = [[qkvpool.tile([HD, 512], BF16, tag=f"qr{h}_{c}",
                               name=f"qr{h}_{c}") for c in range(SQT)]
                 for h in range(HQ)]
        krope = [qkvpool.tile([HD, 512], BF16, tag=f"kr{c}", name=f"kr{c}")
                 for c in range(SQT)]
        vnat = [qkvpool.tile([HD, 512], BF16, tag=f"vn{c}", name=f"vn{c}")
                for c in range(SQT)]

        def kr_at(kst):
            return krope[kst // 4][:, (kst % 4) * HD:(kst % 4 + 1) * HD]

        def vn_at(kst):
            return vnat[kst // 4][:, (kst % 4) * HD:(kst % 4 + 1) * HD]

        # ---- phase 1: QKV projection (fp8 DoubleRow 3-term) + RoPE ----
        with ExitStack() as p1:
            wpool = p1.enter_context(tc.tile_pool(name="w1", bufs=1))
            wq_sb = [wpool.tile([HD, NKT, DQ], F8, tag=f"wq{t}", name=f"wq{t}")
                     for t in range(2)]
            wk_sb = [wpool.tile([HD, NKT, HD], F8, tag=f"wk{t}", name=f"wk{t}")
                     for t in range(2)]
            wv_sb = [wpool.tile([HD, NKT, HD], F8, tag=f"wv{t}", name=f"wv{t}")
                     for t in range(2)]
            cs_sb = wpool.tile([HD, SEQ], BF16, tag="cs")
            sn_sb = wpool.tile([HD, SEQ], BF16, tag="sn")
            psw_sb = wpool.tile([HD, HD], F32R, tag="psw")
            idn_sb = wpool.tile([HD, HD], BF16, tag="idn")

            def _wslice(dst3d, dram, m, lo, hi):
                # ktiles [lo, hi) of a [p, (k m)] pretiled weight tensor
                nc.sync.dma_start(
                    dst3d[:, lo:hi, :],
                    dram[:, lo * m:hi * m].rearrange("p (k m) -> p k m",
                                                     k=hi - lo))

            def emit_w_dma(kg):
                # batched staging: kg==0 -> ktiles 0-4 of everything (small,
                # fast first batch); kg==1 -> ktiles 4-16; kg==3 -> 16-32.
                # One DMA instruction per tensor per batch keeps the HWDGE
                # dispatch ring (625ns/instruction) off the critical path.
                def _wbatch(lo, hi):
                    for t in range(2):
                        if not (t == 0 and lo == 0):
                            _wslice(wq_sb[t], (wqh, wql)[t], DQ, lo, hi)
                        _wslice(wk_sb[t], (wkh, wkl)[t], HD, lo, hi)
                        _wslice(wv_sb[t], (wvh, wvl)[t], HD, lo, hi)

                if kg == 0:
                    _wbatch(0, 4)
                elif kg == 1:
                    _wbatch(4, 16)
                elif kg == 3:
                    _wbatch(16, NKT)
                elif kg == 5:
                    nc.sync.dma_start(psw_sb[:], psw[:])
                    nc.sync.dma_start(idn_sb[:], idn[:])
                    nc.sync.dma_start(cs_sb[:], cs[:])
                    nc.sync.dma_start(sn_sb[:], sn[:])

            xpool = p1.enter_context(tc.tile_pool(name="xstream", bufs=3))
            xchpool = p1.enter_context(tc.tile_pool(name="xch", bufs=2))
            rtmp = p1.enter_context(tc.tile_pool(name="rtmp", bufs=2))
            ps1 = p1.enter_context(tc.tile_pool(name="ps1", bufs=1, space="PSUM"))
            ps1q = p1.enter_context(tc.tile_pool(name="ps1q", bufs=4, space="PSUM"))
            ps1m = p1.enter_context(tc.tile_pool(name="ps1m", bufs=1, space="PSUM"))

            TERMS = ((0, 0), (1, 0), (0, 1))
            xch = {}

            def emit_xch_dmas(stc):
                # full-chunk x for the sequential chunks, in 8-ktile slices
                sc_ = slice(stc * 512, (stc + 1) * 512)
                tiles = [xchpool.tile([HD, NKT, 512], F8, tag=f"xch{t}",
                                      name=f"xch{t}_{stc}") for t in range(2)]
                for t, xd in ((0, xh), (1, xl)):
                    for g in range(4):
                        nc.sync.dma_start(
                            tiles[t][:, g * 8:(g + 1) * 8, :],
                            xd[g * 8 * HD:(g + 1) * 8 * HD, sc_]
                            .rearrange("(k p) m -> p k m", p=HD))
                xch[stc] = tiles

            for st in range(SQT):
                ss = slice(st * 512, (st + 1) * 512)
                pq = [ps1q.tile([HD, 512], F32, tag="pq", name=f"pq{i}")
                      for i in range(HQ)]
                pk = ps1.tile([HD, 512], F32, tag="pk")
                pv = ps1.tile([HD, 512], F32, tag="pv")

                def rope_one(src_ps, dst, dst_sl, scale, on_act):
                    raw = rtmp.tile([HD, 512], F32R, tag="qraw")
                    if on_act:
                        nc.scalar.activation(raw[:], src_ps[:], AF.Copy,
                                             scale=scale)
                    else:
                        nc.vector.tensor_scalar_mul(raw[:], src_ps[:], scale)
                    swp = ps1m.tile([HD, 512], F32, tag="psw")
                    nc.tensor.matmul(swp[:], psw_sb[:], raw[:],
                                     start=True, stop=True)
                    t1 = rtmp.tile([HD, 512], F32, tag="t1", bufs=1)
                    nc.vector.tensor_mul(t1[:], raw[:], cs_sb[:, ss])
                    t2 = rtmp.tile([HD, 512], F32, tag="t2", bufs=1)
                    nc.vector.tensor_mul(t2[:], swp[:], sn_sb[:, ss])
                    nc.vector.tensor_add(dst[:, dst_sl], t1[:], t2[:])

                def v_block():
                    # v: descale + bf16, then transpose to [seq, dv] blocks
                    vraw = rtmp.tile([HD, 512], BF16, tag="vraw", bufs=1)
                    nc.scalar.activation(vraw[:], pv[:], AF.Copy, scale=KSCALE)
                    for j in range(4):
                        vt = ps1m.tile([HD, HD], BF16, tag="pvt")
                        nc.tensor.transpose(vt[:],
                                            vraw[:, j * HD:(j + 1) * HD],
                                            idn_sb[:])
                        if j % 2 == 0:
                            nc.scalar.activation(
                                vnat[st][:, j * HD:(j + 1) * HD], vt[:],
                                AF.Copy)
                        else:
                            nc.vector.tensor_copy(
                                vnat[st][:, j * HD:(j + 1) * HD], vt[:])

                if st < 2:
                    # streaming chunks: x quads interleaved with the matmuls
                    for kg in range(NKT // 4):
                        if st == 0 and kg == 0:
                            _wslice(wq_sb[0], wqh, DQ, 0, 4)
                        xq8 = [xpool.tile([HD, 4, 512], F8, tag=f"xt{t}",
                                          name=f"xt{t}") for t in range(2)]
                        nc.sync.dma_start(
                            xq8[0][:],
                            xh[kg * 4 * HD:(kg + 1) * 4 * HD, ss]
                            .rearrange("(k p) m -> p k m", p=HD))
                        nc.sync.dma_start(
                            xq8[1][:],
                            xl[kg * 4 * HD:(kg + 1) * 4 * HD, ss]
                            .rearrange("(k p) m -> p k m", p=HD))
                        if st == 0:
                            emit_w_dma(kg)
                        if st == 1 and kg == 4:
                            emit_xch_dmas(2)
                        for j in range(2):
                            pp = kg * 2 + j       # global pair index
                            kpair = slice(2 * pp, 2 * pp + 2)
                            xsl = [x8[:, 2 * j:2 * j + 2, :] for x8 in xq8]
                            first = (kg == 0 and j == 0)
                            last = (kg == NKT // 4 - 1 and j == 1)
                            for ti, (wi, xi) in enumerate(TERMS):
                                fl = dict(start=(first and ti == 0),
                                          stop=(last and ti == 2))
                                for mt in range(HQ):
                                    msl = slice(mt * HD, (mt + 1) * HD)
                                    nc.tensor.matmul(
                                        pq[mt][:], wq_sb[wi][:, kpair, msl],
                                        xsl[xi], perf_mode=DR, **fl)
                                nc.tensor.matmul(
                                    pk[:], wk_sb[wi][:, kpair, :], xsl[xi],
                                    perf_mode=DR, **fl)
                                nc.tensor.matmul(
                                    pv[:], wv_sb[wi][:, kpair, :], xsl[xi],
                                    perf_mode=DR, **fl)
                    for mt in range(HQ):
                        rope_one(pq[mt], qrope[mt][st], slice(0, 512), QSCALE,
                                 mt % 2 == 0)
                    rope_one(pk, krope[st], slice(0, 512), KSCALE, True)
                    v_block()
                else:
                    # sequential chunks: full-chunk x already resident;
                    # each output tile immediately runs its RoPE so the
                    # chains overlap the next tile's matmuls
                    if st == 2:
                        emit_xch_dmas(3)
                    xt8 = xch.pop(st)

                    def seq_accum(ps, wsb, msl):
                        for ppi in range(NPAIR):
                            kpair = slice(2 * ppi, 2 * ppi + 2)
                            for ti, (wi, xi) in enumerate(TERMS):
                                lhs = (wsb[wi][:, kpair, msl] if msl
                                       else wsb[wi][:, kpair, :])
                                nc.tensor.matmul(
                                    ps[:], lhs, xt8[xi][:, kpair, :],
                                    perf_mode=DR,
                                    start=(ppi == 0 and ti == 0),
                                    stop=(ppi == NPAIR - 1 and ti == 2))

                    for mt in range(HQ):
                        seq_accum(pq[mt], wq_sb, slice(mt * HD, (mt + 1) * HD))
                        rope_one(pq[mt], qrope[mt][st], slice(0, 512), QSCALE,
                                 mt % 2 == 0)
                    seq_accum(pk, wk_sb, None)
                    rope_one(pk, krope[st], slice(0, 512), KSCALE, True)
                    seq_accum(pv, wv_sb, None)
                    v_block()

        # ---- phase 2: attention;  phase 3: output projection ----
        with ExitStack() as p2:
            wopool = p2.enter_context(tc.tile_pool(name="wo", bufs=1))
            wo_sb = [wopool.tile([HD, HQ, DIM], F8, tag=f"wo{t}", name=f"wo{t}")
                     for t in range(2)]
            wo_dma_emitted = [False]

            def emit_wo_dmas():
                if not wo_dma_emitted[0]:
                    wo_dma_emitted[0] = True
                    nc.sync.dma_start(
                        wo_sb[0][:], woh[:].rearrange("p (k m) -> p k m", k=HQ))
                    nc.sync.dma_start(
                        wo_sb[1][:], wol[:].rearrange("p (k m) -> p k m", k=HQ))

            otpool = p2.enter_context(tc.tile_pool(name="ot", bufs=1))
            # attention output per head, fp8 hi/lo split for the o_proj
            ot8 = [otpool.tile([HD, HQ, SEQ], F8, tag=f"ot8{t}", name=f"ot8{t}")
                   for t in range(2)]

            mpool = p2.enter_context(tc.tile_pool(name="mk", bufs=1))
            spool = p2.enter_context(tc.tile_pool(name="sp", bufs=2))

            mk_sb = None
            if mask_mode == "causal":
                mk_sb = mpool.tile([HD, 4, 512], BF16, tag="mkd")
                nc.sync.dma_start(
                    mk_sb[:], mkt[:].rearrange("p (k m) -> p k m", k=4))

            gen_masks = {}

            def emit_gen_masks(qt):
                qs = slice(qt * 512, (qt + 1) * 512)
                out = {}
                for kst in range(16):
                    m = mpool.tile([HD, 512], F32, tag=f"mk{kst}",
                                   name=f"mk{kst}")
                    nc.sync.dma_start(
                        m[:], mkt[kst * HD:(kst + 1) * HD, qs])
                    out[kst] = m
                return out

            def npair_of(qt):
                return 2 * qt if mask_mode == "causal" else 8

            def nunit_of(qt):
                return npair_of(qt) + (4 if mask_mode == "causal" else 0)

            def issue_scores_for(qt, h, i, ps2):
                npair = npair_of(qt)
                qs = slice(qt * 512, (qt + 1) * 512)
                sp = ps2.tile([HD, 1024], F32, tag="pst")
                if i < npair:
                    for u in range(2):
                        kst = 2 * i + u
                        nc.tensor.matmul(
                            sp[:, u * 512:(u + 1) * 512],
                            kr_at(kst),
                            qrope[h][qt][:],
                            start=True, stop=True)
                else:
                    # diagonal tile, columns < c0 fully masked
                    r = i - npair
                    kst = 4 * qt + r
                    c0 = r * HD
                    nc.tensor.matmul(
                        sp[:, c0:512],
                        kr_at(kst),
                        qrope[h][qt][:, c0:512],
                        start=True, stop=True)
                return sp

            def issue_exp_for(qt, i, sp, ppool):
                npair = npair_of(qt)
                pb = ppool.tile([HD, 1024], BF16, tag="pexp")
                if i < npair:
                    if mask_mode == "general":
                        tmp = ppool.tile([HD, 1024], F32, tag="padd", bufs=2)
                        for u in range(2):
                            usl = slice(u * 512, (u + 1) * 512)
                            nc.vector.tensor_add(
                                tmp[:, usl], sp[:, usl],
                                gen_masks[qt][2 * i + u][:])
                        nc.scalar.activation(pb[:], tmp[:], AF.Exp,
                                             bias=ebias[:])
                    else:
                        nc.scalar.activation(pb[:], sp[:], AF.Exp,
                                             bias=ebias[:])
                else:
                    r = i - npair
                    c0 = r * HD
                    tmp = ppool.tile([HD, 1024], F32, tag="padd", bufs=2)
                    nc.vector.tensor_add(
                        tmp[:, c0:512], sp[:, c0:512], mk_sb[:, r, c0:])
                    nc.scalar.activation(pb[:, c0:512], tmp[:, c0:512],
                                         AF.Exp, bias=ebias[:])
                return pb

            pre_store = {}

            def attn_iter(qt, h, ps2, ps2a, ppool, filler,
                          prescore_next=None, lookahead=2, p8pool=None):
                qs = slice(qt * 512, (qt + 1) * 512)
                npair = npair_of(qt)
                nunit = nunit_of(qt)
                sps = [None] * nunit
                pbs = [None] * nunit

                pre = pre_store.pop((qt, h), None)
                if pre is not None:
                    sps[0], sps[1] = pre
                    if lookahead > 2 and nunit > 2:
                        sps[2] = issue_scores_for(qt, h, 2, ps2)
                else:
                    for j in range(min(lookahead, nunit)):
                        sps[j] = issue_scores_for(qt, h, j, ps2)

                den = ps2a.tile([HD, 512], F32, tag="pden")
                otp = ps2a.tile([HD, 512], F32, tag="pot")
                # den_dr: non-diag pair units compute den as one fp8
                # DoubleRow matmul over a Pool-engine fp8 copy of P (lagged
                # one unit to hide the cast latency)
                den_dr = p8pool is not None and npair > 0
                pend = []

                def flush_den_dr():
                    while pend:
                        p8t, first = pend.pop(0)
                        nc.tensor.matmul(
                            den[:], ones8[:],
                            p8t[:].rearrange("p (u m) -> p u m", u=2),
                            perf_mode=DR, start=first, stop=False)

                for i in range(nunit):
                    if lookahead + i < nunit and sps[lookahead + i] is None:
                        sps[lookahead + i] = issue_scores_for(
                            qt, h, lookahead + i, ps2)
                    pbs[i] = issue_exp_for(qt, i, sps[i], ppool)
                    fl_last = (i == nunit - 1)
                    if i < npair:
                        if den_dr:
                            p8t = p8pool.tile([HD, 1024], F8, tag="p8")
                            nc.gpsimd.tensor_copy(p8t[:], pbs[i][:])
                        for u in range(2):
                            kst = 2 * i + u
                            fl = dict(
                                start=(i == 0 and u == 0),
                                stop=(fl_last and u == 1))
                            pr = pbs[i][:, u * 512:(u + 1) * 512]
                            if not den_dr:
                                nc.tensor.matmul(
                                    den[:], ones_sb[:], pr, **fl)
                            nc.tensor.matmul(
                                otp[:], vn_at(kst), pr, **fl)
                        if den_dr:
                            flush_den_dr()
                            pend.append((p8t, i == 0))
                    else:
                        r = i - npair
                        kst = 4 * qt + r
                        c0 = r * HD
                        if den_dr:
                            flush_den_dr()
                        fl = dict(start=(i == 0), stop=fl_last)
                        pr = pbs[i][:, c0:512]
                        nc.tensor.matmul(
                            den[:, c0:], ones_sb[:], pr,
                            start=(i == 0 and not den_dr), stop=fl_last)
                        nc.tensor.matmul(
                            otp[:, c0:], vn_at(kst), pr, **fl)
                    if fl_last and prescore_next is not None:
                        # pre-issue the next iteration's first two score
                        # units so its exp pipeline starts before this
                        # iteration's DVE drain
                        qn, hn = prescore_next
                        pre_store[(qn, hn)] = (
                            issue_scores_for(qn, hn, 0, ps2),
                            issue_scores_for(qn, hn, 1, ps2))
                    if filler is not None:
                        filler()
                inv = spool.tile([HD, 512], F32, tag="inv")
                nc.vector.reciprocal(inv[:], den[:])
                ots = spool.tile([HD, 512], F32, tag="ots")
                nc.vector.tensor_mul(ots[:], otp[:], inv[:])
                # fp8 hi/lo split of the attention output
                nc.scalar.activation(ot8[0][:, h, qs], ots[:], AF.Copy)
                nc.vector.tensor_sub(ot8[1][:, h, qs], ots[:],
                                     ot8[0][:, h, qs])
                if filler is not None:
                    filler()

            # ---- phase 3 emitter: o_proj (fp8 DoubleRow 3-term), one
            # [128,512] column block per generator step so it can be
            # interleaved into the attention tail as PE filler work ----
            OTERMS = ((0, 0), (1, 0), (0, 1))

            def oproj_units(sts, ps3, opool):
                for st in sts:
                    ss = slice(st * 512, (st + 1) * 512)
                    for eg in range(DIM // HD // 4):
                        last_grp = (st == 0 and eg == DIM // HD // 4 - 1)
                        ocp = opool.tile([HD, 4, 512], BF16, tag="ocp")
                        for ej in range(4):
                            et = eg * 4 + ej
                            esl = slice(et * HD, (et + 1) * HD)
                            po = ps3.tile([HD, 512], F32, tag="po")
                            for pi in range(2):
                                hpair = slice(2 * pi, 2 * pi + 2)
                                for ti, (wi, oi) in enumerate(OTERMS):
                                    nc.tensor.matmul(
                                        po[:],
                                        wo_sb[wi][:, hpair, esl],
                                        ot8[oi][:, hpair, ss],
                                        perf_mode=DR,
                                        start=(pi == 0 and ti == 0),
                                        stop=(pi == 1 and ti == 2),
                                    )
                            osl = ocp[:, ej, :]
                            if ej % 2 == 0:
                                nc.scalar.activation(osl, po[:], AF.Copy,
                                                     scale=OSCALE)
                            else:
                                nc.vector.tensor_scalar_mul(osl, po[:], OSCALE)
                            if last_grp:
                                # final tiles: store per-slice so the last
                                # DMA isn't gated on all four copies
                                nc.sync.dma_start(
                                    outt[et * HD:(et + 1) * HD, ss], osl)
                            yield
                        if not last_grp:
                            nc.sync.dma_start(
                                outt[eg * 4 * HD:(eg + 1) * 4 * HD, ss]
                                .rearrange("(e p) m -> p e m", p=HD),
                                ocp[:])

            if mask_mode == "causal":
                with ExitStack() as patt:
                    ppool = patt.enter_context(tc.tile_pool(name="pp", bufs=4))
                    ps2 = patt.enter_context(
                        tc.tile_pool(name="ps2", bufs=3, space="PSUM"))
                    ps2a = patt.enter_context(
                        tc.tile_pool(name="ps2a", bufs=1, space="PSUM"))
                    emit_wo_dmas()
                    for h in range(HQ):
                        nxt = (3, h + 1) if h + 1 < HQ else None
                        attn_iter(3, h, ps2, ps2a, ppool, None,
                                  prescore_next=nxt)
                # tail: interleave o_proj units into the latency-bound
                # qt=1/qt=0 iterations
                with ExitStack() as ptail:
                    ppool2 = ptail.enter_context(
                        tc.tile_pool(name="pp2", bufs=6))
                    ps2t = ptail.enter_context(
                        tc.tile_pool(name="ps2t", bufs=2, space="PSUM"))
                    ps2a2 = ptail.enter_context(
                        tc.tile_pool(name="ps2a2", bufs=1, space="PSUM"))
                    ps3 = ptail.enter_context(
                        tc.tile_pool(name="ps3", bufs=2, space="PSUM"))
                    opool = ptail.enter_context(
                        tc.tile_pool(name="ostage", bufs=3))
                    gen = oproj_units((3, 2), ps3, opool)
                    # st=3 units (32) are ready once qt=3 is done; st=2
                    # units must wait until all of qt=2 has been emitted
                    pulled = [0]
                    limit = [32]
                    _done = object()

                    def filler_gen():
                        if pulled[0] < limit[0]:
                            if next(gen, _done) is not _done:
                                pulled[0] += 1

                    p8pool = ptail.enter_context(
                        tc.tile_pool(name="p8p", bufs=4))
                    seq = [(qt, h) for qt in (2, 1, 0) for h in range(HQ)]
                    for n, (qt, h) in enumerate(seq[:8]):
                        attn_iter(qt, h, ps2t, ps2a2, ppool2, filler_gen,
                                  prescore_next=seq[n + 1], p8pool=p8pool)
                        if (qt, h) == (2, HQ - 1):
                            limit[0] = 64
                    gen2 = oproj_units((1,), ps3, opool)

                    def filler_tail():
                        if next(gen, _done) is _done:
                            next(gen2, None)

                    for h in range(HQ):
                        nxt = (0, h + 1) if h + 1 < HQ else None
                        attn_iter(0, h, ps2t, ps2a2, ppool2, filler_tail,
                                  prescore_next=nxt)
                    for _ in gen:
                        pass
                    for _ in gen2:
                        pass
                # bulk o_proj drain with deep PSUM rotation
                with ExitStack() as p3d:
                    ps3d = p3d.enter_context(
                        tc.tile_pool(name="ps3d", bufs=4, space="PSUM"))
                    opool2 = p3d.enter_context(
                        tc.tile_pool(name="ostage2", bufs=3))
                    for _ in oproj_units((0,), ps3d, opool2):
                        pass
            else:
                with ExitStack() as patt:
                    ppool = patt.enter_context(tc.tile_pool(name="pp", bufs=4))
                    ps2 = patt.enter_context(
                        tc.tile_pool(name="ps2", bufs=3, space="PSUM"))
                    ps2a = patt.enter_context(
                        tc.tile_pool(name="ps2a", bufs=1, space="PSUM"))
                    emit_wo_dmas()
                    for qt in range(SQT - 1, -1, -1):
                        if mask_mode == "general" and qt not in gen_masks:
                            gen_masks[qt] = emit_gen_masks(qt)
                        for h in range(HQ):
                            attn_iter(qt, h, ps2, ps2a, ppool, None)
                    pre_store.clear()
                with ExitStack() as p3:
                    ps3 = p3.enter_context(
                        tc.tile_pool(name="ps3", bufs=4, space="PSUM"))
                    opool = p3.enter_context(
                        tc.tile_pool(name="ostage", bufs=3))
                    for _ in oproj_units((3, 2, 1, 0), ps3, opool):
                        pass

    nc.compile()
    return nc


def _split8(a, scale=1.0):
    s = np.clip(a * np.float32(scale), -224.0, 224.0)
    hi = s.astype(E4NP)
    lo = np.clip(s - hi.astype(np.float32), -224.0, 224.0).astype(E4NP)
    return np.ascontiguousarray(hi), np.ascontiguousarray(lo)


def _prep_consts(freqs_cos, freqs_sin):
    cos = np.asarray(freqs_cos, dtype=np.float32)
    sin = np.asarray(freqs_sin, dtype=np.float32)
    C = np.empty((HD, SEQ), np.float32)
    S = np.empty((HD, SEQ), np.float32)
    C[0::2] = cos.T
    C[1::2] = cos.T
    S[0::2] = -sin.T
    S[1::2] = sin.T
    psw = np.zeros((HD, HD), np.float32)
    j = np.arange(0, HD, 2)
    psw[j + 1, j] = 1.0
    psw[j, j + 1] = 1.0
    idn = np.eye(HD, dtype=np.float32).astype(BF16NP)
    return C, S, psw, idn


def _mask_mode(mask):
    if not mask.any():
        return "zeros"
    neg = mask.min()
    tril = np.tril(np.ones((SEQ, SEQ), dtype=bool))
    if neg <= -1e8 and not mask[tril].any() and np.all(mask[~tril] == neg):
        return "causal"
    return "general"


def kernel(x, wq, wk, wv, wo, freqs_cos, freqs_sin, mask, start_pos):
    global LAST_RESULT
    assert int(start_pos) == 0, "kernel hardcodes start_pos=0 (full prefill)"
    x = np.asarray(x, dtype=np.float32)
    wq = np.asarray(wq, dtype=np.float32)
    wk = np.asarray(wk, dtype=np.float32)
    wv = np.asarray(wv, dtype=np.float32)
    wo = np.asarray(wo, dtype=np.float32)
    mask = np.asarray(mask, dtype=np.float32)

    mode = _mask_mode(mask)
    if mode not in _cache:
        _cache[mode] = _build(mode)
    nc = _cache[mode]

    xt = np.ascontiguousarray(x.reshape(SEQ, DIM).T)
    xh8, xl8 = _split8(xt)
    C, S, psw, idn = _prep_consts(freqs_cos, freqs_sin)
    mkt = None
    if mode == "causal":
        # 4 relative diagonal tile masks: tile r is mask.T[r*128:(r+1)*128,
        # 0:512] (the pattern depends only on kst - 4*qt)
        mt = np.ascontiguousarray(mask.T[:512, :512])
        mkt = np.concatenate([mt[r * HD:(r + 1) * HD, :] for r in range(4)],
                             axis=1)
        mkt = np.ascontiguousarray(mkt).astype(BF16NP)
    elif mode == "general":
        mkt = np.ascontiguousarray(mask.T)

    def _ptile(a, m):
        # [DIM_contract, m] -> partition-major [128, (ktile m)]
        k = a.shape[0] // HD
        return np.ascontiguousarray(
            a.reshape(k, HD, m).transpose(1, 0, 2).reshape(HD, k * m))

    in_maps = []
    for c in range(NCORES):
        wqh8, wql8 = _split8(wq[c * DQ:(c + 1) * DQ, :].T, WSCALE)
        wkh8, wkl8 = _split8(wk[c * HD:(c + 1) * HD, :].T, WSCALE)
        wvh8, wvl8 = _split8(wv[c * HD:(c + 1) * HD, :].T, WSCALE)
        woh8, wol8 = _split8(wo[:, c * DQ:(c + 1) * DQ].T, WSCALE)
        wqh8, wql8 = _ptile(wqh8, DQ), _ptile(wql8, DQ)
        wkh8, wkl8 = _ptile(wkh8, HD), _ptile(wkl8, HD)
        wvh8, wvl8 = _ptile(wvh8, HD), _ptile(wvl8, HD)
        woh8, wol8 = _ptile(woh8, DIM), _ptile(wol8, DIM)
        m = {
            "xh": xh8, "xl": xl8,
            "wqh": wqh8, "wql": wql8,
            "wkh": wkh8, "wkl": wkl8,
            "wvh": wvh8, "wvl": wvl8,
            "woh": woh8, "wol": wol8,
            "cs": C.astype(BF16NP), "sn": S.astype(BF16NP),
            "psw": psw, "idn": idn,
        }
        if mkt is not None:
            m["mkt"] = mkt
        in_maps.append(m)

    res = run_bass_kernel_spmd(nc, in_maps, core_ids=list(range(NCORES)),
                               trace=TRACE)
    LAST_RESULT = res
    acc = np.zeros((DIM, SEQ), dtype=np.float64)
    for c in range(NCORES):
        acc += res.results[c]["outt"].astype(np.float64)
    return np.ascontiguousarray(acc.T).astype(np.float32).reshape(1, SEQ, DIM)


# revision 67
# speedup vs baseline: 1.0380x; 1.0018x over previous
"""GQA attention block (QKV proj + RoPE + causal attention + o_proj),
tensor-parallel over heads across 8 TRN2 NeuronCores.

Sharding: core c owns q heads [4c, 4c+4) (512 q dims), kv head c
(128 kv dims), and wo columns [512c, 512c+512). Each core computes a
full-shape partial of the output projection; the host sums the 8
partials (the "all-reduce") and transposes back.

Layout convention on device: activations are kept feature-major
([dim, seq]) so every matmul contracts over the partition axis with
no transposes:
  QT/KT [d, s]  ->  scores^T [ks, qs] = KT_tile^T . QT   (lhsT=KT, rhs=QT)
  softmax over ks = partition axis: exp on ACT, denominator via
  ones-matmul on PE, division folded into the PV output scaling
  PV: OT [dv, qs] = V_nat^T . P                           (lhsT=V, rhs=P)
  o_proj: outT [e, s] = woT^T . OT                        (lhsT=woT, rhs=OT)

Precision plan: the dense GEMMs (QKV proj, o_proj) run as fp8e4
DoubleRow matmuls (2 K-tiles contracted per instruction at 0.5
cycles/row) with a hi/lo residual split of both operands and the
three significant cross terms (hi.hi + lo.hi + hi.lo) accumulated in
fp32 PSUM - ~1.5e-3 relative error at 0.75x the bf16/fp32r cycle
cost. Weights are pre-scaled by 64 (power of two, folded back into
the PSUM->SBUF copy scale) so their hi/lo parts stay in fp8e4 normal
range; the attention output is pre-scaled by 16 (via the den "ones"
stationary = 1/16) for the same reason. q/k/v/P/scores run in bf16
(same PE rate as fp32r, half the SBUF/DMA). Output partials are
stored bf16 and summed on host.

Scheduling: weights arrive host-pretiled partition-major in a few
large staged DMAs (the HWDGE dispatch ring costs 625ns per DMA
instruction); x arrives as host-split fp8 hi/lo streams. The first
two seq chunks stream x quads interleaved with the matmuls (the DMA
pipe is saturated by weight loading there); the last two hold the
full chunk of x resident (prefetched while DMA is otherwise idle)
and run their six output tiles sequentially, each immediately
followed by its RoPE, so the RoPE chains overlap the next tile's
matmuls and attention starts without waiting on a rope tail.
Attention runs qt descending with a 2-unit score lookahead and
cross-head score pre-issue; the latency-bound qt<=2 iterations
interleave o_proj column-block emissions between units as PE filler
(gated so an o_proj chunk is only emitted after the attention chunk
feeding it is complete), with the remaining o_proj drained at deeper
PSUM rotation afterwards.
"""

import sys
from contextlib import ExitStack

import numpy as np
import ml_dtypes

for _p in ("/opt/trn_rl_repo", "/opt/trn_rl_repo/concourse"):
    if _p not in sys.path:
        sys.path.insert(0, _p)

import concourse.bacc as bacc
import concourse.bass as bass
import concourse.tile as tile
from concourse import mybir
from concourse.bass_utils import run_bass_kernel_spmd

F32 = mybir.dt.float32
F32R = mybir.dt.float32r
BF16 = mybir.dt.bfloat16
F8 = mybir.dt.float8e4
E4NP = ml_dtypes.float8_e4m3
BF16NP = ml_dtypes.bfloat16
AF = mybir.ActivationFunctionType
DR = mybir.MatmulPerfMode.DoubleRow

DIM = 4096
SEQ = 2048
HD = 128          # head dim
NCORES = 8
HQ = 4            # q heads per core
DQ = HQ * HD      # 512 q dims per core
NKT = DIM // HD   # 32 contraction tiles
NPAIR = NKT // 2  # 16 DoubleRow k-tile pairs
SQT = SEQ // 512  # 4 seq chunks of 512
INV_SQRT_HD = 1.0 / np.sqrt(np.float32(HD))
EXP_BIAS = -4.0   # constant shift inside exp; cancels in softmax.
                  # -4 keeps exp outputs within fp8e4 normal range
                  # for the DoubleRow denominator path (max logit
                  # ~5.5 -> p <= e^1.5; typical p ~0.02 >> 2^-9)
WSCALE = 64.0     # weight pre-scale so fp8 hi/lo stays in normal range
OTSCALE = 16.0    # attention-output pre-scale for its fp8 hi/lo split

TRACE = False
LAST_RESULT = None

_cache = {}


def _build(mask_mode):
    """mask_mode: 'zeros' | 'causal' | 'general'."""
    nc = bacc.Bacc("TRN2", target_bir_lowering=False)
    xh = nc.dram_tensor("xh", [DIM, SEQ], F8, kind="ExternalInput")
    xl = nc.dram_tensor("xl", [DIM, SEQ], F8, kind="ExternalInput")
    # weights arrive pre-tiled partition-major: [p, (ktile m)]
    wqh = nc.dram_tensor("wqh", [HD, NKT * DQ], F8, kind="ExternalInput")
    wql = nc.dram_tensor("wql", [HD, NKT * DQ], F8, kind="ExternalInput")
    wkh = nc.dram_tensor("wkh", [HD, NKT * HD], F8, kind="ExternalInput")
    wkl = nc.dram_tensor("wkl", [HD, NKT * HD], F8, kind="ExternalInput")
    wvh = nc.dram_tensor("wvh", [HD, NKT * HD], F8, kind="ExternalInput")
    wvl = nc.dram_tensor("wvl", [HD, NKT * HD], F8, kind="ExternalInput")
    woh = nc.dram_tensor("woh", [HD, HQ * DIM], F8, kind="ExternalInput")
    wol = nc.dram_tensor("wol", [HD, HQ * DIM], F8, kind="ExternalInput")
    cs = nc.dram_tensor("cs", [HD, SEQ], BF16, kind="ExternalInput")
    sn = nc.dram_tensor("sn", [HD, SEQ], BF16, kind="ExternalInput")
    psw = nc.dram_tensor("psw", [HD, HD], F32R, kind="ExternalInput")
    idn = nc.dram_tensor("idn", [HD, HD], BF16, kind="ExternalInput")
    mkt = None
    if mask_mode == "causal":
        # 4 relative diagonal-tile masks (pattern repeats for every qt)
        mkt = nc.dram_tensor("mkt", [HD, 4 * 512], BF16, kind="ExternalInput")
    elif mask_mode == "general":
        mkt = nc.dram_tensor("mkt", [SEQ, SEQ], F32, kind="ExternalInput")
    outt = nc.dram_tensor("outt", [DIM, SEQ], BF16, kind="ExternalOutput")

    QSCALE = float(INV_SQRT_HD / WSCALE)
    KSCALE = float(1.0 / WSCALE)
    OSCALE = float(1.0 / (WSCALE * OTSCALE))

    with ExitStack() as ctx:
        tc = ctx.enter_context(tile.TileContext(nc))

        # ---- persistent pools ----
        const = ctx.enter_context(tc.tile_pool(name="const", bufs=1))
        ones_f32 = const.tile([HD, HD], F32, tag="ones32")
        # den is accumulated pre-divided by OTSCALE so inv = OTSCALE/den and
        # the attention output is scaled into fp8-friendly range for the
        # o_proj hi/lo split; the final output copy divides it back out.
        nc.vector.memset(ones_f32[:], 1.0 / OTSCALE)
        ones_sb = const.tile([HD, HD], BF16, tag="ones")
        nc.scalar.activation(ones_sb[:], ones_f32[:], AF.Copy)
        ebias = const.tile([HD, 1], F32, tag="ebias")
        nc.vector.memset(ebias[:], EXP_BIAS)
        ones8 = const.tile([HD, 2, HD], F8, tag="ones8")
        for _u in range(2):
            nc.scalar.activation(ones8[:, _u, :], ones_f32[:], AF.Copy)

        qkvpool = ctx.enter_context(tc.tile_pool(name="qkv", bufs=1))
        # per-chunk tiles so attention reads only depend on the chunks they
        # actually touch (no false whole-tile hazards on the last chunk)
        qrope = [[qkvpool.tile([HD, 512], BF16, tag=f"qr{h}_{c}",
                               name=f"qr{h}_{c}") for c in range(SQT)]
                 for h in range(HQ)]
        krope = [qkvpool.tile([HD, 512], BF16, tag=f"kr{c}", name=f"kr{c}")
                 for c in range(SQT)]
        vnat = [qkvpool.tile([HD, 512], BF16, tag=f"vn{c}", name=f"vn{c}")
                for c in range(SQT)]

        def kr_at(kst):
            return krope[kst // 4][:, (kst % 4) * HD:(kst % 4 + 1) * HD]

        def vn_at(kst):
            return vnat[kst // 4][:, (kst % 4) * HD:(kst % 4 + 1) * HD]

        # ---- phase 1: QKV projection (fp8 DoubleRow 3-term) + RoPE ----
        with ExitStack() as p1:
            wpool = p1.enter_context(tc.tile_pool(name="w1", bufs=1))
            wq_sb = [wpool.tile([HD, NKT, DQ], F8, tag=f"wq{t}", name=f"wq{t}")
                     for t in range(2)]
            wk_sb = [wpool.tile([HD, NKT, HD], F8, tag=f"wk{t}", name=f"wk{t}")
                     for t in range(2)]
            wv_sb = [wpool.tile([HD, NKT, HD], F8, tag=f"wv{t}", name=f"wv{t}")
                     for t in range(2)]
            cs_sb = wpool.tile([HD, SEQ], BF16, tag="cs")
            sn_sb = wpool.tile([HD, SEQ], BF16, tag="sn")
            psw_sb = wpool.tile([HD, HD], F32R, tag="psw")
            idn_sb = wpool.tile([HD, HD], BF16, tag="idn")

            def _wslice(dst3d, dram, m, lo, hi):
                # ktiles [lo, hi) of a [p, (k m)] pretiled weight tensor
                nc.sync.dma_start(
                    dst3d[:, lo:hi, :],
                    dram[:, lo * m:hi * m].rearrange("p (k m) -> p k m",
                                                     k=hi - lo))

            def emit_w_dma(kg):
                # batched staging: kg==0 -> ktiles 0-4 of everything (small,
                # fast first batch); kg==1 -> ktiles 4-16; kg==3 -> 16-32.
                # One DMA instruction per tensor per batch keeps the HWDGE
                # dispatch ring (625ns/instruction) off the critical path.
                def _wbatch(lo, hi):
                    for t in range(2):
                        if not (t == 0 and lo == 0):
                            _wslice(wq_sb[t], (wqh, wql)[t], DQ, lo, hi)
                        _wslice(wk_sb[t], (wkh, wkl)[t], HD, lo, hi)
                        _wslice(wv_sb[t], (wvh, wvl)[t], HD, lo, hi)

                if kg == 0:
                    _wbatch(0, 4)
                elif kg == 1:
                    _wbatch(4, 16)
                elif kg == 3:
                    _wbatch(16, NKT)
                elif kg == 5:
                    nc.sync.dma_start(psw_sb[:], psw[:])
                    nc.sync.dma_start(idn_sb[:], idn[:])
                    nc.sync.dma_start(cs_sb[:], cs[:])
                    nc.sync.dma_start(sn_sb[:], sn[:])

            xpool = p1.enter_context(tc.tile_pool(name="xstream", bufs=3))
            xchpool = p1.enter_context(tc.tile_pool(name="xch", bufs=2))
            rtmp = p1.enter_context(tc.tile_pool(name="rtmp", bufs=2))
            ps1 = p1.enter_context(tc.tile_pool(name="ps1", bufs=1, space="PSUM"))
            ps1q = p1.enter_context(tc.tile_pool(name="ps1q", bufs=4, space="PSUM"))
            ps1m = p1.enter_context(tc.tile_pool(name="ps1m", bufs=1, space="PSUM"))

            TERMS = ((0, 0), (1, 0), (0, 1))
            xch = {}

            def emit_xch_dmas(stc):
                # full-chunk x for the sequential chunks, in 8-ktile slices
                sc_ = slice(stc * 512, (stc + 1) * 512)
                tiles = [xchpool.tile([HD, NKT, 512], F8, tag=f"xch{t}",
                                      name=f"xch{t}_{stc}") for t in range(2)]
                for t, xd in ((0, xh), (1, xl)):
                    for g in range(4):
                        nc.sync.dma_start(
                            tiles[t][:, g * 8:(g + 1) * 8, :],
                            xd[g * 8 * HD:(g + 1) * 8 * HD, sc_]
                            .rearrange("(k p) m -> p k m", p=HD))
                xch[stc] = tiles

            for st in range(SQT):
                ss = slice(st * 512, (st + 1) * 512)
                pq = [ps1q.tile([HD, 512], F32, tag="pq", name=f"pq{i}")
                      for i in range(HQ)]
                pk = ps1.tile([HD, 512], F32, tag="pk")
                pv = ps1.tile([HD, 512], F32, tag="pv")

                def rope_one(src_ps, dst, dst_sl, scale, on_act):
                    raw = rtmp.tile([HD, 512], F32R, tag="qraw")
                    if on_act:
                        nc.scalar.activation(raw[:], src_ps[:], AF.Copy,
                                             scale=scale)
                    else:
                        nc.vector.tensor_scalar_mul(raw[:], src_ps[:], scale)
                    swp = ps1m.tile([HD, 512], F32, tag="psw")
                    nc.tensor.matmul(swp[:], psw_sb[:], raw[:],
                                     start=True, stop=True)
                    t1 = rtmp.tile([HD, 512], F32, tag="t1", bufs=1)
                    nc.vector.tensor_mul(t1[:], raw[:], cs_sb[:, ss])
                    t2 = rtmp.tile([HD, 512], F32, tag="t2", bufs=1)
                    nc.vector.tensor_mul(t2[:], swp[:], sn_sb[:, ss])
                    nc.vector.tensor_add(dst[:, dst_sl], t1[:], t2[:])

                def v_block():
                    # v: descale + bf16, then transpose to [seq, dv] blocks
                    vraw = rtmp.tile([HD, 512], BF16, tag="vraw", bufs=1)
                    nc.scalar.activation(vraw[:], pv[:], AF.Copy, scale=KSCALE)
                    for j in range(4):
                        vt = ps1m.tile([HD, HD], BF16, tag="pvt")
                        nc.tensor.transpose(vt[:],
                                            vraw[:, j * HD:(j + 1) * HD],
                                            idn_sb[:])
                        if j % 2 == 0:
                            nc.scalar.activation(
                                vnat[st][:, j * HD:(j + 1) * HD], vt[:],
                                AF.Copy)
                        else:
                            nc.vector.tensor_copy(
                                vnat[st][:, j * HD:(j + 1) * HD], vt[:])

                if st < 2:
                    # streaming chunks: x quads interleaved with the matmuls
                    for kg in range(NKT // 4):
                        if st == 0 and kg == 0:
                            _wslice(wq_sb[0], wqh, DQ, 0, 4)
                        xq8 = [xpool.tile([HD, 4, 512], F8, tag=f"xt{t}",
                                          name=f"xt{t}") for t in range(2)]
                        nc.sync.dma_start(
                            xq8[0][:],
                            xh[kg * 4 * HD:(kg + 1) * 4 * HD, ss]
                            .rearrange("(k p) m -> p k m", p=HD))
                        nc.sync.dma_start(
                            xq8[1][:],
                            xl[kg * 4 * HD:(kg + 1) * 4 * HD, ss]
                            .rearrange("(k p) m -> p k m", p=HD))
                        if st == 0:
                            emit_w_dma(kg)
                        if st == 1 and kg == 4:
                            emit_xch_dmas(2)
                        for j in range(2):
                            pp = kg * 2 + j       # global pair index
                            kpair = slice(2 * pp, 2 * pp + 2)
                            xsl = [x8[:, 2 * j:2 * j + 2, :] for x8 in xq8]
                            first = (kg == 0 and j == 0)
                            last = (kg == NKT // 4 - 1 and j == 1)
                            for ti, (wi, xi) in enumerate(TERMS):
                                fl = dict(start=(first and ti == 0),
                                          stop=(last and ti == 2))
                                for mt in range(HQ):
                                    msl = slice(mt * HD, (mt + 1) * HD)
                                    nc.tensor.matmul(
                                        pq[mt][:], wq_sb[wi][:, kpair, msl],
                                        xsl[xi], perf_mode=DR, **fl)
                                nc.tensor.matmul(
                                    pk[:], wk_sb[wi][:, kpair, :], xsl[xi],
                                    perf_mode=DR, **fl)
                                nc.tensor.matmul(
                                    pv[:], wv_sb[wi][:, kpair, :], xsl[xi],
                                    perf_mode=DR, **fl)
                    for mt in range(HQ):
                        rope_one(pq[mt], qrope[mt][st], slice(0, 512), QSCALE,
                                 mt % 2 == 0)
                    rope_one(pk, krope[st], slice(0, 512), KSCALE, True)
                    v_block()
                else:
                    # sequential chunks: full-chunk x already resident;
                    # each output tile immediately runs its RoPE so the
                    # chains overlap the next tile's matmuls
                    if st == 2:
                        emit_xch_dmas(3)
                    xt8 = xch.pop(st)

                    def seq_accum(ps, wsb, msl):
                        for ppi in range(NPAIR):
                            kpair = slice(2 * ppi, 2 * ppi + 2)
                            for ti, (wi, xi) in enumerate(TERMS):
                                lhs = (wsb[wi][:, kpair, msl] if msl
                                       else wsb[wi][:, kpair, :])
                                nc.tensor.matmul(
                                    ps[:], lhs, xt8[xi][:, kpair, :],
                                    perf_mode=DR,
                                    start=(ppi == 0 and ti == 0),
                                    stop=(ppi == NPAIR - 1 and ti == 2))

                    for mt in range(HQ):
                        seq_accum(pq[mt], wq_sb, slice(mt * HD, (mt + 1) * HD))
                        rope_one(pq[mt], qrope[mt][st], slice(0, 512), QSCALE,
                                 mt % 2 == 0)
                    seq_accum(pk, wk_sb, None)
                    rope_one(pk, krope[st], slice(0, 512), KSCALE, True)
                    seq_accum(pv, wv_sb, None)
                    v_block()

        # ---- phase 2: attention;  phase 3: output projection ----
        with ExitStack() as p2:
            wopool = p2.enter_context(tc.tile_pool(name="wo", bufs=1))
            wo_sb = [wopool.tile([HD, HQ, DIM], F8, tag=f"wo{t}", name=f"wo{t}")
                     for t in range(2)]
            wo_dma_emitted = [False]

            def emit_wo_dmas():
                if not wo_dma_emitted[0]:
                    wo_dma_emitted[0] = True
                    nc.sync.dma_start(
                        wo_sb[0][:], woh[:].rearrange("p (k m) -> p k m", k=HQ))
                    nc.sync.dma_start(
                        wo_sb[1][:], wol[:].rearrange("p (k m) -> p k m", k=HQ))

            otpool = p2.enter_context(tc.tile_pool(name="ot", bufs=1))
            # attention output per head, fp8 hi/lo split for the o_proj
            ot8 = [otpool.tile([HD, HQ, SEQ], F8, tag=f"ot8{t}", name=f"ot8{t}")
                   for t in range(2)]

            mpool = p2.enter_context(tc.tile_pool(name="mk", bufs=1))
            spool = p2.enter_context(tc.tile_pool(name="sp", bufs=2))

            mk_sb = None
            if mask_mode == "causal":
                mk_sb = mpool.tile([HD, 4, 512], BF16, tag="mkd")
                nc.sync.dma_start(
                    mk_sb[:], mkt[:].rearrange("p (k m) -> p k m", k=4))

            gen_masks = {}

            def emit_gen_masks(qt):
                qs = slice(qt * 512, (qt + 1) * 512)
                out = {}
                for kst in range(16):
                    m = mpool.tile([HD, 512], F32, tag=f"mk{kst}",
                                   name=f"mk{kst}")
                    nc.sync.dma_start(
                        m[:], mkt[kst * HD:(kst + 1) * HD, qs])
                    out[kst] = m
                return out

            def npair_of(qt):
                return 2 * qt if mask_mode == "causal" else 8

            def nunit_of(qt):
                return npair_of(qt) + (4 if mask_mode == "causal" else 0)

            def issue_scores_for(qt, h, i, ps2):
                npair = npair_of(qt)
                qs = slice(qt * 512, (qt + 1) * 512)
                sp = ps2.tile([HD, 1024], F32, tag="pst")
                if i < npair:
                    for u in range(2):
                        kst = 2 * i + u
                        nc.tensor.matmul(
                            sp[:, u * 512:(u + 1) * 512],
                            kr_at(kst),
                            qrope[h][qt][:],
                            start=True, stop=True)
                else:
                    # diagonal tile, columns < c0 fully masked
                    r = i - npair
                    kst = 4 * qt + r
                    c0 = r * HD
                    nc.tensor.matmul(
                        sp[:, c0:512],
                        kr_at(kst),
                        qrope[h][qt][:, c0:512],
                        start=True, stop=True)
                return sp

            def issue_exp_for(qt, i, sp, ppool):
                npair = npair_of(qt)
                pb = ppool.tile([HD, 1024], BF16, tag="pexp")
                if i < npair:
                    if mask_mode == "general":
                        tmp = ppool.tile([HD, 1024], F32, tag="padd", bufs=2)
                        for u in range(2):
                            usl = slice(u * 512, (u + 1) * 512)
                            nc.vector.tensor_add(
                                tmp[:, usl], sp[:, usl],
                                gen_masks[qt][2 * i + u][:])
                        nc.scalar.activation(pb[:], tmp[:], AF.Exp,
                                             bias=ebias[:])
                    else:
                        nc.scalar.activation(pb[:], sp[:], AF.Exp,
                                             bias=ebias[:])
                else:
                    r = i - npair
                    c0 = r * HD
                    tmp = ppool.tile([HD, 1024], F32, tag="padd", bufs=2)
                    nc.vector.tensor_add(
                        tmp[:, c0:512], sp[:, c0:512], mk_sb[:, r, c0:])
                    nc.scalar.activation(pb[:, c0:512], tmp[:, c0:512],
                                         AF.Exp, bias=ebias[:])
                return pb

            pre_store = {}

            def attn_iter(qt, h, ps2, ps2a, ppool, filler,
                          prescore_next=None, lookahead=2, p8pool=None):
                qs = slice(qt * 512, (qt + 1) * 512)
                npair = npair_of(qt)
                nunit = nunit_of(qt)
                sps = [None] * nunit
                pbs = [None] * nunit

                pre = pre_store.pop((qt, h), None)
                if pre is not None:
                    sps[0], sps[1] = pre
                    if lookahead > 2 and nunit > 2:
                        sps[2] = issue_scores_for(qt, h, 2, ps2)
                else:
                    for j in range(min(lookahead, nunit)):
                        sps[j] = issue_scores_for(qt, h, j, ps2)

                den = ps2a.tile([HD, 512], F32, tag="pden")
                otp = ps2a.tile([HD, 512], F32, tag="pot")
                # den_dr: non-diag pair units compute den as one fp8
                # DoubleRow matmul over a Pool-engine fp8 copy of P (lagged
                # one unit to hide the cast latency)
                den_dr = p8pool is not None and npair > 0
                pend = []

                def flush_den_dr(keep=0):
                    while len(pend) > keep:
                        p8t, first = pend.pop(0)
                        nc.tensor.matmul(
                            den[:], ones8[:],
                            p8t[:].rearrange("p (u m) -> p u m", u=2),
                            perf_mode=DR, start=first, stop=False)

                for i in range(nunit):
                    if lookahead + i < nunit and sps[lookahead + i] is None:
                        sps[lookahead + i] = issue_scores_for(
                            qt, h, lookahead + i, ps2)
                    pbs[i] = issue_exp_for(qt, i, sps[i], ppool)
                    fl_last = (i == nunit - 1)
                    if i < npair:
                        if den_dr:
                            p8t = p8pool.tile([HD, 1024], F8, tag="p8")
                            nc.gpsimd.tensor_copy(p8t[:], pbs[i][:])
                        for u in range(2):
                            kst = 2 * i + u
                            fl = dict(
                                start=(i == 0 and u == 0),
                                stop=(fl_last and u == 1))
                            pr = pbs[i][:, u * 512:(u + 1) * 512]
                            if not den_dr:
                                nc.tensor.matmul(
                                    den[:], ones_sb[:], pr, **fl)
                            nc.tensor.matmul(
                                otp[:], vn_at(kst), pr, **fl)
                        if den_dr:
                            flush_den_dr(keep=1)
                            pend.append((p8t, i == 0))
                    else:
                        r = i - npair
                        kst = 4 * qt + r
                        c0 = r * HD
                        if den_dr:
                            flush_den_dr()
                        fl = dict(start=(i == 0), stop=fl_last)
                        pr = pbs[i][:, c0:512]
                        nc.tensor.matmul(
                            den[:, c0:], ones_sb[:], pr,
                            start=(i == 0 and not den_dr), stop=fl_last)
                        nc.tensor.matmul(
                            otp[:, c0:], vn_at(kst), pr, **fl)
                    if fl_last and prescore_next is not None:
                        # pre-issue the next iteration's first two score
                        # units so its exp pipeline starts before this
                        # iteration's DVE drain
                        qn, hn = prescore_next
                        pre_store[(qn, hn)] = (
                            issue_scores_for(qn, hn, 0, ps2),
                            issue_scores_for(qn, hn, 1, ps2))
                    if filler is not None:
                        filler()
                inv = spool.tile([HD, 512], F32, tag="inv")
                nc.vector.reciprocal(inv[:], den[:])
                ots = spool.tile([HD, 512], F32, tag="ots")
                nc.vector.tensor_mul(ots[:], otp[:], inv[:])
                # fp8 hi/lo split of the attention output
                nc.scalar.activation(ot8[0][:, h, qs], ots[:], AF.Copy)
                nc.vector.tensor_sub(ot8[1][:, h, qs], ots[:],
                                     ot8[0][:, h, qs])
                if filler is not None:
                    filler()

            # ---- phase 3 emitter: o_proj (fp8 DoubleRow 3-term), one
            # [128,512] column block per generator step so it can be
            # interleaved into the attention tail as PE filler work ----
            OTERMS = ((0, 0), (1, 0), (0, 1))

            def oproj_units(sts, ps3, opool, egs=None):
                for st in sts:
                    ss = slice(st * 512, (st + 1) * 512)
                    for eg in (range(DIM // HD // 4) if egs is None
                               else egs):
                        last_grp = (st == 0 and eg == DIM // HD // 4 - 1)
                        ocp = opool.tile([HD, 4, 512], BF16, tag="ocp")
                        for ej in range(4):
                            et = eg * 4 + ej
                            esl = slice(et * HD, (et + 1) * HD)
                            po = ps3.tile([HD, 512], F32, tag="po")
                            for pi in range(2):
                                hpair = slice(2 * pi, 2 * pi + 2)
                                for ti, (wi, oi) in enumerate(OTERMS):
                                    nc.tensor.matmul(
                                        po[:],
                                        wo_sb[wi][:, hpair, esl],
                                        ot8[oi][:, hpair, ss],
                                        perf_mode=DR,
                                        start=(pi == 0 and ti == 0),
                                        stop=(pi == 1 and ti == 2),
                                    )
                            osl = ocp[:, ej, :]
                            if ej % 2 == 0:
                                nc.scalar.activation(osl, po[:], AF.Copy,
                                                     scale=OSCALE)
                            else:
                                nc.vector.tensor_scalar_mul(osl, po[:], OSCALE)
                            if last_grp:
                                # final tiles: store per-slice so the last
                                # DMA isn't gated on all four copies
                                nc.sync.dma_start(
                                    outt[et * HD:(et + 1) * HD, ss], osl)
                            yield
                        if not last_grp:
                            nc.sync.dma_start(
                                outt[eg * 4 * HD:(eg + 1) * 4 * HD, ss]
                                .rearrange("(e p) m -> p e m", p=HD),
                                ocp[:])

            if mask_mode == "causal":
                with ExitStack() as patt:
                    ppool = patt.enter_context(tc.tile_pool(name="pp", bufs=4))
                    ps2 = patt.enter_context(
                        tc.tile_pool(name="ps2", bufs=3, space="PSUM"))
                    ps2a = patt.enter_context(
                        tc.tile_pool(name="ps2a", bufs=1, space="PSUM"))
                    emit_wo_dmas()
                    for h in range(HQ):
                        nxt = (3, h + 1) if h + 1 < HQ else None
                        attn_iter(3, h, ps2, ps2a, ppool, None,
                                  prescore_next=nxt)
                # tail: interleave o_proj units into the latency-bound
                # qt=1/qt=0 iterations
                with ExitStack() as ptail:
                    ppool2 = ptail.enter_context(
                        tc.tile_pool(name="pp2", bufs=6))
                    ps2t = ptail.enter_context(
                        tc.tile_pool(name="ps2t", bufs=2, space="PSUM"))
                    ps2a2 = ptail.enter_context(
                        tc.tile_pool(name="ps2a2", bufs=1, space="PSUM"))
                    ps3 = ptail.enter_context(
                        tc.tile_pool(name="ps3", bufs=2, space="PSUM"))
                    opool = ptail.enter_context(
                        tc.tile_pool(name="ostage", bufs=3))
                    gen = oproj_units((3, 2), ps3, opool)
                    # st=3 units (32) are ready once qt=3 is done; st=2
                    # units must wait until all of qt=2 has been emitted
                    pulled = [0]
                    limit = [32]
                    _done = object()

                    def filler_gen():
                        if pulled[0] < limit[0]:
                            if next(gen, _done) is not _done:
                                pulled[0] += 1

                    p8pool = ptail.enter_context(
                        tc.tile_pool(name="p8p", bufs=4))
                    seq = [(qt, h) for qt in (2, 1, 0) for h in range(HQ)]
                    for n, (qt, h) in enumerate(seq[:8]):
                        attn_iter(qt, h, ps2t, ps2a2, ppool2, filler_gen,
                                  prescore_next=seq[n + 1], p8pool=p8pool)
                        if (qt, h) == (2, HQ - 1):
                            limit[0] = 64
                    gen2 = oproj_units((1,), ps3, opool, egs=range(0, 4))

                    def filler_tail():
                        if next(gen, _done) is _done:
                            next(gen2, None)

                    for h in range(HQ):
                        nxt = (0, h + 1) if h + 1 < HQ else None
                        attn_iter(0, h, ps2t, ps2a2, ppool2, filler_tail,
                                  prescore_next=nxt)
                    for _ in gen:
                        pass
                    for _ in gen2:
                        pass
                # bulk o_proj drain with deep PSUM rotation
                with ExitStack() as p3d:
                    ps3d = p3d.enter_context(
                        tc.tile_pool(name="ps3d", bufs=4, space="PSUM"))
                    opool2 = p3d.enter_context(
                        tc.tile_pool(name="ostage2", bufs=3))
                    for _ in oproj_units((1,), ps3d, opool2,
                                         egs=range(4, 8)):
                        pass
                    for _ in oproj_units((0,), ps3d, opool2):
                        pass
            else:
                with ExitStack() as patt:
                    ppool = patt.enter_context(tc.tile_pool(name="pp", bufs=4))
                    ps2 = patt.enter_context(
                        tc.tile_pool(name="ps2", bufs=3, space="PSUM"))
                    ps2a = patt.enter_context(
                        tc.tile_pool(name="ps2a", bufs=1, space="PSUM"))
                    emit_wo_dmas()
                    for qt in range(SQT - 1, -1, -1):
                        if mask_mode == "general" and qt not in gen_masks:
                            gen_masks[qt] = emit_gen_masks(qt)
                        for h in range(HQ):
                            attn_iter(qt, h, ps2, ps2a, ppool, None)
                    pre_store.clear()
                with ExitStack() as p3:
                    ps3 = p3.enter_context(
                        tc.tile_pool(name="ps3", bufs=4, space="PSUM"))
                    opool = p3.enter_context(
                        tc.tile_pool(name="ostage", bufs=3))
                    for _ in oproj_units((3, 2, 1, 0), ps3, opool):
                        pass

    nc.compile()
    return nc


def _split8(a, scale=1.0):
    s = np.clip(a * np.float32(scale), -224.0, 224.0)
    hi = s.astype(E4NP)
    lo = np.clip(s - hi.astype(np.float32), -224.0, 224.0).astype(E4NP)
    return np.ascontiguousarray(hi), np.ascontiguousarray(lo)


def _prep_consts(freqs_cos, freqs_sin):
    cos = np.asarray(freqs_cos, dtype=np.float32)
    sin = np.asarray(freqs_sin, dtype=np.float32)
    C = np.empty((HD, SEQ), np.float32)
    S = np.empty((HD, SEQ), np.float32)
    C[0::2] = cos.T
    C[1::2] = cos.T
    S[0::2] = -sin.T
    S[1::2] = sin.T
    psw = np.zeros((HD, HD), np.float32)
    j = np.arange(0, HD, 2)
    psw[j + 1, j] = 1.0
    psw[j, j + 1] = 1.0
    idn = np.eye(HD, dtype=np.float32).astype(BF16NP)
    return C, S, psw, idn


def _mask_mode(mask):
    if not mask.any():
        return "zeros"
    neg = mask.min()
    tril = np.tril(np.ones((SEQ, SEQ), dtype=bool))
    if neg <= -1e8 and not mask[tril].any() and np.all(mask[~tril] == neg):
        return "causal"
    return "general"


def kernel(x, wq, wk, wv, wo, freqs_cos, freqs_sin, mask, start_pos):
    global LAST_RESULT
    assert int(start_pos) == 0, "kernel hardcodes start_pos=0 (full prefill)"
    x = np.asarray(x, dtype=np.float32)
    wq = np.asarray(wq, dtype=np.float32)
    wk = np.asarray(wk, dtype=np.float32)
    wv = np.asarray(wv, dtype=np.float32)
    wo = np.asarray(wo, dtype=np.float32)
    mask = np.asarray(mask, dtype=np.float32)

    mode = _mask_mode(mask)
    if mode not in _cache:
        _cache[mode] = _build(mode)
    nc = _cache[mode]

    xt = np.ascontiguousarray(x.reshape(SEQ, DIM).T)
    xh8, xl8 = _split8(xt)
    C, S, psw, idn = _prep_consts(freqs_cos, freqs_sin)
    mkt = None
    if mode == "causal":
        # 4 relative diagonal tile masks: tile r is mask.T[r*128:(r+1)*128,
        # 0:512] (the pattern depends only on kst - 4*qt)
        mt = np.ascontiguousarray(mask.T[:512, :512])
        mkt = np.concatenate([mt[r * HD:(r + 1) * HD, :] for r in range(4)],
                             axis=1)
        mkt = np.ascontiguousarray(mkt).astype(BF16NP)
    elif mode == "general":
        mkt = np.ascontiguousarray(mask.T)

    def _ptile(a, m):
        # [DIM_contract, m] -> partition-major [128, (ktile m)]
        k = a.shape[0] // HD
        return np.ascontiguousarray(
            a.reshape(k, HD, m).transpose(1, 0, 2).reshape(HD, k * m))

    in_maps = []
    for c in range(NCORES):
        wqh8, wql8 = _split8(wq[c * DQ:(c + 1) * DQ, :].T, WSCALE)
        wkh8, wkl8 = _split8(wk[c * HD:(c + 1) * HD, :].T, WSCALE)
        wvh8, wvl8 = _split8(wv[c * HD:(c + 1) * HD, :].T, WSCALE)
        woh8, wol8 = _split8(wo[:, c * DQ:(c + 1) * DQ].T, WSCALE)
        wqh8, wql8 = _ptile(wqh8, DQ), _ptile(wql8, DQ)
        wkh8, wkl8 = _ptile(wkh8, HD), _ptile(wkl8, HD)
        wvh8, wvl8 = _ptile(wvh8, HD), _ptile(wvl8, HD)
        woh8, wol8 = _ptile(woh8, DIM), _ptile(wol8, DIM)
        m = {
            "xh": xh8, "xl": xl8,
            "wqh": wqh8, "wql": wql8,
            "wkh": wkh8, "wkl": wkl8,
            "wvh": wvh8, "wvl": wvl8,
            "woh": woh8, "wol": wol8,
            "cs": C.astype(BF16NP), "sn": S.astype(BF16NP),
            "psw": psw, "idn": idn,
        }
        if mkt is not None:
            m["mkt"] = mkt
        in_maps.append(m)

    res = run_bass_kernel_spmd(nc, in_maps, core_ids=list(range(NCORES)),
                               trace=TRACE)
    LAST_RESULT = res
    acc = np.zeros((DIM, SEQ), dtype=np.float64)
    for c in range(NCORES):
        acc += res.results[c]["outt"].astype(np.float64)
    return np.ascontiguousarray(acc.T).astype(np.float32).reshape(1, SEQ, DIM)


# revision 74
# speedup vs baseline: 1.0382x; 1.0002x over previous
"""GQA attention block (QKV proj + RoPE + causal attention + o_proj),
tensor-parallel over heads across 8 TRN2 NeuronCores.

Sharding: core c owns q heads [4c, 4c+4) (512 q dims), kv head c
(128 kv dims), and wo columns [512c, 512c+512). Each core computes a
full-shape partial of the output projection; the host sums the 8
partials (the "all-reduce") and transposes back.

Layout convention on device: activations are kept feature-major
([dim, seq]) so every matmul contracts over the partition axis with
no transposes:
  QT/KT [d, s]  ->  scores^T [ks, qs] = KT_tile^T . QT   (lhsT=KT, rhs=QT)
  softmax over ks = partition axis: exp on ACT, denominator via
  ones-matmul on PE, division folded into the PV output scaling
  PV: OT [dv, qs] = V_nat^T . P                           (lhsT=V, rhs=P)
  o_proj: outT [e, s] = woT^T . OT                        (lhsT=woT, rhs=OT)

Precision plan: the dense GEMMs (QKV proj, o_proj) run as fp8e4
DoubleRow matmuls (2 K-tiles contracted per instruction at 0.5
cycles/row) with a hi/lo residual split of both operands and the
three significant cross terms (hi.hi + lo.hi + hi.lo) accumulated in
fp32 PSUM - ~1.5e-3 relative error at 0.75x the bf16/fp32r cycle
cost. Weights are pre-scaled by 64 (power of two, folded back into
the PSUM->SBUF copy scale) so their hi/lo parts stay in fp8e4 normal
range; the attention output is pre-scaled by 16 (via the den "ones"
stationary = 1/16) for the same reason. q/k/v/P/scores run in bf16
(same PE rate as fp32r, half the SBUF/DMA). Output partials are
stored bf16 and summed on host.

Scheduling: weights arrive host-pretiled partition-major in a few
large staged DMAs (the HWDGE dispatch ring costs 625ns per DMA
instruction); x arrives as host-split fp8 hi/lo streams. The first
two seq chunks stream x quads interleaved with the matmuls (the DMA
pipe is saturated by weight loading there); the last two hold the
full chunk of x resident (prefetched while DMA is otherwise idle)
and run their six output tiles sequentially, each immediately
followed by its RoPE, so the RoPE chains overlap the next tile's
matmuls and attention starts without waiting on a rope tail.
Attention runs qt descending with a 2-unit score lookahead and
cross-head score pre-issue; the latency-bound qt<=2 iterations
interleave o_proj column-block emissions between units as PE filler
(gated so an o_proj chunk is only emitted after the attention chunk
feeding it is complete), with the remaining o_proj drained at deeper
PSUM rotation afterwards. In those interleaved iterations the
softmax denominator of full (non-diagonal) score pairs is computed
as a single fp8 DoubleRow matmul over a Pool-engine fp8 copy of P
(lagged two units to hide the cast), which requires EXP_BIAS to keep
exp outputs inside fp8e4 range.
"""

import sys
from contextlib import ExitStack

import numpy as np
import ml_dtypes

for _p in ("/opt/trn_rl_repo", "/opt/trn_rl_repo/concourse"):
    if _p not in sys.path:
        sys.path.insert(0, _p)

import concourse.bacc as bacc
import concourse.bass as bass
import concourse.tile as tile
from concourse import mybir
from concourse.bass_utils import run_bass_kernel_spmd

F32 = mybir.dt.float32
F32R = mybir.dt.float32r
BF16 = mybir.dt.bfloat16
F8 = mybir.dt.float8e4
E4NP = ml_dtypes.float8_e4m3
BF16NP = ml_dtypes.bfloat16
AF = mybir.ActivationFunctionType
DR = mybir.MatmulPerfMode.DoubleRow

DIM = 4096
SEQ = 2048
HD = 128          # head dim
NCORES = 8
HQ = 4            # q heads per core
DQ = HQ * HD      # 512 q dims per core
NKT = DIM // HD   # 32 contraction tiles
NPAIR = NKT // 2  # 16 DoubleRow k-tile pairs
SQT = SEQ // 512  # 4 seq chunks of 512
INV_SQRT_HD = 1.0 / np.sqrt(np.float32(HD))
EXP_BIAS = -4.0   # constant shift inside exp; cancels in softmax.
                  # -4 keeps exp outputs within fp8e4 normal range
                  # for the DoubleRow denominator path (max logit
                  # ~5.5 -> p <= e^1.5; typical p ~0.02 >> 2^-9)
WSCALE = 64.0     # weight pre-scale so fp8 hi/lo stays in normal range
OTSCALE = 16.0    # attention-output pre-scale for its fp8 hi/lo split

TRACE = False
LAST_RESULT = None

_cache = {}


def _build(mask_mode):
    """mask_mode: 'zeros' | 'causal' | 'general'."""
    nc = bacc.Bacc("TRN2", target_bir_lowering=False)
    xh = nc.dram_tensor("xh", [DIM, SEQ], F8, kind="ExternalInput")
    xl = nc.dram_tensor("xl", [DIM, SEQ], F8, kind="ExternalInput")
    # weights arrive pre-tiled partition-major: [p, (ktile m)]
    wqh = nc.dram_tensor("wqh", [HD, NKT * DQ], F8, kind="ExternalInput")
    wql = nc.dram_tensor("wql", [HD, NKT * DQ], F8, kind="ExternalInput")
    wkh = nc.dram_tensor("wkh", [HD, NKT * HD], F8, kind="ExternalInput")
    wkl = nc.dram_tensor("wkl", [HD, NKT * HD], F8, kind="ExternalInput")
    wvh = nc.dram_tensor("wvh", [HD, NKT * HD], F8, kind="ExternalInput")
    wvl = nc.dram_tensor("wvl", [HD, NKT * HD], F8, kind="ExternalInput")
    woh = nc.dram_tensor("woh", [HD, HQ * DIM], F8, kind="ExternalInput")
    wol = nc.dram_tensor("wol", [HD, HQ * DIM], F8, kind="ExternalInput")
    cs = nc.dram_tensor("cs", [HD, SEQ], BF16, kind="ExternalInput")
    sn = nc.dram_tensor("sn", [HD, SEQ], BF16, kind="ExternalInput")
    psw = nc.dram_tensor("psw", [HD, HD], F32R, kind="ExternalInput")
    idn = nc.dram_tensor("idn", [HD, HD], BF16, kind="ExternalInput")
    mkt = None
    if mask_mode == "causal":
        # 4 relative diagonal-tile masks (pattern repeats for every qt)
        mkt = nc.dram_tensor("mkt", [HD, 4 * 512], BF16, kind="ExternalInput")
    elif mask_mode == "general":
        mkt = nc.dram_tensor("mkt", [SEQ, SEQ], F32, kind="ExternalInput")
    outt = nc.dram_tensor("outt", [DIM, SEQ], BF16, kind="ExternalOutput")

    QSCALE = float(INV_SQRT_HD / WSCALE)
    KSCALE = float(1.0 / WSCALE)
    OSCALE = float(1.0 / (WSCALE * OTSCALE))

    with ExitStack() as ctx:
        tc = ctx.enter_context(tile.TileContext(nc))

        # ---- persistent pools ----
        const = ctx.enter_context(tc.tile_pool(name="const", bufs=1))
        ones_f32 = const.tile([HD, HD], F32, tag="ones32")
        # den is accumulated pre-divided by OTSCALE so inv = OTSCALE/den and
        # the attention output is scaled into fp8-friendly range for the
        # o_proj hi/lo split; the final output copy divides it back out.
        nc.vector.memset(ones_f32[:], 1.0 / OTSCALE)
        ones_sb = const.tile([HD, HD], BF16, tag="ones")
        nc.scalar.activation(ones_sb[:], ones_f32[:], AF.Copy)
        ebias = const.tile([HD, 1], F32, tag="ebias")
        nc.vector.memset(ebias[:], EXP_BIAS)
        ones8 = const.tile([HD, 2, HD], F8, tag="ones8")
        for _u in range(2):
            nc.scalar.activation(ones8[:, _u, :], ones_f32[:], AF.Copy)

        qkvpool = ctx.enter_context(tc.tile_pool(name="qkv", bufs=1))
        # per-chunk tiles so attention reads only depend on the chunks they
        # actually touch (no false whole-tile hazards on the last chunk)
        qrope = [[qkvpool.tile([HD, 512], BF16, tag=f"qr{h}_{c}",
                               name=f"qr{h}_{c}") for c in range(SQT)]
                 for h in range(HQ)]
        krope = [qkvpool.tile([HD, 512], BF16, tag=f"kr{c}", name=f"kr{c}")
                 for c in range(SQT)]
        vnat = [qkvpool.tile([HD, 512], BF16, tag=f"vn{c}", name=f"vn{c}")
                for c in range(SQT)]

        def kr_at(kst):
            return krope[kst // 4][:, (kst % 4) * HD:(kst % 4 + 1) * HD]

        def vn_at(kst):
            return vnat[kst // 4][:, (kst % 4) * HD:(kst % 4 + 1) * HD]

        # ---- phase 1: QKV projection (fp8 DoubleRow 3-term) + RoPE ----
        with ExitStack() as p1:
            wpool = p1.enter_context(tc.tile_pool(name="w1", bufs=1))
            wq_sb = [wpool.tile([HD, NKT, DQ], F8, tag=f"wq{t}", name=f"wq{t}")
                     for t in range(2)]
            wk_sb = [wpool.tile([HD, NKT, HD], F8, tag=f"wk{t}", name=f"wk{t}")
                     for t in range(2)]
            wv_sb = [wpool.tile([HD, NKT, HD], F8, tag=f"wv{t}", name=f"wv{t}")
                     for t in range(2)]
            cs_sb = wpool.tile([HD, SEQ], BF16, tag="cs")
            sn_sb = wpool.tile([HD, SEQ], BF16, tag="sn")
            psw_sb = wpool.tile([HD, HD], F32R, tag="psw")
            idn_sb = wpool.tile([HD, HD], BF16, tag="idn")

            def _wslice(dst3d, dram, m, lo, hi):
                # ktiles [lo, hi) of a [p, (k m)] pretiled weight tensor
                nc.sync.dma_start(
                    dst3d[:, lo:hi, :],
                    dram[:, lo * m:hi * m].rearrange("p (k m) -> p k m",
                                                     k=hi - lo))

            def emit_w_dma(kg):
                # batched staging: kg==0 -> ktiles 0-4 of everything (small,
                # fast first batch); kg==1 -> ktiles 4-16; kg==3 -> 16-32.
                # One DMA instruction per tensor per batch keeps the HWDGE
                # dispatch ring (625ns/instruction) off the critical path.
                def _wbatch(lo, hi):
                    for t in range(2):
                        if not (t == 0 and lo == 0):
                            _wslice(wq_sb[t], (wqh, wql)[t], DQ, lo, hi)
                        _wslice(wk_sb[t], (wkh, wkl)[t], HD, lo, hi)
                        _wslice(wv_sb[t], (wvh, wvl)[t], HD, lo, hi)

                if kg == 0:
                    _wbatch(0, 4)
                elif kg == 1:
                    _wbatch(4, 16)
                elif kg == 3:
                    _wbatch(16, NKT)
                elif kg == 5:
                    nc.sync.dma_start(psw_sb[:], psw[:])
                    nc.sync.dma_start(idn_sb[:], idn[:])
                    nc.sync.dma_start(cs_sb[:], cs[:])
                    nc.sync.dma_start(sn_sb[:], sn[:])

            xpool = p1.enter_context(tc.tile_pool(name="xstream", bufs=3))
            xchpool = p1.enter_context(tc.tile_pool(name="xch", bufs=2))
            rtmp = p1.enter_context(tc.tile_pool(name="rtmp", bufs=2))
            ps1 = p1.enter_context(tc.tile_pool(name="ps1", bufs=1, space="PSUM"))
            ps1q = p1.enter_context(tc.tile_pool(name="ps1q", bufs=4, space="PSUM"))
            ps1m = p1.enter_context(tc.tile_pool(name="ps1m", bufs=1, space="PSUM"))

            TERMS = ((0, 0), (1, 0), (0, 1))
            xch = {}

            def emit_xch_dmas(stc):
                # full-chunk x for the sequential chunks, in 8-ktile slices
                sc_ = slice(stc * 512, (stc + 1) * 512)
                tiles = [xchpool.tile([HD, NKT, 512], F8, tag=f"xch{t}",
                                      name=f"xch{t}_{stc}") for t in range(2)]
                for t, xd in ((0, xh), (1, xl)):
                    for g in range(4):
                        nc.sync.dma_start(
                            tiles[t][:, g * 8:(g + 1) * 8, :],
                            xd[g * 8 * HD:(g + 1) * 8 * HD, sc_]
                            .rearrange("(k p) m -> p k m", p=HD))
                xch[stc] = tiles

            for st in range(SQT):
                ss = slice(st * 512, (st + 1) * 512)
                pq = [ps1q.tile([HD, 512], F32, tag="pq", name=f"pq{i}")
                      for i in range(HQ)]
                pk = ps1.tile([HD, 512], F32, tag="pk")
                pv = ps1.tile([HD, 512], F32, tag="pv")

                def rope_one(src_ps, dst, dst_sl, scale, on_act):
                    raw = rtmp.tile([HD, 512], F32R, tag="qraw")
                    if on_act:
                        nc.scalar.activation(raw[:], src_ps[:], AF.Copy,
                                             scale=scale)
                    else:
                        nc.vector.tensor_scalar_mul(raw[:], src_ps[:], scale)
                    swp = ps1m.tile([HD, 512], F32, tag="psw")
                    nc.tensor.matmul(swp[:], psw_sb[:], raw[:],
                                     start=True, stop=True)
                    t1 = rtmp.tile([HD, 512], F32, tag="t1", bufs=1)
                    nc.vector.tensor_mul(t1[:], raw[:], cs_sb[:, ss])
                    t2 = rtmp.tile([HD, 512], F32, tag="t2", bufs=1)
                    nc.vector.tensor_mul(t2[:], swp[:], sn_sb[:, ss])
                    nc.vector.tensor_add(dst[:, dst_sl], t1[:], t2[:])

                def v_block():
                    # v: descale + bf16, then transpose to [seq, dv] blocks
                    vraw = rtmp.tile([HD, 512], BF16, tag="vraw", bufs=1)
                    nc.scalar.activation(vraw[:], pv[:], AF.Copy, scale=KSCALE)
                    for j in range(4):
                        vt = ps1m.tile([HD, HD], BF16, tag="pvt")
                        nc.tensor.transpose(vt[:],
                                            vraw[:, j * HD:(j + 1) * HD],
                                            idn_sb[:])
                        if j % 2 == 0:
                            nc.scalar.activation(
                                vnat[st][:, j * HD:(j + 1) * HD], vt[:],
                                AF.Copy)
                        else:
                            nc.vector.tensor_copy(
                                vnat[st][:, j * HD:(j + 1) * HD], vt[:])

                if st < 2:
                    # streaming chunks: x quads interleaved with the matmuls
                    for kg in range(NKT // 4):
                        if st == 0 and kg == 0:
                            _wslice(wq_sb[0], wqh, DQ, 0, 4)
                        xq8 = [xpool.tile([HD, 4, 512], F8, tag=f"xt{t}",
                                          name=f"xt{t}") for t in range(2)]
                        nc.sync.dma_start(
                            xq8[0][:],
                            xh[kg * 4 * HD:(kg + 1) * 4 * HD, ss]
                            .rearrange("(k p) m -> p k m", p=HD))
                        nc.sync.dma_start(
                            xq8[1][:],
                            xl[kg * 4 * HD:(kg + 1) * 4 * HD, ss]
                            .rearrange("(k p) m -> p k m", p=HD))
                        if st == 0:
                            emit_w_dma(kg)
                        if st == 1 and kg == 4:
                            emit_xch_dmas(2)
                        for j in range(2):
                            pp = kg * 2 + j       # global pair index
                            kpair = slice(2 * pp, 2 * pp + 2)
                            xsl = [x8[:, 2 * j:2 * j + 2, :] for x8 in xq8]
                            first = (kg == 0 and j == 0)
                            last = (kg == NKT // 4 - 1 and j == 1)
                            for ti, (wi, xi) in enumerate(TERMS):
                                fl = dict(start=(first and ti == 0),
                                          stop=(last and ti == 2))
                                for mt in range(HQ):
                                    msl = slice(mt * HD, (mt + 1) * HD)
                                    nc.tensor.matmul(
                                        pq[mt][:], wq_sb[wi][:, kpair, msl],
                                        xsl[xi], perf_mode=DR, **fl)
                                nc.tensor.matmul(
                                    pk[:], wk_sb[wi][:, kpair, :], xsl[xi],
                                    perf_mode=DR, **fl)
                                nc.tensor.matmul(
                                    pv[:], wv_sb[wi][:, kpair, :], xsl[xi],
                                    perf_mode=DR, **fl)
                    for mt in range(HQ):
                        rope_one(pq[mt], qrope[mt][st], slice(0, 512), QSCALE,
                                 mt % 2 == 0)
                    rope_one(pk, krope[st], slice(0, 512), KSCALE, True)
                    v_block()
                else:
                    # sequential chunks: full-chunk x already resident;
                    # each output tile immediately runs its RoPE so the
                    # chains overlap the next tile's matmuls
                    if st == 2:
                        emit_xch_dmas(3)
                    xt8 = xch.pop(st)

                    def seq_accum(ps, wsb, msl):
                        for ppi in range(NPAIR):
                            kpair = slice(2 * ppi, 2 * ppi + 2)
                            for ti, (wi, xi) in enumerate(TERMS):
                                lhs = (wsb[wi][:, kpair, msl] if msl
                                       else wsb[wi][:, kpair, :])
                                nc.tensor.matmul(
                                    ps[:], lhs, xt8[xi][:, kpair, :],
                                    perf_mode=DR,
                                    start=(ppi == 0 and ti == 0),
                                    stop=(ppi == NPAIR - 1 and ti == 2))

                    for mt in range(HQ):
                        seq_accum(pq[mt], wq_sb, slice(mt * HD, (mt + 1) * HD))
                        rope_one(pq[mt], qrope[mt][st], slice(0, 512), QSCALE,
                                 mt % 2 == 0)
                    seq_accum(pk, wk_sb, None)
                    rope_one(pk, krope[st], slice(0, 512), KSCALE, True)
                    seq_accum(pv, wv_sb, None)
                    v_block()

        # ---- phase 2: attention;  phase 3: output projection ----
        with ExitStack() as p2:
            wopool = p2.enter_context(tc.tile_pool(name="wo", bufs=1))
            wo_sb = [wopool.tile([HD, HQ, DIM], F8, tag=f"wo{t}", name=f"wo{t}")
                     for t in range(2)]
            wo_dma_emitted = [False]

            def emit_wo_dmas():
                if not wo_dma_emitted[0]:
                    wo_dma_emitted[0] = True
                    nc.sync.dma_start(
                        wo_sb[0][:], woh[:].rearrange("p (k m) -> p k m", k=HQ))
                    nc.sync.dma_start(
                        wo_sb[1][:], wol[:].rearrange("p (k m) -> p k m", k=HQ))

            otpool = p2.enter_context(tc.tile_pool(name="ot", bufs=1))
            # attention output per head, fp8 hi/lo split for the o_proj
            ot8 = [otpool.tile([HD, HQ, SEQ], F8, tag=f"ot8{t}", name=f"ot8{t}")
                   for t in range(2)]

            mpool = p2.enter_context(tc.tile_pool(name="mk", bufs=1))
            spool = p2.enter_context(tc.tile_pool(name="sp", bufs=3))

            mk_sb = None
            if mask_mode == "causal":
                mk_sb = mpool.tile([HD, 4, 512], BF16, tag="mkd")
                nc.sync.dma_start(
                    mk_sb[:], mkt[:].rearrange("p (k m) -> p k m", k=4))

            gen_masks = {}

            def emit_gen_masks(qt):
                qs = slice(qt * 512, (qt + 1) * 512)
                out = {}
                for kst in range(16):
                    m = mpool.tile([HD, 512], F32, tag=f"mk{kst}",
                                   name=f"mk{kst}")
                    nc.sync.dma_start(
                        m[:], mkt[kst * HD:(kst + 1) * HD, qs])
                    out[kst] = m
                return out

            def npair_of(qt):
                return 2 * qt if mask_mode == "causal" else 8

            def nunit_of(qt):
                return npair_of(qt) + (4 if mask_mode == "causal" else 0)

            def issue_scores_for(qt, h, i, ps2):
                npair = npair_of(qt)
                qs = slice(qt * 512, (qt + 1) * 512)
                sp = ps2.tile([HD, 1024], F32, tag="pst")
                if i < npair:
                    for u in range(2):
                        kst = 2 * i + u
                        nc.tensor.matmul(
                            sp[:, u * 512:(u + 1) * 512],
                            kr_at(kst),
                            qrope[h][qt][:],
                            start=True, stop=True)
                else:
                    # diagonal tile, columns < c0 fully masked
                    r = i - npair
                    kst = 4 * qt + r
                    c0 = r * HD
                    nc.tensor.matmul(
                        sp[:, c0:512],
                        kr_at(kst),
                        qrope[h][qt][:, c0:512],
                        start=True, stop=True)
                return sp

            def issue_exp_for(qt, i, sp, ppool):
                npair = npair_of(qt)
                pb = ppool.tile([HD, 1024], BF16, tag="pexp")
                if i < npair:
                    if mask_mode == "general":
                        tmp = ppool.tile([HD, 1024], F32, tag="padd", bufs=2)
                        for u in range(2):
                            usl = slice(u * 512, (u + 1) * 512)
                            nc.vector.tensor_add(
                                tmp[:, usl], sp[:, usl],
                                gen_masks[qt][2 * i + u][:])
                        nc.scalar.activation(pb[:], tmp[:], AF.Exp,
                                             bias=ebias[:])
                    else:
                        nc.scalar.activation(pb[:], sp[:], AF.Exp,
                                             bias=ebias[:])
                else:
                    r = i - npair
                    c0 = r * HD
                    tmp = ppool.tile([HD, 1024], F32, tag="padd", bufs=2)
                    nc.vector.tensor_add(
                        tmp[:, c0:512], sp[:, c0:512], mk_sb[:, r, c0:])
                    nc.scalar.activation(pb[:, c0:512], tmp[:, c0:512],
                                         AF.Exp, bias=ebias[:])
                return pb

            pre_store = {}

            def attn_iter(qt, h, ps2, ps2a, ppool, filler,
                          prescore_next=None, lookahead=2, p8pool=None):
                qs = slice(qt * 512, (qt + 1) * 512)
                npair = npair_of(qt)
                nunit = nunit_of(qt)
                sps = [None] * nunit
                pbs = [None] * nunit

                pre = pre_store.pop((qt, h), None)
                if pre is not None:
                    sps[0], sps[1] = pre
                    if lookahead > 2 and nunit > 2:
                        sps[2] = issue_scores_for(qt, h, 2, ps2)
                else:
                    for j in range(min(lookahead, nunit)):
                        sps[j] = issue_scores_for(qt, h, j, ps2)

                den = ps2a.tile([HD, 512], F32, tag="pden")
                otp = ps2a.tile([HD, 512], F32, tag="pot")
                # den_dr: non-diag pair units compute den as one fp8
                # DoubleRow matmul over a Pool-engine fp8 copy of P (lagged
                # one unit to hide the cast latency)
                den_dr = p8pool is not None and npair > 0
                pend = []

                def flush_den_dr(keep=0):
                    while len(pend) > keep:
                        p8t, first = pend.pop(0)
                        nc.tensor.matmul(
                            den[:], ones8[:],
                            p8t[:].rearrange("p (u m) -> p u m", u=2),
                            perf_mode=DR, start=first, stop=False)

                for i in range(nunit):
                    if lookahead + i < nunit and sps[lookahead + i] is None:
                        sps[lookahead + i] = issue_scores_for(
                            qt, h, lookahead + i, ps2)
                    pbs[i] = issue_exp_for(qt, i, sps[i], ppool)
                    fl_last = (i == nunit - 1)
                    if i < npair:
                        if den_dr:
                            p8t = p8pool.tile([HD, 1024], F8, tag="p8")
                            nc.gpsimd.tensor_copy(p8t[:], pbs[i][:])
                        for u in range(2):
                            kst = 2 * i + u
                            fl = dict(
                                start=(i == 0 and u == 0),
                                stop=(fl_last and u == 1))
                            pr = pbs[i][:, u * 512:(u + 1) * 512]
                            if not den_dr:
                                nc.tensor.matmul(
                                    den[:], ones_sb[:], pr, **fl)
                            nc.tensor.matmul(
                                otp[:], vn_at(kst), pr, **fl)
                        if den_dr:
                            flush_den_dr(keep=1)
                            pend.append((p8t, i == 0))
                    else:
                        r = i - npair
                        kst = 4 * qt + r
                        c0 = r * HD
                        if den_dr:
                            flush_den_dr()
                        fl = dict(start=(i == 0), stop=fl_last)
                        pr = pbs[i][:, c0:512]
                        nc.tensor.matmul(
                            den[:, c0:], ones_sb[:], pr,
                            start=(i == 0 and not den_dr), stop=fl_last)
                        nc.tensor.matmul(
                            otp[:, c0:], vn_at(kst), pr, **fl)
                    if fl_last and prescore_next is not None:
                        # pre-issue the next iteration's first two score
                        # units so its exp pipeline starts before this
                        # iteration's DVE drain
                        qn, hn = prescore_next
                        pre_store[(qn, hn)] = (
                            issue_scores_for(qn, hn, 0, ps2),
                            issue_scores_for(qn, hn, 1, ps2))
                    if filler is not None:
                        filler()
                inv = spool.tile([HD, 512], F32, tag="inv")
                nc.vector.reciprocal(inv[:], den[:])
                ots = spool.tile([HD, 512], F32, tag="ots")
                nc.vector.tensor_mul(ots[:], otp[:], inv[:])
                # fp8 hi/lo split of the attention output
                nc.scalar.activation(ot8[0][:, h, qs], ots[:], AF.Copy)
                nc.vector.tensor_sub(ot8[1][:, h, qs], ots[:],
                                     ot8[0][:, h, qs])
                if filler is not None:
                    filler()

            # ---- phase 3 emitter: o_proj (fp8 DoubleRow 3-term), one
            # [128,512] column block per generator step so it can be
            # interleaved into the attention tail as PE filler work ----
            OTERMS = ((0, 0), (1, 0), (0, 1))

            def oproj_units(sts, ps3, opool, egs=None):
                for st in sts:
                    ss = slice(st * 512, (st + 1) * 512)
                    for eg in (range(DIM // HD // 4) if egs is None
                               else egs):
                        last_grp = (st == 0 and eg == DIM // HD // 4 - 1)
                        ocp = opool.tile([HD, 4, 512], BF16, tag="ocp")
                        for ej in range(4):
                            et = eg * 4 + ej
                            esl = slice(et * HD, (et + 1) * HD)
                            po = ps3.tile([HD, 512], F32, tag="po")
                            for pi in range(2):
                                hpair = slice(2 * pi, 2 * pi + 2)
                                for ti, (wi, oi) in enumerate(OTERMS):
                                    nc.tensor.matmul(
                                        po[:],
                                        wo_sb[wi][:, hpair, esl],
                                        ot8[oi][:, hpair, ss],
                                        perf_mode=DR,
                                        start=(pi == 0 and ti == 0),
                                        stop=(pi == 1 and ti == 2),
                                    )
                            osl = ocp[:, ej, :]
                            if ej % 2 == 0:
                                nc.scalar.activation(osl, po[:], AF.Copy,
                                                     scale=OSCALE)
                            else:
                                nc.vector.tensor_scalar_mul(osl, po[:], OSCALE)
                            if last_grp:
                                # final tiles: store per-slice so the last
                                # DMA isn't gated on all four copies
                                nc.sync.dma_start(
                                    outt[et * HD:(et + 1) * HD, ss], osl)
                            yield
                        if not last_grp:
                            nc.sync.dma_start(
                                outt[eg * 4 * HD:(eg + 1) * 4 * HD, ss]
                                .rearrange("(e p) m -> p e m", p=HD),
                                ocp[:])

            if mask_mode == "causal":
                with ExitStack() as patt:
                    ppool = patt.enter_context(tc.tile_pool(name="pp", bufs=5))
                    ps2 = patt.enter_context(
                        tc.tile_pool(name="ps2", bufs=3, space="PSUM"))
                    ps2a = patt.enter_context(
                        tc.tile_pool(name="ps2a", bufs=1, space="PSUM"))
                    emit_wo_dmas()
                    for h in range(HQ):
                        nxt = (3, h + 1) if h + 1 < HQ else None
                        attn_iter(3, h, ps2, ps2a, ppool, None,
                                  prescore_next=nxt)
                # tail: interleave o_proj units into the latency-bound
                # qt=1/qt=0 iterations
                with ExitStack() as ptail:
                    ppool2 = ptail.enter_context(
                        tc.tile_pool(name="pp2", bufs=6))
                    ps2t = ptail.enter_context(
                        tc.tile_pool(name="ps2t", bufs=2, space="PSUM"))
                    ps2a2 = ptail.enter_context(
                        tc.tile_pool(name="ps2a2", bufs=1, space="PSUM"))
                    ps3 = ptail.enter_context(
                        tc.tile_pool(name="ps3", bufs=2, space="PSUM"))
                    opool = ptail.enter_context(
                        tc.tile_pool(name="ostage", bufs=4))
                    gen = oproj_units((3, 2), ps3, opool)
                    # st=3 units (32) are ready once qt=3 is done; st=2
                    # units must wait until all of qt=2 has been emitted
                    pulled = [0]
                    limit = [32]
                    _done = object()

                    def filler_gen():
                        if pulled[0] < limit[0]:
                            if next(gen, _done) is not _done:
                                pulled[0] += 1

                    p8pool = ptail.enter_context(
                        tc.tile_pool(name="p8p", bufs=6))
                    seq = [(qt, h) for qt in (2, 1, 0) for h in range(HQ)]
                    for n, (qt, h) in enumerate(seq[:8]):
                        attn_iter(qt, h, ps2t, ps2a2, ppool2, filler_gen,
                                  prescore_next=seq[n + 1], p8pool=p8pool)
                        if (qt, h) == (2, HQ - 1):
                            limit[0] = 64
                    gen2 = oproj_units((1,), ps3, opool, egs=range(0, 4))

                    def filler_tail():
                        if next(gen, _done) is _done:
                            next(gen2, None)

                    for h in range(HQ):
                        nxt = (0, h + 1) if h + 1 < HQ else None
                        attn_iter(0, h, ps2t, ps2a2, ppool2, filler_tail,
                                  prescore_next=nxt)
                    for _ in gen:
                        pass
                    for _ in gen2:
                        pass
                # bulk o_proj drain with deep PSUM rotation
                with ExitStack() as p3d:
                    ps3d = p3d.enter_context(
                        tc.tile_pool(name="ps3d", bufs=4, space="PSUM"))
                    opool2 = p3d.enter_context(
                        tc.tile_pool(name="ostage2", bufs=3))
                    for _ in oproj_units((1,), ps3d, opool2,
                                         egs=range(4, 8)):
                        pass
                    for _ in oproj_units((0,), ps3d, opool2):
                        pass
            else:
                with ExitStack() as patt:
                    ppool = patt.enter_context(tc.tile_pool(name="pp", bufs=5))
                    ps2 = patt.enter_context(
                        tc.tile_pool(name="ps2", bufs=3, space="PSUM"))
                    ps2a = patt.enter_context(
                        tc.tile_pool(name="ps2a", bufs=1, space="PSUM"))
                    emit_wo_dmas()
                    for qt in range(SQT - 1, -1, -1):
                        if mask_mode == "general" and qt not in gen_masks:
                            gen_masks[qt] = emit_gen_masks(qt)
                        for h in range(HQ):
                            attn_iter(qt, h, ps2, ps2a, ppool, None)
                    pre_store.clear()
                with ExitStack() as p3:
                    ps3 = p3.enter_context(
                        tc.tile_pool(name="ps3", bufs=4, space="PSUM"))
                    opool = p3.enter_context(
                        tc.tile_pool(name="ostage", bufs=3))
                    for _ in oproj_units((3, 2, 1, 0), ps3, opool):
                        pass

    nc.compile()
    return nc


def _split8(a, scale=1.0):
    s = np.clip(a * np.float32(scale), -224.0, 224.0)
    hi = s.astype(E4NP)
    lo = np.clip(s - hi.astype(np.float32), -224.0, 224.0).astype(E4NP)
    return np.ascontiguousarray(hi), np.ascontiguousarray(lo)


def _prep_consts(freqs_cos, freqs_sin):
    cos = np.asarray(freqs_cos, dtype=np.float32)
    sin = np.asarray(freqs_sin, dtype=np.float32)
    C = np.empty((HD, SEQ), np.float32)
    S = np.empty((HD, SEQ), np.float32)
    C[0::2] = cos.T
    C[1::2] = cos.T
    S[0::2] = -sin.T
    S[1::2] = sin.T
    psw = np.zeros((HD, HD), np.float32)
    j = np.arange(0, HD, 2)
    psw[j + 1, j] = 1.0
    psw[j, j + 1] = 1.0
    idn = np.eye(HD, dtype=np.float32).astype(BF16NP)
    return C, S, psw, idn


def _mask_mode(mask):
    if not mask.any():
        return "zeros"
    neg = mask.min()
    tril = np.tril(np.ones((SEQ, SEQ), dtype=bool))
    if neg <= -1e8 and not mask[tril].any() and np.all(mask[~tril] == neg):
        return "causal"
    return "general"


def kernel(x, wq, wk, wv, wo, freqs_cos, freqs_sin, mask, start_pos):
    global LAST_RESULT
    assert int(start_pos) == 0, "kernel hardcodes start_pos=0 (full prefill)"
    x = np.asarray(x, dtype=np.float32)
    wq = np.asarray(wq, dtype=np.float32)
    wk = np.asarray(wk, dtype=np.float32)
    wv = np.asarray(wv, dtype=np.float32)
    wo = np.asarray(wo, dtype=np.float32)
    mask = np.asarray(mask, dtype=np.float32)

    mode = _mask_mode(mask)
    if mode not in _cache:
        _cache[mode] = _build(mode)
    nc = _cache[mode]

    xt = np.ascontiguousarray(x.reshape(SEQ, DIM).T)
    xh8, xl8 = _split8(xt)
    C, S, psw, idn = _prep_consts(freqs_cos, freqs_sin)
    mkt = None
    if mode == "causal":
        # 4 relative diagonal tile masks: tile r is mask.T[r*128:(r+1)*128,
        # 0:512] (the pattern depends only on kst - 4*qt)
        mt = np.ascontiguousarray(mask.T[:512, :512])
        mkt = np.concatenate([mt[r * HD:(r + 1) * HD, :] for r in range(4)],
                             axis=1)
        mkt = np.ascontiguousarray(mkt).astype(BF16NP)
    elif mode == "general":
        mkt = np.ascontiguousarray(mask.T)

    def _ptile(a, m):
        # [DIM_contract, m] -> partition-major [128, (ktile m)]
        k = a.shape[0] // HD
        return np.ascontiguousarray(
            a.reshape(k, HD, m).transpose(1, 0, 2).reshape(HD, k * m))

    in_maps = []
    for c in range(NCORES):
        wqh8, wql8 = _split8(wq[c * DQ:(c + 1) * DQ, :].T, WSCALE)
        wkh8, wkl8 = _split8(wk[c * HD:(c + 1) * HD, :].T, WSCALE)
        wvh8, wvl8 = _split8(wv[c * HD:(c + 1) * HD, :].T, WSCALE)
        woh8, wol8 = _split8(wo[:, c * DQ:(c + 1) * DQ].T, WSCALE)
        wqh8, wql8 = _ptile(wqh8, DQ), _ptile(wql8, DQ)
        wkh8, wkl8 = _ptile(wkh8, HD), _ptile(wkl8, HD)
        wvh8, wvl8 = _ptile(wvh8, HD), _ptile(wvl8, HD)
        woh8, wol8 = _ptile(woh8, DIM), _ptile(wol8, DIM)
        m = {
            "xh": xh8, "xl": xl8,
            "wqh": wqh8, "wql": wql8,
            "wkh": wkh8, "wkl": wkl8,
            "wvh": wvh8, "wvl": wvl8,
            "woh": woh8, "wol": wol8,
            "cs": C.astype(BF16NP), "sn": S.astype(BF16NP),
            "psw": psw, "idn": idn,
        }
        if mkt is not None:
            m["mkt"] = mkt
        in_maps.append(m)

    res = run_bass_kernel_spmd(nc, in_maps, core_ids=list(range(NCORES)),
                               trace=TRACE)
    LAST_RESULT = res
    acc = np.zeros((DIM, SEQ), dtype=np.float64)
    for c in range(NCORES):
        acc += res.results[c]["outt"].astype(np.float64)
    return np.ascontiguousarray(acc.T).astype(np.float32).reshape(1, SEQ, DIM)


# revision 80
# speedup vs baseline: 1.0430x; 1.0047x over previous
"""GQA attention block (QKV proj + RoPE + causal attention + o_proj),
tensor-parallel over heads across 8 TRN2 NeuronCores.

Sharding: core c owns q heads [4c, 4c+4) (512 q dims), kv head c
(128 kv dims), and wo columns [512c, 512c+512). Each core computes a
full-shape partial of the output projection; the host sums the 8
partials (the "all-reduce") and transposes back.

Layout convention on device: activations are kept feature-major
([dim, seq]) so every matmul contracts over the partition axis with
no transposes:
  QT/KT [d, s]  ->  scores^T [ks, qs] = KT_tile^T . QT   (lhsT=KT, rhs=QT)
  softmax over ks = partition axis: exp on ACT, denominator via
  ones-matmul on PE, division folded into the PV output scaling
  PV: OT [dv, qs] = V_nat^T . P                           (lhsT=V, rhs=P)
  o_proj: outT [e, s] = woT^T . OT                        (lhsT=woT, rhs=OT)

Precision plan: the dense GEMMs (QKV proj, o_proj) run as fp8e4
DoubleRow matmuls (2 K-tiles contracted per instruction at 0.5
cycles/row) with a hi/lo residual split of both operands and the
three significant cross terms (hi.hi + lo.hi + hi.lo) accumulated in
fp32 PSUM - ~1.5e-3 relative error at 0.75x the bf16/fp32r cycle
cost. Weights are pre-scaled by 64 (power of two, folded back into
the PSUM->SBUF copy scale) so their hi/lo parts stay in fp8e4 normal
range; the attention output is pre-scaled by 16 (via the den "ones"
stationary = 1/16) for the same reason. q/k/v/P/scores run in bf16
(same PE rate as fp32r, half the SBUF/DMA). Output partials are
stored bf16 and summed on host.

Scheduling: weights arrive host-pretiled partition-major in a few
large staged DMAs (the HWDGE dispatch ring costs 625ns per DMA
instruction); x arrives as host-split fp8 hi/lo streams. The first
two seq chunks stream x quads interleaved with the matmuls (the DMA
pipe is saturated by weight loading there); the last two hold the
full chunk of x resident (prefetched while DMA is otherwise idle)
and run their six output tiles sequentially, each immediately
followed by its RoPE, so the RoPE chains overlap the next tile's
matmuls and attention starts without waiting on a rope tail.
Attention runs qt descending with a 2-unit score lookahead and
cross-head score pre-issue; the latency-bound qt<=2 iterations
interleave o_proj column-block emissions between units as PE filler
(gated so an o_proj chunk is only emitted after the attention chunk
feeding it is complete), with the remaining o_proj drained at deeper
PSUM rotation afterwards. In those interleaved iterations the
softmax denominator of full (non-diagonal) score pairs is computed
as a single fp8 DoubleRow matmul over a Pool-engine fp8 copy of P
(lagged two units to hide the cast), which requires EXP_BIAS to keep
exp outputs inside fp8e4 range.
"""

import sys
from contextlib import ExitStack

import numpy as np
import ml_dtypes

for _p in ("/opt/trn_rl_repo", "/opt/trn_rl_repo/concourse"):
    if _p not in sys.path:
        sys.path.insert(0, _p)

import concourse.bacc as bacc
import concourse.bass as bass
import concourse.tile as tile
from concourse import mybir
from concourse.bass_utils import run_bass_kernel_spmd

F32 = mybir.dt.float32
F32R = mybir.dt.float32r
BF16 = mybir.dt.bfloat16
F8 = mybir.dt.float8e4
E4NP = ml_dtypes.float8_e4m3
BF16NP = ml_dtypes.bfloat16
AF = mybir.ActivationFunctionType
DR = mybir.MatmulPerfMode.DoubleRow

DIM = 4096
SEQ = 2048
HD = 128          # head dim
NCORES = 8
HQ = 4            # q heads per core
DQ = HQ * HD      # 512 q dims per core
NKT = DIM // HD   # 32 contraction tiles
NPAIR = NKT // 2  # 16 DoubleRow k-tile pairs
SQT = SEQ // 512  # 4 seq chunks of 512
INV_SQRT_HD = 1.0 / np.sqrt(np.float32(HD))
EXP_BIAS = -4.0   # constant shift inside exp; cancels in softmax.
                  # -4 keeps exp outputs within fp8e4 normal range
                  # for the DoubleRow denominator path (max logit
                  # ~5.5 -> p <= e^1.5; typical p ~0.02 >> 2^-9)
WSCALE = 64.0     # weight pre-scale so fp8 hi/lo stays in normal range
OTSCALE = 16.0    # attention-output pre-scale for its fp8 hi/lo split

TRACE = False
LAST_RESULT = None

_cache = {}


def _build(mask_mode):
    """mask_mode: 'zeros' | 'causal' | 'general'."""
    nc = bacc.Bacc("TRN2", target_bir_lowering=False)
    xh = nc.dram_tensor("xh", [DIM, SEQ], F8, kind="ExternalInput")
    xl = nc.dram_tensor("xl", [DIM, SEQ], F8, kind="ExternalInput")
    # weights arrive pre-tiled partition-major: [p, (ktile m)]
    wqh = nc.dram_tensor("wqh", [HD, NKT * DQ], F8, kind="ExternalInput")
    wql = nc.dram_tensor("wql", [HD, NKT * DQ], F8, kind="ExternalInput")
    wkh = nc.dram_tensor("wkh", [HD, NKT * HD], F8, kind="ExternalInput")
    wkl = nc.dram_tensor("wkl", [HD, NKT * HD], F8, kind="ExternalInput")
    wvh = nc.dram_tensor("wvh", [HD, NKT * HD], F8, kind="ExternalInput")
    wvl = nc.dram_tensor("wvl", [HD, NKT * HD], F8, kind="ExternalInput")
    woh = nc.dram_tensor("woh", [HD, HQ * DIM], F8, kind="ExternalInput")
    wol = nc.dram_tensor("wol", [HD, HQ * DIM], F8, kind="ExternalInput")
    cs = nc.dram_tensor("cs", [HD, SEQ], BF16, kind="ExternalInput")
    sn = nc.dram_tensor("sn", [HD, SEQ], BF16, kind="ExternalInput")
    psw = nc.dram_tensor("psw", [HD, HD], F32R, kind="ExternalInput")
    idn = nc.dram_tensor("idn", [HD, HD], BF16, kind="ExternalInput")
    mkt = None
    if mask_mode == "causal":
        # 4 relative diagonal-tile masks (pattern repeats for every qt)
        mkt = nc.dram_tensor("mkt", [HD, 4 * 512], BF16, kind="ExternalInput")
    elif mask_mode == "general":
        mkt = nc.dram_tensor("mkt", [SEQ, SEQ], F32, kind="ExternalInput")
    outt = nc.dram_tensor("outt", [DIM, SEQ], BF16, kind="ExternalOutput")

    QSCALE = float(INV_SQRT_HD / WSCALE)
    KSCALE = float(1.0 / WSCALE)
    OSCALE = float(1.0 / (WSCALE * OTSCALE))

    with ExitStack() as ctx:
        tc = ctx.enter_context(tile.TileContext(nc))

        # ---- persistent pools ----
        const = ctx.enter_context(tc.tile_pool(name="const", bufs=1))
        ones_f32 = const.tile([HD, HD], F32, tag="ones32")
        # den is accumulated pre-divided by OTSCALE so inv = OTSCALE/den and
        # the attention output is scaled into fp8-friendly range for the
        # o_proj hi/lo split; the final output copy divides it back out.
        nc.vector.memset(ones_f32[:], 1.0 / OTSCALE)
        ones_sb = const.tile([HD, HD], BF16, tag="ones")
        nc.scalar.activation(ones_sb[:], ones_f32[:], AF.Copy)
        ebias = const.tile([HD, 1], F32, tag="ebias")
        nc.vector.memset(ebias[:], EXP_BIAS)
        ones8 = const.tile([HD, 2, HD], F8, tag="ones8")
        for _u in range(2):
            nc.scalar.activation(ones8[:, _u, :], ones_f32[:], AF.Copy)

        qkvpool = ctx.enter_context(tc.tile_pool(name="qkv", bufs=1))
        # per-chunk tiles so attention reads only depend on the chunks they
        # actually touch (no false whole-tile hazards on the last chunk)
        qrope = [[qkvpool.tile([HD, 512], BF16, tag=f"qr{h}_{c}",
                               name=f"qr{h}_{c}") for c in range(SQT)]
                 for h in range(HQ)]
        krope = [qkvpool.tile([HD, 512], BF16, tag=f"kr{c}", name=f"kr{c}")
                 for c in range(SQT)]
        vnat = [qkvpool.tile([HD, 512], BF16, tag=f"vn{c}", name=f"vn{c}")
                for c in range(SQT)]

        def kr_at(kst):
            return krope[kst // 4][:, (kst % 4) * HD:(kst % 4 + 1) * HD]

        def vn_at(kst):
            return vnat[kst // 4][:, (kst % 4) * HD:(kst % 4 + 1) * HD]

        # ---- phase 1: QKV projection (fp8 DoubleRow 3-term) + RoPE ----
        with ExitStack() as p1:
            wpool = p1.enter_context(tc.tile_pool(name="w1", bufs=1))
            wq_sb = [wpool.tile([HD, NKT, DQ], F8, tag=f"wq{t}", name=f"wq{t}")
                     for t in range(2)]
            wk_sb = [wpool.tile([HD, NKT, HD], F8, tag=f"wk{t}", name=f"wk{t}")
                     for t in range(2)]
            wv_sb = [wpool.tile([HD, NKT, HD], F8, tag=f"wv{t}", name=f"wv{t}")
                     for t in range(2)]
            cs_sb = wpool.tile([HD, SEQ], BF16, tag="cs")
            sn_sb = wpool.tile([HD, SEQ], BF16, tag="sn")
            psw_sb = wpool.tile([HD, HD], F32R, tag="psw")
            idn_sb = wpool.tile([HD, HD], BF16, tag="idn")

            def _wslice(dst3d, dram, m, lo, hi):
                # ktiles [lo, hi) of a [p, (k m)] pretiled weight tensor
                nc.sync.dma_start(
                    dst3d[:, lo:hi, :],
                    dram[:, lo * m:hi * m].rearrange("p (k m) -> p k m",
                                                     k=hi - lo))

            def emit_w_dma(kg):
                # batched staging: kg==0 -> ktiles 0-4 of everything (small,
                # fast first batch); kg==1 -> ktiles 4-16; kg==3 -> 16-32.
                # One DMA instruction per tensor per batch keeps the HWDGE
                # dispatch ring (625ns/instruction) off the critical path.
                def _wbatch(lo, hi):
                    for t in range(2):
                        if not (t == 0 and lo == 0):
                            _wslice(wq_sb[t], (wqh, wql)[t], DQ, lo, hi)
                        _wslice(wk_sb[t], (wkh, wkl)[t], HD, lo, hi)
                        _wslice(wv_sb[t], (wvh, wvl)[t], HD, lo, hi)

                if kg == 0:
                    _wbatch(0, 4)
                elif kg == 1:
                    _wbatch(4, 16)
                elif kg == 3:
                    for t in range(2):
                        _wslice(wq_sb[t], (wqh, wql)[t], DQ, 16, NKT)
                elif kg == 4:
                    for t in range(2):
                        _wslice(wk_sb[t], (wkh, wkl)[t], HD, 16, NKT)
                        _wslice(wv_sb[t], (wvh, wvl)[t], HD, 16, NKT)
                elif kg == 5:
                    nc.sync.dma_start(psw_sb[:], psw[:])
                    nc.sync.dma_start(idn_sb[:], idn[:])
                    nc.sync.dma_start(cs_sb[:], cs[:])
                    nc.sync.dma_start(sn_sb[:], sn[:])

            xpool = p1.enter_context(tc.tile_pool(name="xstream", bufs=3))
            xchpool = p1.enter_context(tc.tile_pool(name="xch", bufs=2))
            rtmp = p1.enter_context(tc.tile_pool(name="rtmp", bufs=2))
            ps1 = p1.enter_context(tc.tile_pool(name="ps1", bufs=1, space="PSUM"))
            ps1q = p1.enter_context(tc.tile_pool(name="ps1q", bufs=4, space="PSUM"))
            ps1m = p1.enter_context(tc.tile_pool(name="ps1m", bufs=1, space="PSUM"))

            TERMS = ((0, 0), (1, 0), (0, 1))
            xch = {}

            def emit_xch_dmas(stc):
                # full-chunk x for the sequential chunks, in 8-ktile slices
                sc_ = slice(stc * 512, (stc + 1) * 512)
                tiles = [xchpool.tile([HD, NKT, 512], F8, tag=f"xch{t}",
                                      name=f"xch{t}_{stc}") for t in range(2)]
                for t, xd in ((0, xh), (1, xl)):
                    for g in range(4):
                        nc.sync.dma_start(
                            tiles[t][:, g * 8:(g + 1) * 8, :],
                            xd[g * 8 * HD:(g + 1) * 8 * HD, sc_]
                            .rearrange("(k p) m -> p k m", p=HD))
                xch[stc] = tiles

            for st in range(SQT):
                ss = slice(st * 512, (st + 1) * 512)
                pq = [ps1q.tile([HD, 512], F32, tag="pq", name=f"pq{i}")
                      for i in range(HQ)]
                pk = ps1.tile([HD, 512], F32, tag="pk")
                pv = ps1.tile([HD, 512], F32, tag="pv")

                def rope_one(src_ps, dst, dst_sl, scale, on_act):
                    raw = rtmp.tile([HD, 512], F32R, tag="qraw")
                    if on_act:
                        nc.scalar.activation(raw[:], src_ps[:], AF.Copy,
                                             scale=scale)
                    else:
                        nc.vector.tensor_scalar_mul(raw[:], src_ps[:], scale)
                    swp = ps1m.tile([HD, 512], F32, tag="psw")
                    nc.tensor.matmul(swp[:], psw_sb[:], raw[:],
                                     start=True, stop=True)
                    t1 = rtmp.tile([HD, 512], F32, tag="t1", bufs=1)
                    nc.vector.tensor_mul(t1[:], raw[:], cs_sb[:, ss])
                    t2 = rtmp.tile([HD, 512], F32, tag="t2", bufs=1)
                    nc.vector.tensor_mul(t2[:], swp[:], sn_sb[:, ss])
                    nc.vector.tensor_add(dst[:, dst_sl], t1[:], t2[:])

                def v_block():
                    # v: descale + bf16, then transpose to [seq, dv] blocks
                    vraw = rtmp.tile([HD, 512], BF16, tag="vraw", bufs=1)
                    nc.scalar.activation(vraw[:], pv[:], AF.Copy, scale=KSCALE)
                    for j in range(4):
                        vt = ps1m.tile([HD, HD], BF16, tag="pvt")
                        nc.tensor.transpose(vt[:],
                                            vraw[:, j * HD:(j + 1) * HD],
                                            idn_sb[:])
                        if j % 2 == 0:
                            nc.scalar.activation(
                                vnat[st][:, j * HD:(j + 1) * HD], vt[:],
                                AF.Copy)
                        else:
                            nc.vector.tensor_copy(
                                vnat[st][:, j * HD:(j + 1) * HD], vt[:])

                if st < 2:
                    # streaming chunks: x quads interleaved with the matmuls
                    for kg in range(NKT // 4):
                        if st == 0 and kg == 0:
                            _wslice(wq_sb[0], wqh, DQ, 0, 4)
                        xq8 = [xpool.tile([HD, 4, 512], F8, tag=f"xt{t}",
                                          name=f"xt{t}") for t in range(2)]
                        nc.sync.dma_start(
                            xq8[0][:],
                            xh[kg * 4 * HD:(kg + 1) * 4 * HD, ss]
                            .rearrange("(k p) m -> p k m", p=HD))
                        nc.sync.dma_start(
                            xq8[1][:],
                            xl[kg * 4 * HD:(kg + 1) * 4 * HD, ss]
                            .rearrange("(k p) m -> p k m", p=HD))
                        if st == 0:
                            emit_w_dma(kg)
                        if st == 1 and kg == 4:
                            emit_xch_dmas(2)
                        for j in range(2):
                            pp = kg * 2 + j       # global pair index
                            kpair = slice(2 * pp, 2 * pp + 2)
                            xsl = [x8[:, 2 * j:2 * j + 2, :] for x8 in xq8]
                            first = (kg == 0 and j == 0)
                            last = (kg == NKT // 4 - 1 and j == 1)
                            for ti, (wi, xi) in enumerate(TERMS):
                                fl = dict(start=(first and ti == 0),
                                          stop=(last and ti == 2))
                                for mt in range(HQ):
                                    msl = slice(mt * HD, (mt + 1) * HD)
                                    nc.tensor.matmul(
                                        pq[mt][:], wq_sb[wi][:, kpair, msl],
                                        xsl[xi], perf_mode=DR, **fl)
                                nc.tensor.matmul(
                                    pk[:], wk_sb[wi][:, kpair, :], xsl[xi],
                                    perf_mode=DR, **fl)
                                nc.tensor.matmul(
                                    pv[:], wv_sb[wi][:, kpair, :], xsl[xi],
                                    perf_mode=DR, **fl)
                    for mt in range(HQ):
                        rope_one(pq[mt], qrope[mt][st], slice(0, 512), QSCALE,
                                 mt % 2 == 0)
                    rope_one(pk, krope[st], slice(0, 512), KSCALE, True)
                    v_block()
                else:
                    # sequential chunks: full-chunk x already resident;
                    # each output tile immediately runs its RoPE so the
                    # chains overlap the next tile's matmuls
                    if st == 2:
                        emit_xch_dmas(3)
                    xt8 = xch.pop(st)

                    def seq_accum(ps, wsb, msl):
                        for ppi in range(NPAIR):
                            kpair = slice(2 * ppi, 2 * ppi + 2)
                            for ti, (wi, xi) in enumerate(TERMS):
                                lhs = (wsb[wi][:, kpair, msl] if msl
                                       else wsb[wi][:, kpair, :])
                                nc.tensor.matmul(
                                    ps[:], lhs, xt8[xi][:, kpair, :],
                                    perf_mode=DR,
                                    start=(ppi == 0 and ti == 0),
                                    stop=(ppi == NPAIR - 1 and ti == 2))

                    for mt in range(HQ):
                        seq_accum(pq[mt], wq_sb, slice(mt * HD, (mt + 1) * HD))
                        rope_one(pq[mt], qrope[mt][st], slice(0, 512), QSCALE,
                                 mt % 2 == 0)
                    seq_accum(pk, wk_sb, None)
                    rope_one(pk, krope[st], slice(0, 512), KSCALE, True)
                    seq_accum(pv, wv_sb, None)
                    v_block()

        # ---- phase 2: attention;  phase 3: output projection ----
        with ExitStack() as p2:
            wopool = p2.enter_context(tc.tile_pool(name="wo", bufs=1))
            wo_sb = [wopool.tile([HD, HQ, DIM], F8, tag=f"wo{t}", name=f"wo{t}")
                     for t in range(2)]
            wo_dma_emitted = [False]

            def emit_wo_dmas():
                if not wo_dma_emitted[0]:
                    wo_dma_emitted[0] = True
                    nc.sync.dma_start(
                        wo_sb[0][:], woh[:].rearrange("p (k m) -> p k m", k=HQ))
                    nc.sync.dma_start(
                        wo_sb[1][:], wol[:].rearrange("p (k m) -> p k m", k=HQ))

            otpool = p2.enter_context(tc.tile_pool(name="ot", bufs=1))
            # attention output per head, fp8 hi/lo split for the o_proj
            ot8 = [otpool.tile([HD, HQ, SEQ], F8, tag=f"ot8{t}", name=f"ot8{t}")
                   for t in range(2)]

            mpool = p2.enter_context(tc.tile_pool(name="mk", bufs=1))
            spool = p2.enter_context(tc.tile_pool(name="sp", bufs=3))

            mk_sb = None
            if mask_mode == "causal":
                mk_sb = mpool.tile([HD, 4, 512], BF16, tag="mkd")
                nc.sync.dma_start(
                    mk_sb[:], mkt[:].rearrange("p (k m) -> p k m", k=4))

            gen_masks = {}

            def emit_gen_masks(qt):
                qs = slice(qt * 512, (qt + 1) * 512)
                out = {}
                for kst in range(16):
                    m = mpool.tile([HD, 512], F32, tag=f"mk{kst}",
                                   name=f"mk{kst}")
                    nc.sync.dma_start(
                        m[:], mkt[kst * HD:(kst + 1) * HD, qs])
                    out[kst] = m
                return out

            def npair_of(qt):
                return 2 * qt if mask_mode == "causal" else 8

            def nunit_of(qt):
                return npair_of(qt) + (4 if mask_mode == "causal" else 0)

            def issue_scores_for(qt, h, i, ps2):
                npair = npair_of(qt)
                qs = slice(qt * 512, (qt + 1) * 512)
                sp = ps2.tile([HD, 1024], F32, tag="pst")
                if i < npair:
                    for u in range(2):
                        kst = 2 * i + u
                        nc.tensor.matmul(
                            sp[:, u * 512:(u + 1) * 512],
                            kr_at(kst),
                            qrope[h][qt][:],
                            start=True, stop=True)
                else:
                    # diagonal tile, columns < c0 fully masked
                    r = i - npair
                    kst = 4 * qt + r
                    c0 = r * HD
                    nc.tensor.matmul(
                        sp[:, c0:512],
                        kr_at(kst),
                        qrope[h][qt][:, c0:512],
                        start=True, stop=True)
                return sp

            def issue_exp_for(qt, i, sp, ppool):
                npair = npair_of(qt)
                pb = ppool.tile([HD, 1024], BF16, tag="pexp")
                if i < npair:
                    if mask_mode == "general":
                        tmp = ppool.tile([HD, 1024], F32, tag="padd", bufs=2)
                        for u in range(2):
                            usl = slice(u * 512, (u + 1) * 512)
                            nc.vector.tensor_add(
                                tmp[:, usl], sp[:, usl],
                                gen_masks[qt][2 * i + u][:])
                        nc.scalar.activation(pb[:], tmp[:], AF.Exp,
                                             bias=ebias[:])
                    else:
                        nc.scalar.activation(pb[:], sp[:], AF.Exp,
                                             bias=ebias[:])
                else:
                    r = i - npair
                    c0 = r * HD
                    tmp = ppool.tile([HD, 1024], F32, tag="padd", bufs=2)
                    nc.vector.tensor_add(
                        tmp[:, c0:512], sp[:, c0:512], mk_sb[:, r, c0:])
                    nc.scalar.activation(pb[:, c0:512], tmp[:, c0:512],
                                         AF.Exp, bias=ebias[:])
                return pb

            pre_store = {}

            def attn_iter(qt, h, ps2, ps2a, ppool, filler,
                          prescore_next=None, lookahead=2, p8pool=None):
                qs = slice(qt * 512, (qt + 1) * 512)
                npair = npair_of(qt)
                nunit = nunit_of(qt)
                sps = [None] * nunit
                pbs = [None] * nunit

                pre = pre_store.pop((qt, h), None)
                if pre is not None:
                    sps[0], sps[1] = pre
                    if lookahead > 2 and nunit > 2:
                        sps[2] = issue_scores_for(qt, h, 2, ps2)
                else:
                    for j in range(min(lookahead, nunit)):
                        sps[j] = issue_scores_for(qt, h, j, ps2)

                den = ps2a.tile([HD, 512], F32, tag="pden")
                otp = ps2a.tile([HD, 512], F32, tag="pot")
                # den_dr: non-diag pair units compute den as one fp8
                # DoubleRow matmul over a Pool-engine fp8 copy of P (lagged
                # one unit to hide the cast latency)
                den_dr = p8pool is not None and npair > 0
                pend = []

                def flush_den_dr(keep=0):
                    while len(pend) > keep:
                        p8t, first = pend.pop(0)
                        nc.tensor.matmul(
                            den[:], ones8[:],
                            p8t[:].rearrange("p (u m) -> p u m", u=2),
                            perf_mode=DR, start=first, stop=False)

                for i in range(nunit):
                    if lookahead + i < nunit and sps[lookahead + i] is None:
                        sps[lookahead + i] = issue_scores_for(
                            qt, h, lookahead + i, ps2)
                    pbs[i] = issue_exp_for(qt, i, sps[i], ppool)
                    fl_last = (i == nunit - 1)
                    if i < npair:
                        if den_dr:
                            p8t = p8pool.tile([HD, 1024], F8, tag="p8")
                            nc.gpsimd.tensor_copy(p8t[:], pbs[i][:])
                        for u in range(2):
                            kst = 2 * i + u
                            fl = dict(
                                start=(i == 0 and u == 0),
                                stop=(fl_last and u == 1))
                            pr = pbs[i][:, u * 512:(u + 1) * 512]
                            if not den_dr:
                                nc.tensor.matmul(
                                    den[:], ones_sb[:], pr, **fl)
                            nc.tensor.matmul(
                                otp[:], vn_at(kst), pr, **fl)
                        if den_dr:
                            flush_den_dr(keep=1)
                            pend.append((p8t, i == 0))
                    else:
                        r = i - npair
                        kst = 4 * qt + r
                        c0 = r * HD
                        if den_dr:
                            flush_den_dr()
                        fl = dict(start=(i == 0), stop=fl_last)
                        pr = pbs[i][:, c0:512]
                        nc.tensor.matmul(
                            den[:, c0:], ones_sb[:], pr,
                            start=(i == 0 and not den_dr), stop=fl_last)
                        nc.tensor.matmul(
                            otp[:, c0:], vn_at(kst), pr, **fl)
                    if fl_last and prescore_next is not None:
                        # pre-issue the next iteration's first two score
                        # units so its exp pipeline starts before this
                        # iteration's DVE drain
                        qn, hn = prescore_next
                        pre_store[(qn, hn)] = (
                            issue_scores_for(qn, hn, 0, ps2),
                            issue_scores_for(qn, hn, 1, ps2))
                    if filler is not None:
                        filler()
                inv = spool.tile([HD, 512], F32, tag="inv")
                nc.vector.reciprocal(inv[:], den[:])
                ots = spool.tile([HD, 512], F32, tag="ots")
                nc.vector.tensor_mul(ots[:], otp[:], inv[:])
                # fp8 hi/lo split of the attention output
                nc.scalar.activation(ot8[0][:, h, qs], ots[:], AF.Copy)
                nc.vector.tensor_sub(ot8[1][:, h, qs], ots[:],
                                     ot8[0][:, h, qs])
                if filler is not None:
                    filler()

            # ---- phase 3 emitter: o_proj (fp8 DoubleRow 3-term), one
            # [128,512] column block per generator step so it can be
            # interleaved into the attention tail as PE filler work ----
            OTERMS = ((0, 0), (1, 0), (0, 1))

            def oproj_units(sts, ps3, opool, egs=None):
                for st in sts:
                    ss = slice(st * 512, (st + 1) * 512)
                    for eg in (range(DIM // HD // 4) if egs is None
                               else egs):
                        last_grp = (st == 0 and eg == DIM // HD // 4 - 1)
                        ocp = opool.tile([HD, 4, 512], BF16, tag="ocp")
                        for ej in range(4):
                            et = eg * 4 + ej
                            esl = slice(et * HD, (et + 1) * HD)
                            po = ps3.tile([HD, 512], F32, tag="po")
                            for pi in range(2):
                                hpair = slice(2 * pi, 2 * pi + 2)
                                for ti, (wi, oi) in enumerate(OTERMS):
                                    nc.tensor.matmul(
                                        po[:],
                                        wo_sb[wi][:, hpair, esl],
                                        ot8[oi][:, hpair, ss],
                                        perf_mode=DR,
                                        start=(pi == 0 and ti == 0),
                                        stop=(pi == 1 and ti == 2),
                                    )
                            osl = ocp[:, ej, :]
                            if ej % 2 == 0:
                                nc.scalar.activation(osl, po[:], AF.Copy,
                                                     scale=OSCALE)
                            else:
                                nc.vector.tensor_scalar_mul(osl, po[:], OSCALE)
                            if last_grp:
                                # final tiles: store per-slice so the last
                                # DMA isn't gated on all four copies
                                nc.sync.dma_start(
                                    outt[et * HD:(et + 1) * HD, ss], osl)
                            yield
                        if not last_grp:
                            nc.sync.dma_start(
                                outt[eg * 4 * HD:(eg + 1) * 4 * HD, ss]
                                .rearrange("(e p) m -> p e m", p=HD),
                                ocp[:])

            if mask_mode == "causal":
                with ExitStack() as patt:
                    ppool = patt.enter_context(tc.tile_pool(name="pp", bufs=5))
                    ps2 = patt.enter_context(
                        tc.tile_pool(name="ps2", bufs=3, space="PSUM"))
                    ps2a = patt.enter_context(
                        tc.tile_pool(name="ps2a", bufs=1, space="PSUM"))
                    emit_wo_dmas()
                    for h in range(HQ):
                        nxt = (3, h + 1) if h + 1 < HQ else None
                        attn_iter(3, h, ps2, ps2a, ppool, None,
                                  prescore_next=nxt)
                # tail: interleave o_proj units into the latency-bound
                # qt=1/qt=0 iterations
                with ExitStack() as ptail:
                    ppool2 = ptail.enter_context(
                        tc.tile_pool(name="pp2", bufs=6))
                    ps2t = ptail.enter_context(
                        tc.tile_pool(name="ps2t", bufs=2, space="PSUM"))
                    ps2a2 = ptail.enter_context(
                        tc.tile_pool(name="ps2a2", bufs=1, space="PSUM"))
                    ps3 = ptail.enter_context(
                        tc.tile_pool(name="ps3", bufs=2, space="PSUM"))
                    opool = ptail.enter_context(
                        tc.tile_pool(name="ostage", bufs=4))
                    gen = oproj_units((3, 2), ps3, opool)
                    # st=3 units (32) are ready once qt=3 is done; st=2
                    # units must wait until all of qt=2 has been emitted
                    pulled = [0]
                    limit = [32]
                    _done = object()

                    def filler_gen():
                        if pulled[0] < limit[0]:
                            if next(gen, _done) is not _done:
                                pulled[0] += 1

                    p8pool = ptail.enter_context(
                        tc.tile_pool(name="p8p", bufs=6))
                    seq = [(qt, h) for qt in (2, 1, 0) for h in range(HQ)]
                    for n, (qt, h) in enumerate(seq[:8]):
                        attn_iter(qt, h, ps2t, ps2a2, ppool2, filler_gen,
                                  prescore_next=seq[n + 1], p8pool=p8pool)
                        if (qt, h) == (2, HQ - 1):
                            limit[0] = 64
                    gen2 = oproj_units((1,), ps3, opool, egs=range(0, 4))

                    def filler_tail():
                        if next(gen, _done) is _done:
                            next(gen2, None)

                    for h in range(HQ):
                        nxt = (0, h + 1) if h + 1 < HQ else None
                        attn_iter(0, h, ps2t, ps2a2, ppool2, filler_tail,
                                  prescore_next=nxt)
                    for _ in gen:
                        pass
                    for _ in gen2:
                        pass
                # bulk o_proj drain with deep PSUM rotation
                with ExitStack() as p3d:
                    ps3d = p3d.enter_context(
                        tc.tile_pool(name="ps3d", bufs=4, space="PSUM"))
                    opool2 = p3d.enter_context(
                        tc.tile_pool(name="ostage2", bufs=3))
                    for _ in oproj_units((1,), ps3d, opool2,
                                         egs=range(4, 8)):
                        pass
                    for _ in oproj_units((0,), ps3d, opool2):
                        pass
            else:
                with ExitStack() as patt:
                    ppool = patt.enter_context(tc.tile_pool(name="pp", bufs=5))
                    ps2 = patt.enter_context(
                        tc.tile_pool(name="ps2", bufs=3, space="PSUM"))
                    ps2a = patt.enter_context(
                        tc.tile_pool(name="ps2a", bufs=1, space="PSUM"))
                    emit_wo_dmas()
                    for qt in range(SQT - 1, -1, -1):
                        if mask_mode == "general" and qt not in gen_masks:
                            gen_masks[qt] = emit_gen_masks(qt)
                        for h in range(HQ):
                            attn_iter(qt, h, ps2, ps2a, ppool, None)
                    pre_store.clear()
                with ExitStack() as p3:
                    ps3 = p3.enter_context(
                        tc.tile_pool(name="ps3", bufs=4, space="PSUM"))
                    opool = p3.enter_context(
                        tc.tile_pool(name="ostage", bufs=3))
                    for _ in oproj_units((3, 2, 1, 0), ps3, opool):
                        pass

    nc.compile()
    return nc


def _split8(a, scale=1.0):
    s = np.clip(a * np.float32(scale), -224.0, 224.0)
    hi = s.astype(E4NP)
    lo = np.clip(s - hi.astype(np.float32), -224.0, 224.0).astype(E4NP)
    return np.ascontiguousarray(hi), np.ascontiguousarray(lo)


def _prep_consts(freqs_cos, freqs_sin):
    cos = np.asarray(freqs_cos, dtype=np.float32)
    sin = np.asarray(freqs_sin, dtype=np.float32)
    C = np.empty((HD, SEQ), np.float32)
    S = np.empty((HD, SEQ), np.float32)
    C[0::2] = cos.T
    C[1::2] = cos.T
    S[0::2] = -sin.T
    S[1::2] = sin.T
    psw = np.zeros((HD, HD), np.float32)
    j = np.arange(0, HD, 2)
    psw[j + 1, j] = 1.0
    psw[j, j + 1] = 1.0
    idn = np.eye(HD, dtype=np.float32).astype(BF16NP)
    return C, S, psw, idn


def _mask_mode(mask):
    if not mask.any():
        return "zeros"
    neg = mask.min()
    tril = np.tril(np.ones((SEQ, SEQ), dtype=bool))
    if neg <= -1e8 and not mask[tril].any() and np.all(mask[~tril] == neg):
        return "causal"
    return "general"


def kernel(x, wq, wk, wv, wo, freqs_cos, freqs_sin, mask, start_pos):
    global LAST_RESULT
    assert int(start_pos) == 0, "kernel hardcodes start_pos=0 (full prefill)"
    x = np.asarray(x, dtype=np.float32)
    wq = np.asarray(wq, dtype=np.float32)
    wk = np.asarray(wk, dtype=np.float32)
    wv = np.asarray(wv, dtype=np.float32)
    wo = np.asarray(wo, dtype=np.float32)
    mask = np.asarray(mask, dtype=np.float32)

    mode = _mask_mode(mask)
    if mode not in _cache:
        _cache[mode] = _build(mode)
    nc = _cache[mode]

    xt = np.ascontiguousarray(x.reshape(SEQ, DIM).T)
    xh8, xl8 = _split8(xt)
    C, S, psw, idn = _prep_consts(freqs_cos, freqs_sin)
    mkt = None
    if mode == "causal":
        # 4 relative diagonal tile masks: tile r is mask.T[r*128:(r+1)*128,
        # 0:512] (the pattern depends only on kst - 4*qt)
        mt = np.ascontiguousarray(mask.T[:512, :512])
        mkt = np.concatenate([mt[r * HD:(r + 1) * HD, :] for r in range(4)],
                             axis=1)
        mkt = np.ascontiguousarray(mkt).astype(BF16NP)
    elif mode == "general":
        mkt = np.ascontiguousarray(mask.T)

    def _ptile(a, m):
        # [DIM_contract, m] -> partition-major [128, (ktile m)]
        k = a.shape[0] // HD
        return np.ascontiguousarray(
            a.reshape(k, HD, m).transpose(1, 0, 2).reshape(HD, k * m))

    in_maps = []
    for c in range(NCORES):
        wqh8, wql8 = _split8(wq[c * DQ:(c + 1) * DQ, :].T, WSCALE)
        wkh8, wkl8 = _split8(wk[c * HD:(c + 1) * HD, :].T, WSCALE)
        wvh8, wvl8 = _split8(wv[c * HD:(c + 1) * HD, :].T, WSCALE)
        woh8, wol8 = _split8(wo[:, c * DQ:(c + 1) * DQ].T, WSCALE)
        wqh8, wql8 = _ptile(wqh8, DQ), _ptile(wql8, DQ)
        wkh8, wkl8 = _ptile(wkh8, HD), _ptile(wkl8, HD)
        wvh8, wvl8 = _ptile(wvh8, HD), _ptile(wvl8, HD)
        woh8, wol8 = _ptile(woh8, DIM), _ptile(wol8, DIM)
        m = {
            "xh": xh8, "xl": xl8,
            "wqh": wqh8, "wql": wql8,
            "wkh": wkh8, "wkl": wkl8,
            "wvh": wvh8, "wvl": wvl8,
            "woh": woh8, "wol": wol8,
            "cs": C.astype(BF16NP), "sn": S.astype(BF16NP),
            "psw": psw, "idn": idn,
        }
        if mkt is not None:
            m["mkt"] = mkt
        in_maps.append(m)

    res = run_bass_kernel_spmd(nc, in_maps, core_ids=list(range(NCORES)),
                               trace=TRACE)
    LAST_RESULT = res
    acc = np.zeros((DIM, SEQ), dtype=np.float64)
    for c in range(NCORES):
        acc += res.results[c]["outt"].astype(np.float64)
    return np.ascontiguousarray(acc.T).astype(np.float32).reshape(1, SEQ, DIM)


# revision 83
# speedup vs baseline: 1.0434x; 1.0003x over previous
"""GQA attention block (QKV proj + RoPE + causal attention + o_proj),
tensor-parallel over heads across 8 TRN2 NeuronCores.

Sharding: core c owns q heads [4c, 4c+4) (512 q dims), kv head c
(128 kv dims), and wo columns [512c, 512c+512). Each core computes a
full-shape partial of the output projection; the host sums the 8
partials (the "all-reduce") and transposes back.

Layout convention on device: activations are kept feature-major
([dim, seq]) so every matmul contracts over the partition axis with
no transposes:
  QT/KT [d, s]  ->  scores^T [ks, qs] = KT_tile^T . QT   (lhsT=KT, rhs=QT)
  softmax over ks = partition axis: exp on ACT, denominator via
  ones-matmul on PE, division folded into the PV output scaling
  PV: OT [dv, qs] = V_nat^T . P                           (lhsT=V, rhs=P)
  o_proj: outT [e, s] = woT^T . OT                        (lhsT=woT, rhs=OT)

Precision plan: the dense GEMMs (QKV proj, o_proj) run as fp8e4
DoubleRow matmuls (2 K-tiles contracted per instruction at 0.5
cycles/row) with a hi/lo residual split of both operands and the
three significant cross terms (hi.hi + lo.hi + hi.lo) accumulated in
fp32 PSUM - ~1.5e-3 relative error at 0.75x the bf16/fp32r cycle
cost. Weights are pre-scaled by 64 (power of two, folded back into
the PSUM->SBUF copy scale) so their hi/lo parts stay in fp8e4 normal
range; the attention output is pre-scaled by 16 (via the den "ones"
stationary = 1/16) for the same reason. q/k/v/P/scores run in bf16
(same PE rate as fp32r, half the SBUF/DMA). Output partials are
stored bf16 and summed on host.

Scheduling: weights arrive host-pretiled partition-major in a few
large staged DMAs (the HWDGE dispatch ring costs 625ns per DMA
instruction); x arrives as host-split fp8 hi/lo streams. The first
two seq chunks stream x quads interleaved with the matmuls (the DMA
pipe is saturated by weight loading there); the last two hold the
full chunk of x resident (prefetched while DMA is otherwise idle)
and run their six output tiles sequentially, each immediately
followed by its RoPE, so the RoPE chains overlap the next tile's
matmuls and attention starts without waiting on a rope tail.
Attention runs qt descending with a 2-unit score lookahead and
cross-head score pre-issue; the latency-bound qt<=2 iterations
interleave o_proj column-block emissions between units as PE filler
(gated so an o_proj chunk is only emitted after the attention chunk
feeding it is complete), with the remaining o_proj drained at deeper
PSUM rotation afterwards. In those interleaved iterations the
softmax denominator of full (non-diagonal) score pairs is computed
as a single fp8 DoubleRow matmul over a Pool-engine fp8 copy of P
(lagged two units to hide the cast), which requires EXP_BIAS to keep
exp outputs inside fp8e4 range.
"""

import sys
from contextlib import ExitStack

import numpy as np
import ml_dtypes

for _p in ("/opt/trn_rl_repo", "/opt/trn_rl_repo/concourse"):
    if _p not in sys.path:
        sys.path.insert(0, _p)

import concourse.bacc as bacc
import concourse.bass as bass
import concourse.tile as tile
from concourse import mybir
from concourse.bass_utils import run_bass_kernel_spmd

F32 = mybir.dt.float32
F32R = mybir.dt.float32r
BF16 = mybir.dt.bfloat16
F8 = mybir.dt.float8e4
E4NP = ml_dtypes.float8_e4m3
BF16NP = ml_dtypes.bfloat16
AF = mybir.ActivationFunctionType
DR = mybir.MatmulPerfMode.DoubleRow

DIM = 4096
SEQ = 2048
HD = 128          # head dim
NCORES = 8
HQ = 4            # q heads per core
DQ = HQ * HD      # 512 q dims per core
NKT = DIM // HD   # 32 contraction tiles
NPAIR = NKT // 2  # 16 DoubleRow k-tile pairs
SQT = SEQ // 512  # 4 seq chunks of 512
INV_SQRT_HD = 1.0 / np.sqrt(np.float32(HD))
EXP_BIAS = -4.0   # constant shift inside exp; cancels in softmax.
                  # -4 keeps exp outputs within fp8e4 normal range
                  # for the DoubleRow denominator path (max logit
                  # ~5.5 -> p <= e^1.5; typical p ~0.02 >> 2^-9)
WSCALE = 64.0     # weight pre-scale so fp8 hi/lo stays in normal range
OTSCALE = 16.0    # attention-output pre-scale for its fp8 hi/lo split

TRACE = False
LAST_RESULT = None

_cache = {}


def _build(mask_mode):
    """mask_mode: 'zeros' | 'causal' | 'general'."""
    nc = bacc.Bacc("TRN2", target_bir_lowering=False)
    xh = nc.dram_tensor("xh", [DIM, SEQ], F8, kind="ExternalInput")
    xl = nc.dram_tensor("xl", [DIM, SEQ], F8, kind="ExternalInput")
    # weights arrive pre-tiled partition-major: [p, (ktile m)]
    wqh = nc.dram_tensor("wqh", [HD, NKT * DQ], F8, kind="ExternalInput")
    wql = nc.dram_tensor("wql", [HD, NKT * DQ], F8, kind="ExternalInput")
    wkh = nc.dram_tensor("wkh", [HD, NKT * HD], F8, kind="ExternalInput")
    wkl = nc.dram_tensor("wkl", [HD, NKT * HD], F8, kind="ExternalInput")
    wvh = nc.dram_tensor("wvh", [HD, NKT * HD], F8, kind="ExternalInput")
    wvl = nc.dram_tensor("wvl", [HD, NKT * HD], F8, kind="ExternalInput")
    woh = nc.dram_tensor("woh", [HD, HQ * DIM], F8, kind="ExternalInput")
    wol = nc.dram_tensor("wol", [HD, HQ * DIM], F8, kind="ExternalInput")
    cs = nc.dram_tensor("cs", [HD, SEQ], BF16, kind="ExternalInput")
    sn = nc.dram_tensor("sn", [HD, SEQ], BF16, kind="ExternalInput")
    psw = nc.dram_tensor("psw", [HD, HD], F32R, kind="ExternalInput")
    idn = nc.dram_tensor("idn", [HD, HD], BF16, kind="ExternalInput")
    mkt = None
    if mask_mode == "causal":
        # 4 relative diagonal-tile masks (pattern repeats for every qt)
        mkt = nc.dram_tensor("mkt", [HD, 4 * 512], BF16, kind="ExternalInput")
    elif mask_mode == "general":
        mkt = nc.dram_tensor("mkt", [SEQ, SEQ], F32, kind="ExternalInput")
    outt = nc.dram_tensor("outt", [DIM, SEQ], BF16, kind="ExternalOutput")

    QSCALE = float(INV_SQRT_HD / WSCALE)
    KSCALE = float(1.0 / WSCALE)
    OSCALE = float(1.0 / (WSCALE * OTSCALE))

    with ExitStack() as ctx:
        tc = ctx.enter_context(tile.TileContext(nc))

        # ---- persistent pools ----
        const = ctx.enter_context(tc.tile_pool(name="const", bufs=1))
        ones_f32 = const.tile([HD, HD], F32, tag="ones32")
        # den is accumulated pre-divided by OTSCALE so inv = OTSCALE/den and
        # the attention output is scaled into fp8-friendly range for the
        # o_proj hi/lo split; the final output copy divides it back out.
        nc.vector.memset(ones_f32[:], 1.0 / OTSCALE)
        ones_sb = const.tile([HD, HD], BF16, tag="ones")
        nc.scalar.activation(ones_sb[:], ones_f32[:], AF.Copy)
        ebias = const.tile([HD, 1], F32, tag="ebias")
        nc.vector.memset(ebias[:], EXP_BIAS)
        ones8 = const.tile([HD, 2, HD], F8, tag="ones8")
        for _u in range(2):
            nc.scalar.activation(ones8[:, _u, :], ones_f32[:], AF.Copy)

        qkvpool = ctx.enter_context(tc.tile_pool(name="qkv", bufs=1))
        # per-chunk tiles so attention reads only depend on the chunks they
        # actually touch (no false whole-tile hazards on the last chunk)
        qrope = [[qkvpool.tile([HD, 512], BF16, tag=f"qr{h}_{c}",
                               name=f"qr{h}_{c}") for c in range(SQT)]
                 for h in range(HQ)]
        krope = [qkvpool.tile([HD, 512], BF16, tag=f"kr{c}", name=f"kr{c}")
                 for c in range(SQT)]
        vnat = [qkvpool.tile([HD, 512], BF16, tag=f"vn{c}", name=f"vn{c}")
                for c in range(SQT)]

        def kr_at(kst):
            return krope[kst // 4][:, (kst % 4) * HD:(kst % 4 + 1) * HD]

        def vn_at(kst):
            return vnat[kst // 4][:, (kst % 4) * HD:(kst % 4 + 1) * HD]

        # ---- phase 1: QKV projection (fp8 DoubleRow 3-term) + RoPE ----
        with ExitStack() as p1:
            wpool = p1.enter_context(tc.tile_pool(name="w1", bufs=1))
            wq_sb = [wpool.tile([HD, NKT, DQ], F8, tag=f"wq{t}", name=f"wq{t}")
                     for t in range(2)]
            wk_sb = [wpool.tile([HD, NKT, HD], F8, tag=f"wk{t}", name=f"wk{t}")
                     for t in range(2)]
            wv_sb = [wpool.tile([HD, NKT, HD], F8, tag=f"wv{t}", name=f"wv{t}")
                     for t in range(2)]
            cs_sb = wpool.tile([HD, SEQ], BF16, tag="cs")
            sn_sb = wpool.tile([HD, SEQ], BF16, tag="sn")
            psw_sb = wpool.tile([HD, HD], F32R, tag="psw")
            idn_sb = wpool.tile([HD, HD], BF16, tag="idn")

            def _wslice(dst3d, dram, m, lo, hi):
                # ktiles [lo, hi) of a [p, (k m)] pretiled weight tensor
                nc.sync.dma_start(
                    dst3d[:, lo:hi, :],
                    dram[:, lo * m:hi * m].rearrange("p (k m) -> p k m",
                                                     k=hi - lo))

            def emit_w_dma(kg):
                # batched staging: kg==0 -> ktiles 0-4 of everything (small,
                # fast first batch); kg==1 -> ktiles 4-16; kg==3 -> 16-32.
                # One DMA instruction per tensor per batch keeps the HWDGE
                # dispatch ring (625ns/instruction) off the critical path.
                def _wbatch(lo, hi):
                    for t in range(2):
                        if not (t == 0 and lo == 0):
                            _wslice(wq_sb[t], (wqh, wql)[t], DQ, lo, hi)
                        _wslice(wk_sb[t], (wkh, wkl)[t], HD, lo, hi)
                        _wslice(wv_sb[t], (wvh, wvl)[t], HD, lo, hi)

                if kg == 0:
                    _wbatch(0, 4)
                elif kg == 1:
                    for t in range(2):
                        _wslice(wq_sb[t], (wqh, wql)[t], DQ, 4, 12)
                        _wslice(wk_sb[t], (wkh, wkl)[t], HD, 4, 16)
                        _wslice(wv_sb[t], (wvh, wvl)[t], HD, 4, 16)
                elif kg == 2:
                    for t in range(2):
                        _wslice(wq_sb[t], (wqh, wql)[t], DQ, 12, 16)
                elif kg == 3:
                    for t in range(2):
                        _wslice(wq_sb[t], (wqh, wql)[t], DQ, 16, NKT)
                elif kg == 4:
                    for t in range(2):
                        _wslice(wk_sb[t], (wkh, wkl)[t], HD, 16, NKT)
                        _wslice(wv_sb[t], (wvh, wvl)[t], HD, 16, NKT)
                elif kg == 5:
                    nc.sync.dma_start(psw_sb[:], psw[:])
                    nc.sync.dma_start(idn_sb[:], idn[:])
                    nc.sync.dma_start(cs_sb[:], cs[:])
                    nc.sync.dma_start(sn_sb[:], sn[:])

            xpool = p1.enter_context(tc.tile_pool(name="xstream", bufs=3))
            xchpool = p1.enter_context(tc.tile_pool(name="xch", bufs=2))
            rtmp = p1.enter_context(tc.tile_pool(name="rtmp", bufs=2))
            ps1 = p1.enter_context(tc.tile_pool(name="ps1", bufs=1, space="PSUM"))
            ps1q = p1.enter_context(tc.tile_pool(name="ps1q", bufs=4, space="PSUM"))
            ps1m = p1.enter_context(tc.tile_pool(name="ps1m", bufs=1, space="PSUM"))

            TERMS = ((0, 0), (1, 0), (0, 1))
            xch = {}

            def emit_xch_dmas(stc):
                # full-chunk x for the sequential chunks, in 8-ktile slices
                sc_ = slice(stc * 512, (stc + 1) * 512)
                tiles = [xchpool.tile([HD, NKT, 512], F8, tag=f"xch{t}",
                                      name=f"xch{t}_{stc}") for t in range(2)]
                for t, xd in ((0, xh), (1, xl)):
                    for g in range(4):
                        nc.sync.dma_start(
                            tiles[t][:, g * 8:(g + 1) * 8, :],
                            xd[g * 8 * HD:(g + 1) * 8 * HD, sc_]
                            .rearrange("(k p) m -> p k m", p=HD))
                xch[stc] = tiles

            for st in range(SQT):
                ss = slice(st * 512, (st + 1) * 512)
                pq = [ps1q.tile([HD, 512], F32, tag="pq", name=f"pq{i}")
                      for i in range(HQ)]
                pk = ps1.tile([HD, 512], F32, tag="pk")
                pv = ps1.tile([HD, 512], F32, tag="pv")

                def rope_one(src_ps, dst, dst_sl, scale, on_act):
                    raw = rtmp.tile([HD, 512], F32R, tag="qraw")
                    if on_act:
                        nc.scalar.activation(raw[:], src_ps[:], AF.Copy,
                                             scale=scale)
                    else:
                        nc.vector.tensor_scalar_mul(raw[:], src_ps[:], scale)
                    swp = ps1m.tile([HD, 512], F32, tag="psw")
                    nc.tensor.matmul(swp[:], psw_sb[:], raw[:],
                                     start=True, stop=True)
                    t1 = rtmp.tile([HD, 512], F32, tag="t1", bufs=1)
                    nc.vector.tensor_mul(t1[:], raw[:], cs_sb[:, ss])
                    t2 = rtmp.tile([HD, 512], F32, tag="t2", bufs=1)
                    nc.vector.tensor_mul(t2[:], swp[:], sn_sb[:, ss])
                    nc.vector.tensor_add(dst[:, dst_sl], t1[:], t2[:])

                def v_block():
                    # v: descale + bf16, then transpose to [seq, dv] blocks
                    vraw = rtmp.tile([HD, 512], BF16, tag="vraw", bufs=1)
                    nc.scalar.activation(vraw[:], pv[:], AF.Copy, scale=KSCALE)
                    for j in range(4):
                        vt = ps1m.tile([HD, HD], BF16, tag="pvt")
                        nc.tensor.transpose(vt[:],
                                            vraw[:, j * HD:(j + 1) * HD],
                                            idn_sb[:])
                        if j % 2 == 0:
                            nc.scalar.activation(
                                vnat[st][:, j * HD:(j + 1) * HD], vt[:],
                                AF.Copy)
                        else:
                            nc.vector.tensor_copy(
                                vnat[st][:, j * HD:(j + 1) * HD], vt[:])

                if st < 2:
                    # streaming chunks: x quads interleaved with the matmuls
                    for kg in range(NKT // 4):
                        if st == 0 and kg == 0:
                            _wslice(wq_sb[0], wqh, DQ, 0, 4)
                        xq8 = [xpool.tile([HD, 4, 512], F8, tag=f"xt{t}",
                                          name=f"xt{t}") for t in range(2)]
                        nc.sync.dma_start(
                            xq8[0][:],
                            xh[kg * 4 * HD:(kg + 1) * 4 * HD, ss]
                            .rearrange("(k p) m -> p k m", p=HD))
                        nc.sync.dma_start(
                            xq8[1][:],
                            xl[kg * 4 * HD:(kg + 1) * 4 * HD, ss]
                            .rearrange("(k p) m -> p k m", p=HD))
                        if st == 0:
                            emit_w_dma(kg)
                        if st == 1 and kg == 4:
                            emit_xch_dmas(2)
                        for j in range(2):
                            pp = kg * 2 + j       # global pair index
                            kpair = slice(2 * pp, 2 * pp + 2)
                            xsl = [x8[:, 2 * j:2 * j + 2, :] for x8 in xq8]
                            first = (kg == 0 and j == 0)
                            last = (kg == NKT // 4 - 1 and j == 1)
                            for ti, (wi, xi) in enumerate(TERMS):
                                fl = dict(start=(first and ti == 0),
                                          stop=(last and ti == 2))
                                for mt in range(HQ):
                                    msl = slice(mt * HD, (mt + 1) * HD)
                                    nc.tensor.matmul(
                                        pq[mt][:], wq_sb[wi][:, kpair, msl],
                                        xsl[xi], perf_mode=DR, **fl)
                                nc.tensor.matmul(
                                    pk[:], wk_sb[wi][:, kpair, :], xsl[xi],
                                    perf_mode=DR, **fl)
                                nc.tensor.matmul(
                                    pv[:], wv_sb[wi][:, kpair, :], xsl[xi],
                                    perf_mode=DR, **fl)
                    for mt in range(HQ):
                        rope_one(pq[mt], qrope[mt][st], slice(0, 512), QSCALE,
                                 mt % 2 == 0)
                    rope_one(pk, krope[st], slice(0, 512), KSCALE, True)
                    v_block()
                else:
                    # sequential chunks: full-chunk x already resident;
                    # each output tile immediately runs its RoPE so the
                    # chains overlap the next tile's matmuls
                    if st == 2:
                        emit_xch_dmas(3)
                    xt8 = xch.pop(st)

                    def seq_accum(ps, wsb, msl):
                        for ppi in range(NPAIR):
                            kpair = slice(2 * ppi, 2 * ppi + 2)
                            for ti, (wi, xi) in enumerate(TERMS):
                                lhs = (wsb[wi][:, kpair, msl] if msl
                                       else wsb[wi][:, kpair, :])
                                nc.tensor.matmul(
                                    ps[:], lhs, xt8[xi][:, kpair, :],
                                    perf_mode=DR,
                                    start=(ppi == 0 and ti == 0),
                                    stop=(ppi == NPAIR - 1 and ti == 2))

                    for mt in range(HQ):
                        seq_accum(pq[mt], wq_sb, slice(mt * HD, (mt + 1) * HD))
                        rope_one(pq[mt], qrope[mt][st], slice(0, 512), QSCALE,
                                 mt % 2 == 0)
                    seq_accum(pk, wk_sb, None)
                    rope_one(pk, krope[st], slice(0, 512), KSCALE, True)
                    seq_accum(pv, wv_sb, None)
                    v_block()

        # ---- phase 2: attention;  phase 3: output projection ----
        with ExitStack() as p2:
            wopool = p2.enter_context(tc.tile_pool(name="wo", bufs=1))
            wo_sb = [wopool.tile([HD, HQ, DIM], F8, tag=f"wo{t}", name=f"wo{t}")
                     for t in range(2)]
            wo_dma_emitted = [False]

            def emit_wo_dmas():
                if not wo_dma_emitted[0]:
                    wo_dma_emitted[0] = True
                    nc.sync.dma_start(
                        wo_sb[0][:], woh[:].rearrange("p (k m) -> p k m", k=HQ))
                    nc.sync.dma_start(
                        wo_sb[1][:], wol[:].rearrange("p (k m) -> p k m", k=HQ))

            otpool = p2.enter_context(tc.tile_pool(name="ot", bufs=1))
            # attention output per head, fp8 hi/lo split for the o_proj
            ot8 = [otpool.tile([HD, HQ, SEQ], F8, tag=f"ot8{t}", name=f"ot8{t}")
                   for t in range(2)]

            mpool = p2.enter_context(tc.tile_pool(name="mk", bufs=1))
            spool = p2.enter_context(tc.tile_pool(name="sp", bufs=3))

            mk_sb = None
            if mask_mode == "causal":
                mk_sb = mpool.tile([HD, 4, 512], BF16, tag="mkd")
                nc.sync.dma_start(
                    mk_sb[:], mkt[:].rearrange("p (k m) -> p k m", k=4))

            gen_masks = {}

            def emit_gen_masks(qt):
                qs = slice(qt * 512, (qt + 1) * 512)
                out = {}
                for kst in range(16):
                    m = mpool.tile([HD, 512], F32, tag=f"mk{kst}",
                                   name=f"mk{kst}")
                    nc.sync.dma_start(
                        m[:], mkt[kst * HD:(kst + 1) * HD, qs])
                    out[kst] = m
                return out

            def npair_of(qt):
                return 2 * qt if mask_mode == "causal" else 8

            def nunit_of(qt):
                return npair_of(qt) + (4 if mask_mode == "causal" else 0)

            def issue_scores_for(qt, h, i, ps2):
                npair = npair_of(qt)
                qs = slice(qt * 512, (qt + 1) * 512)
                sp = ps2.tile([HD, 1024], F32, tag="pst")
                if i < npair:
                    for u in range(2):
                        kst = 2 * i + u
                        nc.tensor.matmul(
                            sp[:, u * 512:(u + 1) * 512],
                            kr_at(kst),
                            qrope[h][qt][:],
                            start=True, stop=True)
                else:
                    # diagonal tile, columns < c0 fully masked
                    r = i - npair
                    kst = 4 * qt + r
                    c0 = r * HD
                    nc.tensor.matmul(
                        sp[:, c0:512],
                        kr_at(kst),
                        qrope[h][qt][:, c0:512],
                        start=True, stop=True)
                return sp

            def issue_exp_for(qt, i, sp, ppool):
                npair = npair_of(qt)
                pb = ppool.tile([HD, 1024], BF16, tag="pexp")
                if i < npair:
                    if mask_mode == "general":
                        tmp = ppool.tile([HD, 1024], F32, tag="padd", bufs=2)
                        for u in range(2):
                            usl = slice(u * 512, (u + 1) * 512)
                            nc.vector.tensor_add(
                                tmp[:, usl], sp[:, usl],
                                gen_masks[qt][2 * i + u][:])
                        nc.scalar.activation(pb[:], tmp[:], AF.Exp,
                                             bias=ebias[:])
                    else:
                        nc.scalar.activation(pb[:], sp[:], AF.Exp,
                                             bias=ebias[:])
                else:
                    r = i - npair
                    c0 = r * HD
                    tmp = ppool.tile([HD, 1024], F32, tag="padd", bufs=2)
                    nc.vector.tensor_add(
                        tmp[:, c0:512], sp[:, c0:512], mk_sb[:, r, c0:])
                    nc.scalar.activation(pb[:, c0:512], tmp[:, c0:512],
                                         AF.Exp, bias=ebias[:])
                return pb

            pre_store = {}

            def attn_iter(qt, h, ps2, ps2a, ppool, filler,
                          prescore_next=None, lookahead=2, p8pool=None):
                qs = slice(qt * 512, (qt + 1) * 512)
                npair = npair_of(qt)
                nunit = nunit_of(qt)
                sps = [None] * nunit
                pbs = [None] * nunit

                pre = pre_store.pop((qt, h), None)
                if pre is not None:
                    sps[0], sps[1] = pre
                    if lookahead > 2 and nunit > 2:
                        sps[2] = issue_scores_for(qt, h, 2, ps2)
                else:
                    for j in range(min(lookahead, nunit)):
                        sps[j] = issue_scores_for(qt, h, j, ps2)

                den = ps2a.tile([HD, 512], F32, tag="pden")
                otp = ps2a.tile([HD, 512], F32, tag="pot")
                # den_dr: non-diag pair units compute den as one fp8
                # DoubleRow matmul over a Pool-engine fp8 copy of P (lagged
                # one unit to hide the cast latency)
                den_dr = p8pool is not None and npair > 0
                pend = []

                def flush_den_dr(keep=0):
                    while len(pend) > keep:
                        p8t, first = pend.pop(0)
                        nc.tensor.matmul(
                            den[:], ones8[:],
                            p8t[:].rearrange("p (u m) -> p u m", u=2),
                            perf_mode=DR, start=first, stop=False)

                for i in range(nunit):
                    if lookahead + i < nunit and sps[lookahead + i] is None:
                        sps[lookahead + i] = issue_scores_for(
                            qt, h, lookahead + i, ps2)
                    pbs[i] = issue_exp_for(qt, i, sps[i], ppool)
                    fl_last = (i == nunit - 1)
                    if i < npair:
                        if den_dr:
                            p8t = p8pool.tile([HD, 1024], F8, tag="p8")
                            nc.gpsimd.tensor_copy(p8t[:], pbs[i][:])
                        for u in range(2):
                            kst = 2 * i + u
                            fl = dict(
                                start=(i == 0 and u == 0),
                                stop=(fl_last and u == 1))
                            pr = pbs[i][:, u * 512:(u + 1) * 512]
                            if not den_dr:
                                nc.tensor.matmul(
                                    den[:], ones_sb[:], pr, **fl)
                            nc.tensor.matmul(
                                otp[:], vn_at(kst), pr, **fl)
                        if den_dr:
                            flush_den_dr(keep=1)
                            pend.append((p8t, i == 0))
                    else:
                        r = i - npair
                        kst = 4 * qt + r
                        c0 = r * HD
                        if den_dr:
                            flush_den_dr()
                        fl = dict(start=(i == 0), stop=fl_last)
                        pr = pbs[i][:, c0:512]
                        nc.tensor.matmul(
                            den[:, c0:], ones_sb[:], pr,
                            start=(i == 0 and not den_dr), stop=fl_last)
                        nc.tensor.matmul(
                            otp[:, c0:], vn_at(kst), pr, **fl)
                    if fl_last and prescore_next is not None:
                        # pre-issue the next iteration's first two score
                        # units so its exp pipeline starts before this
                        # iteration's DVE drain
                        qn, hn = prescore_next
                        pre_store[(qn, hn)] = (
                            issue_scores_for(qn, hn, 0, ps2),
                            issue_scores_for(qn, hn, 1, ps2))
                    if filler is not None:
                        filler()
                inv = spool.tile([HD, 512], F32, tag="inv")
                nc.vector.reciprocal(inv[:], den[:])
                ots = spool.tile([HD, 512], F32, tag="ots")
                nc.vector.tensor_mul(ots[:], otp[:], inv[:])
                # fp8 hi/lo split of the attention output
                nc.scalar.activation(ot8[0][:, h, qs], ots[:], AF.Copy)
                nc.vector.tensor_sub(ot8[1][:, h, qs], ots[:],
                                     ot8[0][:, h, qs])
                if filler is not None:
                    filler()

            # ---- phase 3 emitter: o_proj (fp8 DoubleRow 3-term), one
            # [128,512] column block per generator step so it can be
            # interleaved into the attention tail as PE filler work ----
            OTERMS = ((0, 0), (1, 0), (0, 1))

            def oproj_units(sts, ps3, opool, egs=None):
                for st in sts:
                    ss = slice(st * 512, (st + 1) * 512)
                    for eg in (range(DIM // HD // 4) if egs is None
                               else egs):
                        last_grp = (st == 0 and eg == DIM // HD // 4 - 1)
                        ocp = opool.tile([HD, 4, 512], BF16, tag="ocp")
                        for ej in range(4):
                            et = eg * 4 + ej
                            esl = slice(et * HD, (et + 1) * HD)
                            po = ps3.tile([HD, 512], F32, tag="po")
                            for pi in range(2):
                                hpair = slice(2 * pi, 2 * pi + 2)
                                for ti, (wi, oi) in enumerate(OTERMS):
                                    nc.tensor.matmul(
                                        po[:],
                                        wo_sb[wi][:, hpair, esl],
                                        ot8[oi][:, hpair, ss],
                                        perf_mode=DR,
                                        start=(pi == 0 and ti == 0),
                                        stop=(pi == 1 and ti == 2),
                                    )
                            osl = ocp[:, ej, :]
                            if ej % 2 == 0:
                                nc.scalar.activation(osl, po[:], AF.Copy,
                                                     scale=OSCALE)
                            else:
                                nc.vector.tensor_scalar_mul(osl, po[:], OSCALE)
                            if last_grp:
                                # final tiles: store per-slice so the last
                                # DMA isn't gated on all four copies
                                nc.sync.dma_start(
                                    outt[et * HD:(et + 1) * HD, ss], osl)
                            yield
                        if not last_grp:
                            nc.sync.dma_start(
                                outt[eg * 4 * HD:(eg + 1) * 4 * HD, ss]
                                .rearrange("(e p) m -> p e m", p=HD),
                                ocp[:])

            if mask_mode == "causal":
                with ExitStack() as patt:
                    ppool = patt.enter_context(tc.tile_pool(name="pp", bufs=5))
                    ps2 = patt.enter_context(
                        tc.tile_pool(name="ps2", bufs=3, space="PSUM"))
                    ps2a = patt.enter_context(
                        tc.tile_pool(name="ps2a", bufs=1, space="PSUM"))
                    emit_wo_dmas()
                    for h in range(HQ):
                        nxt = (3, h + 1) if h + 1 < HQ else None
                        attn_iter(3, h, ps2, ps2a, ppool, None,
                                  prescore_next=nxt)
                # tail: interleave o_proj units into the latency-bound
                # qt=1/qt=0 iterations
                with ExitStack() as ptail:
                    ppool2 = ptail.enter_context(
                        tc.tile_pool(name="pp2", bufs=6))
                    ps2t = ptail.enter_context(
                        tc.tile_pool(name="ps2t", bufs=2, space="PSUM"))
                    ps2a2 = ptail.enter_context(
                        tc.tile_pool(name="ps2a2", bufs=1, space="PSUM"))
                    ps3 = ptail.enter_context(
                        tc.tile_pool(name="ps3", bufs=2, space="PSUM"))
                    opool = ptail.enter_context(
                        tc.tile_pool(name="ostage", bufs=4))
                    gen = oproj_units((3, 2), ps3, opool)
                    # st=3 units (32) are ready once qt=3 is done; st=2
                    # units must wait until all of qt=2 has been emitted
                    pulled = [0]
                    limit = [32]
                    _done = object()

                    def filler_gen():
                        if pulled[0] < limit[0]:
                            if next(gen, _done) is not _done:
                                pulled[0] += 1

                    p8pool = ptail.enter_context(
                        tc.tile_pool(name="p8p", bufs=6))
                    seq = [(qt, h) for qt in (2, 1, 0) for h in range(HQ)]
                    for n, (qt, h) in enumerate(seq[:8]):
                        attn_iter(qt, h, ps2t, ps2a2, ppool2, filler_gen,
                                  prescore_next=seq[n + 1], p8pool=p8pool)
                        if (qt, h) == (2, HQ - 1):
                            limit[0] = 64
                    gen2 = oproj_units((1,), ps3, opool, egs=range(0, 4))

                    def filler_tail():
                        if next(gen, _done) is _done:
                            next(gen2, None)

                    for h in range(HQ):
                        nxt = (0, h + 1) if h + 1 < HQ else None
                        attn_iter(0, h, ps2t, ps2a2, ppool2, filler_tail,
                                  prescore_next=nxt)
                    for _ in gen:
                        pass
                    for _ in gen2:
                        pass
                # bulk o_proj drain with deep PSUM rotation
                with ExitStack() as p3d:
                    ps3d = p3d.enter_context(
                        tc.tile_pool(name="ps3d", bufs=4, space="PSUM"))
                    opool2 = p3d.enter_context(
                        tc.tile_pool(name="ostage2", bufs=3))
                    for _ in oproj_units((1,), ps3d, opool2,
                                         egs=range(4, 8)):
                        pass
                    for _ in oproj_units((0,), ps3d, opool2):
                        pass
            else:
                with ExitStack() as patt:
                    ppool = patt.enter_context(tc.tile_pool(name="pp", bufs=5))
                    ps2 = patt.enter_context(
                        tc.tile_pool(name="ps2", bufs=3, space="PSUM"))
                    ps2a = patt.enter_context(
                        tc.tile_pool(name="ps2a", bufs=1, space="PSUM"))
                    emit_wo_dmas()
                    for qt in range(SQT - 1, -1, -1):
                        if mask_mode == "general" and qt not in gen_masks:
                            gen_masks[qt] = emit_gen_masks(qt)
                        for h in range(HQ):
                            attn_iter(qt, h, ps2, ps2a, ppool, None)
                    pre_store.clear()
                with ExitStack() as p3:
                    ps3 = p3.enter_context(
                        tc.tile_pool(name="ps3", bufs=4, space="PSUM"))
                    opool = p3.enter_context(
                        tc.tile_pool(name="ostage", bufs=3))
                    for _ in oproj_units((3, 2, 1, 0), ps3, opool):
                        pass

    nc.compile()
    return nc


def _split8(a, scale=1.0):
    s = np.clip(a * np.float32(scale), -224.0, 224.0)
    hi = s.astype(E4NP)
    lo = np.clip(s - hi.astype(np.float32), -224.0, 224.0).astype(E4NP)
    return np.ascontiguousarray(hi), np.ascontiguousarray(lo)


def _prep_consts(freqs_cos, freqs_sin):
    cos = np.asarray(freqs_cos, dtype=np.float32)
    sin = np.asarray(freqs_sin, dtype=np.float32)
    C = np.empty((HD, SEQ), np.float32)
    S = np.empty((HD, SEQ), np.float32)
    C[0::2] = cos.T
    C[1::2] = cos.T
    S[0::2] = -sin.T
    S[1::2] = sin.T
    psw = np.zeros((HD, HD), np.float32)
    j = np.arange(0, HD, 2)
    psw[j + 1, j] = 1.0
    psw[j, j + 1] = 1.0
    idn = np.eye(HD, dtype=np.float32).astype(BF16NP)
    return C, S, psw, idn


def _mask_mode(mask):
    if not mask.any():
        return "zeros"
    neg = mask.min()
    tril = np.tril(np.ones((SEQ, SEQ), dtype=bool))
    if neg <= -1e8 and not mask[tril].any() and np.all(mask[~tril] == neg):
        return "causal"
    return "general"


def kernel(x, wq, wk, wv, wo, freqs_cos, freqs_sin, mask, start_pos):
    global LAST_RESULT
    assert int(start_pos) == 0, "kernel hardcodes start_pos=0 (full prefill)"
    x = np.asarray(x, dtype=np.float32)
    wq = np.asarray(wq, dtype=np.float32)
    wk = np.asarray(wk, dtype=np.float32)
    wv = np.asarray(wv, dtype=np.float32)
    wo = np.asarray(wo, dtype=np.float32)
    mask = np.asarray(mask, dtype=np.float32)

    mode = _mask_mode(mask)
    if mode not in _cache:
        _cache[mode] = _build(mode)
    nc = _cache[mode]

    xt = np.ascontiguousarray(x.reshape(SEQ, DIM).T)
    xh8, xl8 = _split8(xt)
    C, S, psw, idn = _prep_consts(freqs_cos, freqs_sin)
    mkt = None
    if mode == "causal":
        # 4 relative diagonal tile masks: tile r is mask.T[r*128:(r+1)*128,
        # 0:512] (the pattern depends only on kst - 4*qt)
        mt = np.ascontiguousarray(mask.T[:512, :512])
        mkt = np.concatenate([mt[r * HD:(r + 1) * HD, :] for r in range(4)],
                             axis=1)
        mkt = np.ascontiguousarray(mkt).astype(BF16NP)
    elif mode == "general":
        mkt = np.ascontiguousarray(mask.T)

    def _ptile(a, m):
        # [DIM_contract, m] -> partition-major [128, (ktile m)]
        k = a.shape[0] // HD
        return np.ascontiguousarray(
            a.reshape(k, HD, m).transpose(1, 0, 2).reshape(HD, k * m))

    in_maps = []
    for c in range(NCORES):
        wqh8, wql8 = _split8(wq[c * DQ:(c + 1) * DQ, :].T, WSCALE)
        wkh8, wkl8 = _split8(wk[c * HD:(c + 1) * HD, :].T, WSCALE)
        wvh8, wvl8 = _split8(wv[c * HD:(c + 1) * HD, :].T, WSCALE)
        woh8, wol8 = _split8(wo[:, c * DQ:(c + 1) * DQ].T, WSCALE)
        wqh8, wql8 = _ptile(wqh8, DQ), _ptile(wql8, DQ)
        wkh8, wkl8 = _ptile(wkh8, HD), _ptile(wkl8, HD)
        wvh8, wvl8 = _ptile(wvh8, HD), _ptile(wvl8, HD)
        woh8, wol8 = _ptile(woh8, DIM), _ptile(wol8, DIM)
        m = {
            "xh": xh8, "xl": xl8,
            "wqh": wqh8, "wql": wql8,
            "wkh": wkh8, "wkl": wkl8,
            "wvh": wvh8, "wvl": wvl8,
            "woh": woh8, "wol": wol8,
            "cs": C.astype(BF16NP), "sn": S.astype(BF16NP),
            "psw": psw, "idn": idn,
        }
        if mkt is not None:
            m["mkt"] = mkt
        in_maps.append(m)

    res = run_bass_kernel_spmd(nc, in_maps, core_ids=list(range(NCORES)),
                               trace=TRACE)
    LAST_RESULT = res
    acc = np.zeros((DIM, SEQ), dtype=np.float64)
    for c in range(NCORES):
        acc += res.results[c]["outt"].astype(np.float64)
    return np.ascontiguousarray(acc.T).astype(np.float32).reshape(1, SEQ, DIM)


# revision 84
# speedup vs baseline: 1.0563x; 1.0124x over previous
"""GQA attention block (QKV proj + RoPE + causal attention + o_proj),
tensor-parallel over heads across 8 TRN2 NeuronCores.

Sharding: core c owns q heads [4c, 4c+4) (512 q dims), kv head c
(128 kv dims), and wo columns [512c, 512c+512). Each core computes a
full-shape partial of the output projection; the host sums the 8
partials (the "all-reduce") and transposes back.

Layout convention on device: activations are kept feature-major
([dim, seq]) so every matmul contracts over the partition axis with
no transposes:
  QT/KT [d, s]  ->  scores^T [ks, qs] = KT_tile^T . QT   (lhsT=KT, rhs=QT)
  softmax over ks = partition axis: exp on ACT, denominator via
  ones-matmul on PE, division folded into the PV output scaling
  PV: OT [dv, qs] = V_nat^T . P                           (lhsT=V, rhs=P)
  o_proj: outT [e, s] = woT^T . OT                        (lhsT=woT, rhs=OT)

Precision plan: the dense GEMMs (QKV proj, o_proj) run as fp8e4
DoubleRow matmuls (2 K-tiles contracted per instruction at 0.5
cycles/row) with a hi/lo residual split of both operands and the
three significant cross terms (hi.hi + lo.hi + hi.lo) accumulated in
fp32 PSUM - ~1.5e-3 relative error at 0.75x the bf16/fp32r cycle
cost. Weights are pre-scaled by 64 (power of two, folded back into
the PSUM->SBUF copy scale) so their hi/lo parts stay in fp8e4 normal
range; the attention output is pre-scaled by 16 (via the den "ones"
stationary = 1/16) for the same reason. q/k/v/P/scores run in bf16
(same PE rate as fp32r, half the SBUF/DMA). Output partials are
stored bf16 and summed on host.

Scheduling: weights arrive host-pretiled partition-major in a few
large staged DMAs (the HWDGE dispatch ring costs 625ns per DMA
instruction); x arrives as host-split fp8 hi/lo streams. The first
two seq chunks stream x quads interleaved with the matmuls (the DMA
pipe is saturated by weight loading there); the last two hold the
full chunk of x resident (prefetched while DMA is otherwise idle)
and run their six output tiles sequentially, each immediately
followed by its RoPE, so the RoPE chains overlap the next tile's
matmuls and attention starts without waiting on a rope tail.
Attention runs qt descending with a 2-unit score lookahead and
cross-head score pre-issue; the latency-bound qt<=2 iterations
interleave o_proj column-block emissions between units as PE filler
(gated so an o_proj chunk is only emitted after the attention chunk
feeding it is complete), with the remaining o_proj drained at deeper
PSUM rotation afterwards. In those interleaved iterations the
softmax denominator of full (non-diagonal) score pairs is computed
as a single fp8 DoubleRow matmul over a Pool-engine fp8 copy of P
(lagged two units to hide the cast), which requires EXP_BIAS to keep
exp outputs inside fp8e4 range.
"""

import sys
from contextlib import ExitStack

import numpy as np
import ml_dtypes

for _p in ("/opt/trn_rl_repo", "/opt/trn_rl_repo/concourse"):
    if _p not in sys.path:
        sys.path.insert(0, _p)

import concourse.bacc as bacc
import concourse.bass as bass
import concourse.tile as tile
from concourse import mybir
from concourse.bass_utils import run_bass_kernel_spmd

F32 = mybir.dt.float32
F32R = mybir.dt.float32r
BF16 = mybir.dt.bfloat16
F8 = mybir.dt.float8e4
E4NP = ml_dtypes.float8_e4m3
BF16NP = ml_dtypes.bfloat16
AF = mybir.ActivationFunctionType
DR = mybir.MatmulPerfMode.DoubleRow

DIM = 4096
SEQ = 2048
HD = 128          # head dim
NCORES = 8
HQ = 4            # q heads per core
DQ = HQ * HD      # 512 q dims per core
NKT = DIM // HD   # 32 contraction tiles
NPAIR = NKT // 2  # 16 DoubleRow k-tile pairs
SQT = SEQ // 512  # 4 seq chunks of 512
INV_SQRT_HD = 1.0 / np.sqrt(np.float32(HD))
EXP_BIAS = -4.0   # constant shift inside exp; cancels in softmax.
                  # -4 keeps exp outputs within fp8e4 normal range
                  # for the DoubleRow denominator path (max logit
                  # ~5.5 -> p <= e^1.5; typical p ~0.02 >> 2^-9)
WSCALE = 64.0     # weight pre-scale so fp8 hi/lo stays in normal range
OTSCALE = 16.0    # attention-output pre-scale for its fp8 hi/lo split

TRACE = False
LAST_RESULT = None

_cache = {}


def _build(mask_mode):
    """mask_mode: 'zeros' | 'causal' | 'general'."""
    nc = bacc.Bacc("TRN2", target_bir_lowering=False)
    xh = nc.dram_tensor("xh", [DIM, SEQ], F8, kind="ExternalInput")
    xl = nc.dram_tensor("xl", [DIM, SEQ], F8, kind="ExternalInput")
    # weights arrive pre-tiled partition-major: [p, (ktile m)]
    wqh = nc.dram_tensor("wqh", [HD, NKT * DQ], F8, kind="ExternalInput")
    wql = nc.dram_tensor("wql", [HD, NKT * DQ], F8, kind="ExternalInput")
    wkh = nc.dram_tensor("wkh", [HD, NKT * HD], F8, kind="ExternalInput")
    wkl = nc.dram_tensor("wkl", [HD, NKT * HD], F8, kind="ExternalInput")
    wvh = nc.dram_tensor("wvh", [HD, NKT * HD], F8, kind="ExternalInput")
    wvl = nc.dram_tensor("wvl", [HD, NKT * HD], F8, kind="ExternalInput")
    woh = nc.dram_tensor("woh", [HD, HQ * DIM], F8, kind="ExternalInput")
    wol = nc.dram_tensor("wol", [HD, HQ * DIM], F8, kind="ExternalInput")
    cs = nc.dram_tensor("cs", [HD, SEQ], BF16, kind="ExternalInput")
    sn = nc.dram_tensor("sn", [HD, SEQ], BF16, kind="ExternalInput")
    psw = nc.dram_tensor("psw", [HD, HD], F32R, kind="ExternalInput")
    idn = nc.dram_tensor("idn", [HD, HD], BF16, kind="ExternalInput")
    mkt = None
    if mask_mode == "causal":
        # 4 relative diagonal-tile masks (pattern repeats for every qt)
        mkt = nc.dram_tensor("mkt", [HD, 4 * 512], BF16, kind="ExternalInput")
    elif mask_mode == "general":
        mkt = nc.dram_tensor("mkt", [SEQ, SEQ], F32, kind="ExternalInput")
    outt = nc.dram_tensor("outt", [DIM, SEQ], BF16, kind="ExternalOutput")

    QSCALE = float(INV_SQRT_HD / WSCALE)
    KSCALE = float(1.0 / WSCALE)
    OSCALE = float(1.0 / (WSCALE * OTSCALE))

    with ExitStack() as ctx:
        tc = ctx.enter_context(tile.TileContext(nc))

        # ---- persistent pools ----
        const = ctx.enter_context(tc.tile_pool(name="const", bufs=1))
        ones_f32 = const.tile([HD, HD], F32, tag="ones32")
        # den is accumulated pre-divided by OTSCALE so inv = OTSCALE/den and
        # the attention output is scaled into fp8-friendly range for the
        # o_proj hi/lo split; the final output copy divides it back out.
        nc.vector.memset(ones_f32[:], 1.0 / OTSCALE)
        ones_sb = const.tile([HD, HD], BF16, tag="ones")
        nc.scalar.activation(ones_sb[:], ones_f32[:], AF.Copy)
        ebias = const.tile([HD, 1], F32, tag="ebias")
        nc.vector.memset(ebias[:], EXP_BIAS)
        ones8 = const.tile([HD, 2, HD], F8, tag="ones8")
        for _u in range(2):
            nc.scalar.activation(ones8[:, _u, :], ones_f32[:], AF.Copy)

        qkvpool = ctx.enter_context(tc.tile_pool(name="qkv", bufs=1))
        # per-chunk tiles so attention reads only depend on the chunks they
        # actually touch (no false whole-tile hazards on the last chunk)
        qrope = [[qkvpool.tile([HD, 512], BF16, tag=f"qr{h}_{c}",
                               name=f"qr{h}_{c}") for c in range(SQT)]
                 for h in range(HQ)]
        krope = [qkvpool.tile([HD, 512], BF16, tag=f"kr{c}", name=f"kr{c}")
                 for c in range(SQT)]
        vnat = [qkvpool.tile([HD, 512], BF16, tag=f"vn{c}", name=f"vn{c}")
                for c in range(SQT)]

        def kr_at(kst):
            return krope[kst // 4][:, (kst % 4) * HD:(kst % 4 + 1) * HD]

        def vn_at(kst):
            return vnat[kst // 4][:, (kst % 4) * HD:(kst % 4 + 1) * HD]

        # ---- phase 1: QKV projection (fp8 DoubleRow 3-term) + RoPE ----
        with ExitStack() as p1:
            wpool = p1.enter_context(tc.tile_pool(name="w1", bufs=1))
            wq_sb = [wpool.tile([HD, NKT, DQ], F8, tag=f"wq{t}", name=f"wq{t}")
                     for t in range(2)]
            wk_sb = [wpool.tile([HD, NKT, HD], F8, tag=f"wk{t}", name=f"wk{t}")
                     for t in range(2)]
            wv_sb = [wpool.tile([HD, NKT, HD], F8, tag=f"wv{t}", name=f"wv{t}")
                     for t in range(2)]
            cs_sb = wpool.tile([HD, SEQ], BF16, tag="cs")
            sn_sb = wpool.tile([HD, SEQ], BF16, tag="sn")
            psw_sb = wpool.tile([HD, HD], F32R, tag="psw")
            idn_sb = wpool.tile([HD, HD], BF16, tag="idn")

            def _wslice(dst3d, dram, m, lo, hi):
                # ktiles [lo, hi) of a [p, (k m)] pretiled weight tensor
                nc.sync.dma_start(
                    dst3d[:, lo:hi, :],
                    dram[:, lo * m:hi * m].rearrange("p (k m) -> p k m",
                                                     k=hi - lo))

            def emit_w_dma(kg):
                # batched staging: kg==0 -> ktiles 0-4 of everything (small,
                # fast first batch); kg==1 -> ktiles 4-16; kg==3 -> 16-32.
                # One DMA instruction per tensor per batch keeps the HWDGE
                # dispatch ring (625ns/instruction) off the critical path.
                def _wbatch(lo, hi):
                    for t in range(2):
                        if not (t == 0 and lo == 0):
                            _wslice(wq_sb[t], (wqh, wql)[t], DQ, lo, hi)
                        _wslice(wk_sb[t], (wkh, wkl)[t], HD, lo, hi)
                        _wslice(wv_sb[t], (wvh, wvl)[t], HD, lo, hi)

                if kg == 0:
                    _wbatch(0, 4)
                elif kg == 1:
                    for t in range(2):
                        _wslice(wq_sb[t], (wqh, wql)[t], DQ, 4, 12)
                        _wslice(wk_sb[t], (wkh, wkl)[t], HD, 4, 16)
                        _wslice(wv_sb[t], (wvh, wvl)[t], HD, 4, 16)
                elif kg == 2:
                    for t in range(2):
                        _wslice(wq_sb[t], (wqh, wql)[t], DQ, 12, 16)
                elif kg == 3:
                    for t in range(2):
                        _wslice(wq_sb[t], (wqh, wql)[t], DQ, 16, NKT)
                elif kg == 4:
                    for t in range(2):
                        _wslice(wk_sb[t], (wkh, wkl)[t], HD, 16, NKT)
                        _wslice(wv_sb[t], (wvh, wvl)[t], HD, 16, NKT)
                elif kg == 5:
                    nc.sync.dma_start(psw_sb[:], psw[:])
                    nc.sync.dma_start(idn_sb[:], idn[:])
                    nc.sync.dma_start(cs_sb[:], cs[:])
                    nc.sync.dma_start(sn_sb[:], sn[:])

            xpool = p1.enter_context(tc.tile_pool(name="xstream", bufs=3))
            xchpool = p1.enter_context(tc.tile_pool(name="xch", bufs=2))
            rtmp = p1.enter_context(tc.tile_pool(name="rtmp", bufs=2))
            ps1 = p1.enter_context(tc.tile_pool(name="ps1", bufs=1, space="PSUM"))
            ps1q = p1.enter_context(tc.tile_pool(name="ps1q", bufs=4, space="PSUM"))
            ps1m = p1.enter_context(tc.tile_pool(name="ps1m", bufs=1, space="PSUM"))

            TERMS = ((0, 0), (1, 0), (0, 1))
            xch = {}

            def emit_xch_dmas(stc):
                # full-chunk x for the sequential chunks, in 8-ktile slices
                sc_ = slice(stc * 512, (stc + 1) * 512)
                tiles = [xchpool.tile([HD, NKT, 512], F8, tag=f"xch{t}",
                                      name=f"xch{t}_{stc}") for t in range(2)]
                for t, xd in ((0, xh), (1, xl)):
                    for g in range(4):
                        nc.sync.dma_start(
                            tiles[t][:, g * 8:(g + 1) * 8, :],
                            xd[g * 8 * HD:(g + 1) * 8 * HD, sc_]
                            .rearrange("(k p) m -> p k m", p=HD))
                xch[stc] = tiles

            for st in range(SQT):
                ss = slice(st * 512, (st + 1) * 512)
                pq = [ps1q.tile([HD, 512], F32, tag="pq", name=f"pq{i}")
                      for i in range(HQ)]
                pk = ps1.tile([HD, 512], F32, tag="pk")
                pv = ps1.tile([HD, 512], F32, tag="pv")

                def rope_one(src_ps, dst, dst_sl, scale, on_act):
                    raw = rtmp.tile([HD, 512], F32R, tag="qraw")
                    if on_act:
                        nc.scalar.activation(raw[:], src_ps[:], AF.Copy,
                                             scale=scale)
                    else:
                        nc.vector.tensor_scalar_mul(raw[:], src_ps[:], scale)
                    swp = ps1m.tile([HD, 512], F32, tag="psw")
                    nc.tensor.matmul(swp[:], psw_sb[:], raw[:],
                                     start=True, stop=True)
                    t1 = rtmp.tile([HD, 512], F32, tag="t1", bufs=1)
                    nc.vector.tensor_mul(t1[:], raw[:], cs_sb[:, ss])
                    t2 = rtmp.tile([HD, 512], F32, tag="t2", bufs=1)
                    nc.vector.tensor_mul(t2[:], swp[:], sn_sb[:, ss])
                    nc.vector.tensor_add(dst[:, dst_sl], t1[:], t2[:])

                def v_block():
                    # v: descale + bf16, then transpose to [seq, dv] blocks
                    vraw = rtmp.tile([HD, 512], BF16, tag="vraw", bufs=1)
                    nc.scalar.activation(vraw[:], pv[:], AF.Copy, scale=KSCALE)
                    for j in range(4):
                        vt = ps1m.tile([HD, HD], BF16, tag="pvt")
                        nc.tensor.transpose(vt[:],
                                            vraw[:, j * HD:(j + 1) * HD],
                                            idn_sb[:])
                        if j % 2 == 0:
                            nc.scalar.activation(
                                vnat[st][:, j * HD:(j + 1) * HD], vt[:],
                                AF.Copy)
                        else:
                            nc.vector.tensor_copy(
                                vnat[st][:, j * HD:(j + 1) * HD], vt[:])

                if st < 2:
                    # streaming chunks: x quads interleaved with the matmuls
                    for kg in range(NKT // 4):
                        if st == 0 and kg == 0:
                            _wslice(wq_sb[0], wqh, DQ, 0, 4)
                        xq8 = [xpool.tile([HD, 4, 512], F8, tag=f"xt{t}",
                                          name=f"xt{t}") for t in range(2)]
                        nc.sync.dma_start(
                            xq8[0][:],
                            xh[kg * 4 * HD:(kg + 1) * 4 * HD, ss]
                            .rearrange("(k p) m -> p k m", p=HD))
                        nc.sync.dma_start(
                            xq8[1][:],
                            xl[kg * 4 * HD:(kg + 1) * 4 * HD, ss]
                            .rearrange("(k p) m -> p k m", p=HD))
                        if st == 0:
                            emit_w_dma(kg)
                        if st == 1 and kg == 4:
                            emit_xch_dmas(2)
                        for j in range(2):
                            pp = kg * 2 + j       # global pair index
                            kpair = slice(2 * pp, 2 * pp + 2)
                            xsl = [x8[:, 2 * j:2 * j + 2, :] for x8 in xq8]
                            first = (kg == 0 and j == 0)
                            last = (kg == NKT // 4 - 1 and j == 1)
                            for ti, (wi, xi) in enumerate(TERMS):
                                fl = dict(start=(first and ti == 0),
                                          stop=(last and ti == 2))
                                for mt in range(HQ):
                                    msl = slice(mt * HD, (mt + 1) * HD)
                                    nc.tensor.matmul(
                                        pq[mt][:], wq_sb[wi][:, kpair, msl],
                                        xsl[xi], perf_mode=DR, **fl)
                                nc.tensor.matmul(
                                    pk[:], wk_sb[wi][:, kpair, :], xsl[xi],
                                    perf_mode=DR, **fl)
                                nc.tensor.matmul(
                                    pv[:], wv_sb[wi][:, kpair, :], xsl[xi],
                                    perf_mode=DR, **fl)
                    for mt in range(HQ):
                        rope_one(pq[mt], qrope[mt][st], slice(0, 512), QSCALE,
                                 mt % 2 == 0)
                    rope_one(pk, krope[st], slice(0, 512), KSCALE, True)
                    v_block()
                else:
                    # sequential chunks: full-chunk x already resident;
                    # each output tile immediately runs its RoPE so the
                    # chains overlap the next tile's matmuls
                    if st == 2:
                        emit_xch_dmas(3)
                    xt8 = xch.pop(st)

                    def seq_accum(ps, wsb, msl):
                        for ppi in range(NPAIR):
                            kpair = slice(2 * ppi, 2 * ppi + 2)
                            for ti, (wi, xi) in enumerate(TERMS):
                                lhs = (wsb[wi][:, kpair, msl] if msl
                                       else wsb[wi][:, kpair, :])
                                nc.tensor.matmul(
                                    ps[:], lhs, xt8[xi][:, kpair, :],
                                    perf_mode=DR,
                                    start=(ppi == 0 and ti == 0),
                                    stop=(ppi == NPAIR - 1 and ti == 2))

                    for mt in range(HQ):
                        seq_accum(pq[mt], wq_sb, slice(mt * HD, (mt + 1) * HD))
                        rope_one(pq[mt], qrope[mt][st], slice(0, 512), QSCALE,
                                 mt % 2 == 0)
                    seq_accum(pk, wk_sb, None)
                    rope_one(pk, krope[st], slice(0, 512), KSCALE, True)
                    seq_accum(pv, wv_sb, None)
                    v_block()

        # ---- phase 2: attention;  phase 3: output projection ----
        with ExitStack() as p2:
            wopool = p2.enter_context(tc.tile_pool(name="wo", bufs=1))
            wo_sb = [wopool.tile([HD, HQ, DIM], F8, tag=f"wo{t}", name=f"wo{t}")
                     for t in range(2)]
            wo_dma_emitted = [False]

            def emit_wo_dmas():
                if not wo_dma_emitted[0]:
                    wo_dma_emitted[0] = True
                    nc.sync.dma_start(
                        wo_sb[0][:], woh[:].rearrange("p (k m) -> p k m", k=HQ))
                    nc.sync.dma_start(
                        wo_sb[1][:], wol[:].rearrange("p (k m) -> p k m", k=HQ))

            otpool = p2.enter_context(tc.tile_pool(name="ot", bufs=1))
            # attention output per head, fp8 hi/lo split for the o_proj
            ot8 = [otpool.tile([HD, HQ, SEQ], F8, tag=f"ot8{t}", name=f"ot8{t}")
                   for t in range(2)]

            mpool = p2.enter_context(tc.tile_pool(name="mk", bufs=1))
            spool = p2.enter_context(tc.tile_pool(name="sp", bufs=3))

            mk_sb = None
            if mask_mode == "causal":
                mk_sb = mpool.tile([HD, 4, 512], BF16, tag="mkd")
                nc.sync.dma_start(
                    mk_sb[:], mkt[:].rearrange("p (k m) -> p k m", k=4))

            gen_masks = {}

            def emit_gen_masks(qt):
                qs = slice(qt * 512, (qt + 1) * 512)
                out = {}
                for kst in range(16):
                    m = mpool.tile([HD, 512], F32, tag=f"mk{kst}",
                                   name=f"mk{kst}")
                    nc.sync.dma_start(
                        m[:], mkt[kst * HD:(kst + 1) * HD, qs])
                    out[kst] = m
                return out

            def npair_of(qt):
                return 2 * qt if mask_mode == "causal" else 8

            def nunit_of(qt):
                return npair_of(qt) + (4 if mask_mode == "causal" else 0)

            def issue_scores_for(qt, h, i, ps2):
                npair = npair_of(qt)
                qs = slice(qt * 512, (qt + 1) * 512)
                sp = ps2.tile([HD, 1024], F32, tag="pst")
                if i < npair:
                    for u in range(2):
                        kst = 2 * i + u
                        nc.tensor.matmul(
                            sp[:, u * 512:(u + 1) * 512],
                            kr_at(kst),
                            qrope[h][qt][:],
                            start=True, stop=True)
                else:
                    # diagonal tile, columns < c0 fully masked
                    r = i - npair
                    kst = 4 * qt + r
                    c0 = r * HD
                    nc.tensor.matmul(
                        sp[:, c0:512],
                        kr_at(kst),
                        qrope[h][qt][:, c0:512],
                        start=True, stop=True)
                return sp

            def issue_exp_for(qt, i, sp, ppool):
                npair = npair_of(qt)
                pb = ppool.tile([HD, 1024], BF16, tag="pexp")
                if i < npair:
                    if mask_mode == "general":
                        tmp = ppool.tile([HD, 1024], F32, tag="padd", bufs=2)
                        for u in range(2):
                            usl = slice(u * 512, (u + 1) * 512)
                            nc.vector.tensor_add(
                                tmp[:, usl], sp[:, usl],
                                gen_masks[qt][2 * i + u][:])
                        nc.scalar.activation(pb[:], tmp[:], AF.Exp,
                                             bias=ebias[:])
                    else:
                        nc.scalar.activation(pb[:], sp[:], AF.Exp,
                                             bias=ebias[:])
                else:
                    r = i - npair
                    c0 = r * HD
                    tmp = ppool.tile([HD, 1024], F32, tag="padd", bufs=2)
                    nc.vector.tensor_add(
                        tmp[:, c0:512], sp[:, c0:512], mk_sb[:, r, c0:])
                    nc.scalar.activation(pb[:, c0:512], tmp[:, c0:512],
                                         AF.Exp, bias=ebias[:])
                return pb

            pre_store = {}

            def attn_iter(qt, h, ps2, ps2a, ppool, filler,
                          prescore_next=None, lookahead=2, p8pool=None):
                qs = slice(qt * 512, (qt + 1) * 512)
                npair = npair_of(qt)
                nunit = nunit_of(qt)
                sps = [None] * nunit
                pbs = [None] * nunit

                pre = pre_store.pop((qt, h), None)
                if pre is not None:
                    sps[0], sps[1] = pre
                    if lookahead > 2 and nunit > 2:
                        sps[2] = issue_scores_for(qt, h, 2, ps2)
                else:
                    for j in range(min(lookahead, nunit)):
                        sps[j] = issue_scores_for(qt, h, j, ps2)

                den = ps2a.tile([HD, 512], F32, tag="pden")
                otp = ps2a.tile([HD, 512], F32, tag="pot")
                # den_dr: non-diag pair units compute den as one fp8
                # DoubleRow matmul over a Pool-engine fp8 copy of P (lagged
                # one unit to hide the cast latency)
                den_dr = p8pool is not None and npair > 0
                pend = []

                def flush_den_dr(keep=0):
                    while len(pend) > keep:
                        p8t, first = pend.pop(0)
                        nc.tensor.matmul(
                            den[:], ones8[:],
                            p8t[:].rearrange("p (u m) -> p u m", u=2),
                            perf_mode=DR, start=first, stop=False)

                for i in range(nunit):
                    if lookahead + i < nunit and sps[lookahead + i] is None:
                        sps[lookahead + i] = issue_scores_for(
                            qt, h, lookahead + i, ps2)
                    pbs[i] = issue_exp_for(qt, i, sps[i], ppool)
                    fl_last = (i == nunit - 1)
                    if i < npair:
                        if den_dr:
                            p8t = p8pool.tile([HD, 1024], F8, tag="p8")
                            eng = nc.gpsimd if i % 2 == 0 else nc.vector
                            eng.tensor_copy(p8t[:], pbs[i][:])
                        for u in range(2):
                            kst = 2 * i + u
                            fl = dict(
                                start=(i == 0 and u == 0),
                                stop=(fl_last and u == 1))
                            pr = pbs[i][:, u * 512:(u + 1) * 512]
                            if not den_dr:
                                nc.tensor.matmul(
                                    den[:], ones_sb[:], pr, **fl)
                            nc.tensor.matmul(
                                otp[:], vn_at(kst), pr, **fl)
                        if den_dr:
                            flush_den_dr(keep=1)
                            pend.append((p8t, i == 0))
                    else:
                        r = i - npair
                        kst = 4 * qt + r
                        c0 = r * HD
                        if den_dr:
                            flush_den_dr()
                        fl = dict(start=(i == 0), stop=fl_last)
                        pr = pbs[i][:, c0:512]
                        nc.tensor.matmul(
                            den[:, c0:], ones_sb[:], pr,
                            start=(i == 0 and not den_dr), stop=fl_last)
                        nc.tensor.matmul(
                            otp[:, c0:], vn_at(kst), pr, **fl)
                    if fl_last and prescore_next is not None:
                        # pre-issue the next iteration's first two score
                        # units so its exp pipeline starts before this
                        # iteration's DVE drain
                        qn, hn = prescore_next
                        pre_store[(qn, hn)] = (
                            issue_scores_for(qn, hn, 0, ps2),
                            issue_scores_for(qn, hn, 1, ps2))
                    if filler is not None:
                        filler()
                inv = spool.tile([HD, 512], F32, tag="inv")
                nc.vector.reciprocal(inv[:], den[:])
                ots = spool.tile([HD, 512], F32, tag="ots")
                nc.vector.tensor_mul(ots[:], otp[:], inv[:])
                # fp8 hi/lo split of the attention output
                nc.scalar.activation(ot8[0][:, h, qs], ots[:], AF.Copy)
                nc.vector.tensor_sub(ot8[1][:, h, qs], ots[:],
                                     ot8[0][:, h, qs])
                if filler is not None:
                    filler()

            # ---- phase 3 emitter: o_proj (fp8 DoubleRow 3-term), one
            # [128,512] column block per generator step so it can be
            # interleaved into the attention tail as PE filler work ----
            OTERMS = ((0, 0), (1, 0), (0, 1))

            def oproj_units(sts, ps3, opool, egs=None):
                for st in sts:
                    ss = slice(st * 512, (st + 1) * 512)
                    for eg in (range(DIM // HD // 4) if egs is None
                               else egs):
                        last_grp = (st == 0 and eg == DIM // HD // 4 - 1)
                        ocp = opool.tile([HD, 4, 512], BF16, tag="ocp")
                        for ej in range(4):
                            et = eg * 4 + ej
                            esl = slice(et * HD, (et + 1) * HD)
                            po = ps3.tile([HD, 512], F32, tag="po")
                            for pi in range(2):
                                hpair = slice(2 * pi, 2 * pi + 2)
                                for ti, (wi, oi) in enumerate(OTERMS):
                                    nc.tensor.matmul(
                                        po[:],
                                        wo_sb[wi][:, hpair, esl],
                                        ot8[oi][:, hpair, ss],
                                        perf_mode=DR,
                                        start=(pi == 0 and ti == 0),
                                        stop=(pi == 1 and ti == 2),
                                    )
                            osl = ocp[:, ej, :]
                            if ej % 2 == 0:
                                nc.scalar.activation(osl, po[:], AF.Copy,
                                                     scale=OSCALE)
                            else:
                                nc.vector.tensor_scalar_mul(osl, po[:], OSCALE)
                            if last_grp:
                                # final tiles: store per-slice so the last
                                # DMA isn't gated on all four copies
                                nc.sync.dma_start(
                                    outt[et * HD:(et + 1) * HD, ss], osl)
                            yield
                        if not last_grp:
                            nc.sync.dma_start(
                                outt[eg * 4 * HD:(eg + 1) * 4 * HD, ss]
                                .rearrange("(e p) m -> p e m", p=HD),
                                ocp[:])

            if mask_mode == "causal":
                p8pool = p2.enter_context(tc.tile_pool(name="p8p", bufs=6))
                with ExitStack() as patt:
                    ppool = patt.enter_context(tc.tile_pool(name="pp", bufs=5))
                    ps2 = patt.enter_context(
                        tc.tile_pool(name="ps2", bufs=3, space="PSUM"))
                    ps2a = patt.enter_context(
                        tc.tile_pool(name="ps2a", bufs=1, space="PSUM"))
                    emit_wo_dmas()
                    for h in range(HQ):
                        nxt = (3, h + 1) if h + 1 < HQ else None
                        attn_iter(3, h, ps2, ps2a, ppool, None,
                                  prescore_next=nxt, p8pool=p8pool)
                # tail: interleave o_proj units into the latency-bound
                # qt=1/qt=0 iterations
                with ExitStack() as ptail:
                    ppool2 = ptail.enter_context(
                        tc.tile_pool(name="pp2", bufs=6))
                    ps2t = ptail.enter_context(
                        tc.tile_pool(name="ps2t", bufs=2, space="PSUM"))
                    ps2a2 = ptail.enter_context(
                        tc.tile_pool(name="ps2a2", bufs=1, space="PSUM"))
                    ps3 = ptail.enter_context(
                        tc.tile_pool(name="ps3", bufs=2, space="PSUM"))
                    opool = ptail.enter_context(
                        tc.tile_pool(name="ostage", bufs=4))
                    gen = oproj_units((3, 2), ps3, opool)
                    # st=3 units (32) are ready once qt=3 is done; st=2
                    # units must wait until all of qt=2 has been emitted
                    pulled = [0]
                    limit = [32]
                    _done = object()

                    def filler_gen():
                        if pulled[0] < limit[0]:
                            if next(gen, _done) is not _done:
                                pulled[0] += 1

                    seq = [(qt, h) for qt in (2, 1, 0) for h in range(HQ)]
                    for n, (qt, h) in enumerate(seq[:8]):
                        attn_iter(qt, h, ps2t, ps2a2, ppool2, filler_gen,
                                  prescore_next=seq[n + 1], p8pool=p8pool)
                        if (qt, h) == (2, HQ - 1):
                            limit[0] = 64
                    gen2 = oproj_units((1,), ps3, opool, egs=range(0, 4))

                    def filler_tail():
                        if next(gen, _done) is _done:
                            next(gen2, None)

                    for h in range(HQ):
                        nxt = (0, h + 1) if h + 1 < HQ else None
                        attn_iter(0, h, ps2t, ps2a2, ppool2, filler_tail,
                                  prescore_next=nxt)
                    for _ in gen:
                        pass
                    for _ in gen2:
                        pass
                # bulk o_proj drain with deep PSUM rotation
                with ExitStack() as p3d:
                    ps3d = p3d.enter_context(
                        tc.tile_pool(name="ps3d", bufs=4, space="PSUM"))
                    opool2 = p3d.enter_context(
                        tc.tile_pool(name="ostage2", bufs=3))
                    for _ in oproj_units((1,), ps3d, opool2,
                                         egs=range(4, 8)):
                        pass
                    for _ in oproj_units((0,), ps3d, opool2):
                        pass
            else:
                with ExitStack() as patt:
                    ppool = patt.enter_context(tc.tile_pool(name="pp", bufs=5))
                    ps2 = patt.enter_context(
                        tc.tile_pool(name="ps2", bufs=3, space="PSUM"))
                    ps2a = patt.enter_context(
                        tc.tile_pool(name="ps2a", bufs=1, space="PSUM"))
                    emit_wo_dmas()
                    for qt in range(SQT - 1, -1, -1):
                        if mask_mode == "general" and qt not in gen_masks:
                            gen_masks[qt] = emit_gen_masks(qt)
                        for h in range(HQ):
                            attn_iter(qt, h, ps2, ps2a, ppool, None)
                    pre_store.clear()
                with ExitStack() as p3:
                    ps3 = p3.enter_context(
                        tc.tile_pool(name="ps3", bufs=4, space="PSUM"))
                    opool = p3.enter_context(
                        tc.tile_pool(name="ostage", bufs=3))
                    for _ in oproj_units((3, 2, 1, 0), ps3, opool):
                        pass

    nc.compile()
    return nc


def _split8(a, scale=1.0):
    s = np.clip(a * np.float32(scale), -224.0, 224.0)
    hi = s.astype(E4NP)
    lo = np.clip(s - hi.astype(np.float32), -224.0, 224.0).astype(E4NP)
    return np.ascontiguousarray(hi), np.ascontiguousarray(lo)


def _prep_consts(freqs_cos, freqs_sin):
    cos = np.asarray(freqs_cos, dtype=np.float32)
    sin = np.asarray(freqs_sin, dtype=np.float32)
    C = np.empty((HD, SEQ), np.float32)
    S = np.empty((HD, SEQ), np.float32)
    C[0::2] = cos.T
    C[1::2] = cos.T
    S[0::2] = -sin.T
    S[1::2] = sin.T
    psw = np.zeros((HD, HD), np.float32)
    j = np.arange(0, HD, 2)
    psw[j + 1, j] = 1.0
    psw[j, j + 1] = 1.0
    idn = np.eye(HD, dtype=np.float32).astype(BF16NP)
    return C, S, psw, idn


def _mask_mode(mask):
    if not mask.any():
        return "zeros"
    neg = mask.min()
    tril = np.tril(np.ones((SEQ, SEQ), dtype=bool))
    if neg <= -1e8 and not mask[tril].any() and np.all(mask[~tril] == neg):
        return "causal"
    return "general"


def kernel(x, wq, wk, wv, wo, freqs_cos, freqs_sin, mask, start_pos):
    global LAST_RESULT
    assert int(start_pos) == 0, "kernel hardcodes start_pos=0 (full prefill)"
    x = np.asarray(x, dtype=np.float32)
    wq = np.asarray(wq, dtype=np.float32)
    wk = np.asarray(wk, dtype=np.float32)
    wv = np.asarray(wv, dtype=np.float32)
    wo = np.asarray(wo, dtype=np.float32)
    mask = np.asarray(mask, dtype=np.float32)

    mode = _mask_mode(mask)
    if mode not in _cache:
        _cache[mode] = _build(mode)
    nc = _cache[mode]

    xt = np.ascontiguousarray(x.reshape(SEQ, DIM).T)
    xh8, xl8 = _split8(xt)
    C, S, psw, idn = _prep_consts(freqs_cos, freqs_sin)
    mkt = None
    if mode == "causal":
        # 4 relative diagonal tile masks: tile r is mask.T[r*128:(r+1)*128,
        # 0:512] (the pattern depends only on kst - 4*qt)
        mt = np.ascontiguousarray(mask.T[:512, :512])
        mkt = np.concatenate([mt[r * HD:(r + 1) * HD, :] for r in range(4)],
                             axis=1)
        mkt = np.ascontiguousarray(mkt).astype(BF16NP)
    elif mode == "general":
        mkt = np.ascontiguousarray(mask.T)

    def _ptile(a, m):
        # [DIM_contract, m] -> partition-major [128, (ktile m)]
        k = a.shape[0] // HD
        return np.ascontiguousarray(
            a.reshape(k, HD, m).transpose(1, 0, 2).reshape(HD, k * m))

    in_maps = []
    for c in range(NCORES):
        wqh8, wql8 = _split8(wq[c * DQ:(c + 1) * DQ, :].T, WSCALE)
        wkh8, wkl8 = _split8(wk[c * HD:(c + 1) * HD, :].T, WSCALE)
        wvh8, wvl8 = _split8(wv[c * HD:(c + 1) * HD, :].T, WSCALE)
        woh8, wol8 = _split8(wo[:, c * DQ:(c + 1) * DQ].T, WSCALE)
        wqh8, wql8 = _ptile(wqh8, DQ), _ptile(wql8, DQ)
        wkh8, wkl8 = _ptile(wkh8, HD), _ptile(wkl8, HD)
        wvh8, wvl8 = _ptile(wvh8, HD), _ptile(wvl8, HD)
        woh8, wol8 = _ptile(woh8, DIM), _ptile(wol8, DIM)
        m = {
            "xh": xh8, "xl": xl8,
            "wqh": wqh8, "wql": wql8,
            "wkh": wkh8, "wkl": wkl8,
            "wvh": wvh8, "wvl": wvl8,
            "woh": woh8, "wol": wol8,
            "cs": C.astype(BF16NP), "sn": S.astype(BF16NP),
            "psw": psw, "idn": idn,
        }
        if mkt is not None:
            m["mkt"] = mkt
        in_maps.append(m)

    res = run_bass_kernel_spmd(nc, in_maps, core_ids=list(range(NCORES)),
                               trace=TRACE)
    LAST_RESULT = res
    acc = np.zeros((DIM, SEQ), dtype=np.float64)
    for c in range(NCORES):
        acc += res.results[c]["outt"].astype(np.float64)
    return np.ascontiguousarray(acc.T).astype(np.float32).reshape(1, SEQ, DIM)
